# revision 1
# baseline (speedup 1.0000x reference)
"""GATv2-based CGNN forward pass on 8 Trainium2 NeuronCores.

Strategy (dst-node sharded, no collectives):
  - Each core owns N/8 destination nodes. Host buckets edges (incl. self
    loops) by dst core, then by 128-node dst chunk within the core.
  - Dense phase (on device, replicated): xl/xr feature tables
    [node, 260] fp16 where cols 0:256 are the per-head GAT features and
    cols 256:260 carry beta = 0.6 * (feat @ att) per head (the linear
    part of att.lrelu, since lrelu(z) = 0.6 z + 0.4 |z|).
  - Edge phase: batched indirect-DMA gather of xl[src] rows, one-hot
    matmuls reconstruct xr[dst] per edge and scatter-add per-chunk
    aggregates in PSUM.  logits = beta_l[src]+beta_r[dst] + sum(0.4*att*|z|)
    via tensor_tensor_reduce; exp on ScalarE; messages weighted on DVE.
  - Finish: per-chunk softmax normalization, head mean, relu, classifier.
Pad edges have all-zero one-hot columns so they contribute nothing.
"""

import os
import sys

import numpy as np
import ml_dtypes

for _p in ("/opt/trn_rl_repo",):
    if _p not in sys.path and os.path.isdir(_p):
        sys.path.insert(0, _p)

import concourse.bass as bass
import concourse.tile as tile
from concourse import bacc, mybir
from concourse.bass_utils import run_bass_kernel_spmd

FP16 = mybir.dt.float16
FP32 = mybir.dt.float32
INT32 = mybir.dt.int32
AF = mybir.ActivationFunctionType
ALU = mybir.AluOpType

P = 128
HID = 64
HEADS = 4
OUT_DIM = 16
IN_DIM = 256
FEAT = HEADS * HID          # 256
ROW = FEAT + HEADS          # 260 = features + beta columns
NEG = 0.2                   # leaky relu slope

f16 = ml_dtypes.float16 if hasattr(ml_dtypes, "float16") else np.float16


def _cdiv(a, b):
    return (a + b - 1) // b


# ----------------------------------------------------------------------------
# Device program
# ----------------------------------------------------------------------------

TROW = 384                      # padded table row (768B, 256B-aligned)
LO_ROWS = 32768                 # int16 index range per gather table


def build_program(n_nodes_pad, npc_dense, npc_chunks, t_lo, t_hi, n_cores):
    """Build the SPMD Bass program.

    n_nodes_pad: all-node count padded to 512 (dense phase A, groups of 4 tiles)
    npc_dense: per-core dst nodes padded to 512 (phase B loop/table rows)
    npc_chunks: per-core dst nodes padded to 128 (edge-phase chunk count)
    t_lo/t_hi: edge tiles per chunk whose src is in the lo/hi gather table
    """
    NB = 4                        # node tiles per dense group
    GA = n_nodes_pad // (NB * P)  # phase A groups
    GB = npc_dense // (NB * P)
    C_CHUNKS = npc_chunks // P
    t_ch = t_lo + t_hi
    hi_rows = max(n_nodes_pad - LO_ROWS, P)

    nc = bacc.Bacc("TRN2", target_bir_lowering=False, debug=False,
                   num_devices=n_cores)

    def din(name, shape, dtype=FP16):
        return nc.dram_tensor(name, shape, dtype, kind="ExternalInput").ap()

    # --- external inputs (host prepared) ---
    xg_all = din("xg_all", [GA, P, 2, NB * P])          # x.T swizzled, all nodes
    xg_own = din("xg_own", [GB, P, 2, NB * P])          # x.T swizzled, own nodes
    w_in_a = din("w_in_a", [P, HID])
    w_in_b = din("w_in_b", [P, HID])
    wl260 = din("wl260", [HID, ROW])
    ql260 = din("ql260", [HID, ROW])
    bl260 = din("bl260", [1, ROW])
    wr260 = din("wr260", [HID, ROW])
    qr260 = din("qr260", [HID, ROW])
    br260 = din("br260", [1, ROW])
    att04 = din("att04", [P, FEAT])                     # 0.4*att row replicated
    ident = din("ident", [P, P])
    ident32 = din("ident32", [P, P], FP32)
    ones64 = din("ones64", [HID, 1])
    ones1_64 = din("ones1_64", [1, HID])
    ones1_128 = din("ones1_128", [1, P])
    ones1_512 = din("ones1_512", [1, NB * P])
    b_in_col = din("b_in_col", [1, HID])
    gbias_rep = din("gbias_rep", [P, HID], FP32)
    w_cls = din("w_cls", [HID, OUT_DIM])
    bcls_row = din("bcls_row", [1, OUT_DIM])
    idx_lo = din("idx_lo", [P, C_CHUNKS * t_lo * 8], mybir.dt.int16)
    idx_hi = (din("idx_hi", [P, C_CHUNKS * t_hi * 8], mybir.dt.int16)
              if t_hi else None)
    onehot_t = din("onehot_t", [C_CHUNKS, P, t_ch * P])   # [n, e] node-major
    onehot_e = din("onehot_e", [C_CHUNKS, P, t_ch * P])   # [e, n] edge-major

    out_ext = nc.dram_tensor("out", [npc_chunks, OUT_DIM], FP32,
                             kind="ExternalOutput").ap()

    # --- internal DRAM tables ---
    xl_lo_tab = nc.dram_tensor("xl_lo_tab", [min(n_nodes_pad, LO_ROWS), TROW],
                               FP16).ap()
    xl_hi_tab = nc.dram_tensor("xl_hi_tab", [hi_rows, TROW], FP16).ap()
    xr_tab = nc.dram_tensor("xr_tab", [npc_dense, ROW], FP16).ap()

    with tile.TileContext(nc) as tc:
        cpool = tc.tile_pool(name="consts", bufs=1)
        with cpool as cp:
            w_in_a_sb = cp.tile([P, HID], FP16)
            nc.sync.dma_start(w_in_a_sb[:], w_in_a[:])
            w_in_b_sb = cp.tile([P, HID], FP16)
            nc.sync.dma_start(w_in_b_sb[:], w_in_b[:])
            wl_sb = cp.tile([HID, ROW], FP16)
            nc.sync.dma_start(wl_sb[:], wl260[:])
            ql_sb = cp.tile([HID, ROW], FP16)
            nc.sync.dma_start(ql_sb[:], ql260[:])
            bl_sb = cp.tile([1, ROW], FP16)
            nc.sync.dma_start(bl_sb[:], bl260[:])
            wr_sb = cp.tile([HID, ROW], FP16)
            nc.sync.dma_start(wr_sb[:], wr260[:])
            qr_sb = cp.tile([HID, ROW], FP16)
            nc.sync.dma_start(qr_sb[:], qr260[:])
            br_sb = cp.tile([1, ROW], FP16)
            nc.sync.dma_start(br_sb[:], br260[:])
            att_sb = cp.tile([P, FEAT], FP16)
            nc.sync.dma_start(att_sb[:], att04[:])
            id_sb = cp.tile([P, P], FP16)
            nc.sync.dma_start(id_sb[:], ident[:])
            id32_sb = cp.tile([P, P], FP32)
            nc.sync.dma_start(id32_sb[:], ident32[:])
            ones64_sb = cp.tile([HID, 1], FP16)
            nc.sync.dma_start(ones64_sb[:], ones64[:])
            o1_64_sb = cp.tile([1, HID], FP16)
            nc.sync.dma_start(o1_64_sb[:], ones1_64[:])
            o1_128_sb = cp.tile([1, P], FP16)
            nc.sync.dma_start(o1_128_sb[:], ones1_128[:])
            o1_512_sb = cp.tile([1, NB * P], FP16)
            nc.sync.dma_start(o1_512_sb[:], ones1_512[:])
            b_in_sb = cp.tile([1, HID], FP16)
            nc.sync.dma_start(b_in_sb[:], b_in_col[:])
            gbias_sb = cp.tile([P, HID], FP32)
            nc.sync.dma_start(gbias_sb[:], gbias_rep[:])
            wcls_sb = cp.tile([HID, OUT_DIM], FP16)
            nc.sync.dma_start(wcls_sb[:], w_cls[:])
            bcls_sb = cp.tile([1, OUT_DIM], FP16)
            nc.sync.dma_start(bcls_sb[:], bcls_row[:])
            idxlo_sb = cp.tile([P, C_CHUNKS * t_lo * 8], mybir.dt.int16)
            nc.sync.dma_start(idxlo_sb[:], idx_lo[:])
            if t_hi:
                idxhi_sb = cp.tile([P, C_CHUNKS * t_hi * 8], mybir.dt.int16)
                nc.sync.dma_start(idxhi_sb[:], idx_hi[:])

            # ---------------- dense phase ----------------
            def dense_group(g, xg, w260, q260, b260, row_sink, sb, ps):
                W = NB * P
                xsb = sb.tile([P, 2 * W], FP16, tag="xsb")
                nc.sync.dma_start(xsb[:], xg[g].rearrange("p j n -> p (j n)"))
                ht_ps = ps.tile([HID, W], FP32, tag="ht_ps")
                nc.tensor.matmul(out=ht_ps[:], lhsT=w_in_a_sb[:],
                                 rhs=xsb[:, 0:W], start=True, stop=False)
                nc.tensor.matmul(out=ht_ps[:], lhsT=w_in_b_sb[:],
                                 rhs=xsb[:, W:2 * W], start=False, stop=False)
                nc.tensor.matmul(out=ht_ps[:], lhsT=b_in_sb[:],
                                 rhs=o1_512_sb[:], start=False, stop=True)
                ht = sb.tile([HID, W], FP16, tag="ht")
                nc.scalar.activation(ht[:], ht_ps[:], AF.Relu)
                rsq = sb.tile([HID, W], FP16, tag="rsq")
                nc.scalar.activation(rsq[:], ht[:], AF.Square)
                ssum = ps.tile([P, NB], FP32, tag="ssum")
                for t in range(NB):
                    nc.tensor.matmul(out=ssum[:, t:t + 1],
                                     lhsT=rsq[:, t * P:(t + 1) * P],
                                     rhs=ones64_sb[:], start=True, stop=True)
                nrm = sb.tile([P, NB], FP32, tag="nrm")
                nc.scalar.activation(nrm[:], ssum[:], AF.Sqrt)
                nrm2 = sb.tile([P, NB], FP32, tag="nrm2")
                nc.vector.tensor_scalar_add(nrm2[:], nrm[:], 1e-12)
                inv = sb.tile([P, NB], FP32, tag="inv")
                nc.vector.reciprocal(inv[:], nrm2[:])
                for t in range(NB):
                    xl_ps = ps.tile([P, ROW], FP32, tag=f"xl_ps{t % 2}")
                    nc.tensor.matmul(out=xl_ps[:], lhsT=ht[:, t * P:(t + 1) * P],
                                     rhs=w260[:], start=True, stop=False)
                    nc.tensor.matmul(out=xl_ps[:], lhsT=o1_128_sb[:],
                                     rhs=b260[:], start=False, stop=True)
                    u_ps = ps.tile([P, ROW], FP32, tag=f"u_ps{t % 2}")
                    nc.tensor.matmul(out=u_ps[:], lhsT=ht[:, t * P:(t + 1) * P],
                                     rhs=q260[:], start=True, stop=True)
                    # sem-part scaled per node by inv (1/|h|) on ScalarE
                    sem_sb = sb.tile([P, ROW], FP16, tag=f"sem{t % 2}")
                    nc.scalar.activation(sem_sb[:], u_ps[:], AF.Copy,
                                         scale=inv[:, t:t + 1])
                    xlo = sb.tile([P, ROW], FP16, tag=f"xlo{t % 2}")
                    nc.vector.tensor_add(xlo[:], xl_ps[:], sem_sb[:])
                    nc.sync.dma_start(row_sink(g * NB + t), xlo[:])

            def xl_sink(tile_i):
                r = tile_i * P
                if r < LO_ROWS:
                    return xl_lo_tab[r:r + P, 0:ROW]
                return xl_hi_tab[r - LO_ROWS:r - LO_ROWS + P, 0:ROW]

            def xr_sink(tile_i):
                return xr_tab[tile_i * P:(tile_i + 1) * P, :]

            with tc.tile_pool(name="dsb", bufs=3) as dsb, \
                    tc.tile_pool(name="dps", bufs=1, space="PSUM") as dps:
                for g in range(GA):
                    dense_group(g, xg_all, wl_sb, ql_sb, bl_sb, xl_sink,
                                dsb, dps)
                for g in range(GB):
                    dense_group(g, xg_own, wr_sb, qr_sb, br_sb, xr_sink,
                                dsb, dps)

            # ---------------- edge phase ----------------
            with tc.tile_pool(name="esb", bufs=2) as esb, \
                    tc.tile_pool(name="msb", bufs=3) as msb, \
                    tc.tile_pool(name="eps", bufs=2, space="PSUM") as eps, \
                    tc.tile_pool(name="zps", bufs=3, space="PSUM") as zps, \
                    tc.tile_pool(name="ops", bufs=1, space="PSUM") as ops:
                GB_T = 8        # tiles per dma_gather call (<=1024 indices)
                for c in range(C_CHUNKS):
                    xlg = esb.tile([P, t_ch * TROW], FP16, tag="xlg")
                    segs = [(t_lo, 0, xl_lo_tab, idxlo_sb)]
                    if t_hi:
                        segs.append((t_hi, t_lo, xl_hi_tab, idxhi_sb))
                    for t_seg, off, tab, idx_sb_ in segs:
                        for b in range(0, t_seg, GB_T):
                            nt = min(GB_T, t_seg - b)
                            nc.gpsimd.dma_gather(
                                out_ap=xlg[:, (off + b) * TROW:
                                           (off + b + nt) * TROW].rearrange(
                                    "p (t r) -> p t r", r=TROW),
                                in_ap=tab[:],
                                idxs_ap=idx_sb_[:, (c * t_seg + b) * 8:
                                                (c * t_seg + b + nt) * 8],
                                num_idxs=nt * P, num_idxs_reg=nt * P,
                                elem_size=TROW)
                    oht = esb.tile([P, t_ch * P], FP16, tag="oht")
                    nc.sync.dma_start(oht[:], onehot_t[c])
                    ohe = esb.tile([P, t_ch * P], FP16, tag="ohe")
                    nc.sync.dma_start(ohe[:], onehot_e[c])
                    xr_sb = esb.tile([P, ROW], FP16, tag="xr_sb")
                    nc.sync.dma_start(xr_sb[:], xr_tab[c * P:(c + 1) * P, :])

                    logits = esb.tile([P, t_ch * HEADS], FP32, tag="logits")
                    for t in range(t_ch):
                        z_ps = zps.tile([P, ROW], FP32, tag="z_ps")
                        nc.tensor.matmul(out=z_ps[:],
                                         lhsT=oht[:, t * P:(t + 1) * P],
                                         rhs=xr_sb[:], start=True, stop=False)
                        nc.tensor.matmul(out=z_ps[:], lhsT=id_sb[:],
                                         rhs=xlg[:, t * TROW:t * TROW + ROW],
                                         start=False, stop=True)
                        q = msb.tile([P, FEAT], FP16, tag="q")
                        nc.scalar.activation(q[:], z_ps[:, 0:FEAT], AF.Abs)
                        prod = msb.tile([P, FEAT], FP16, tag="prod")
                        nc.vector.tensor_mul(prod[:], q[:], att_sb[:])
                        red4 = msb.tile([P, HEADS], FP32, tag="red4")
                        nc.vector.tensor_reduce(
                            out=red4[:],
                            in_=prod[:].rearrange("p (h c) -> p h c", h=HEADS),
                            axis=mybir.AxisListType.X, op=ALU.add)
                        nc.vector.tensor_add(
                            logits[:, t * HEADS:(t + 1) * HEADS],
                            red4[:], z_ps[:, FEAT:ROW])
                    expv = esb.tile([P, t_ch * HEADS], FP32, tag="expv")
                    nc.scalar.activation(expv[:], logits[:], AF.Exp)

                    agg_ps = eps.tile([P, ROW], FP32, tag="agg_ps")
                    for t in range(t_ch):
                        msg = msb.tile([P, ROW], FP16, tag="msg")
                        for h in range(HEADS):
                            nc.vector.tensor_scalar_mul(
                                msg[:, h * HID:(h + 1) * HID],
                                xlg[:, t * TROW + h * HID:
                                    t * TROW + (h + 1) * HID],
                                expv[:, t * HEADS + h:t * HEADS + h + 1])
                        nc.vector.tensor_copy(
                            msg[:, FEAT:ROW],
                            expv[:, t * HEADS:(t + 1) * HEADS])
                        nc.tensor.matmul(out=agg_ps[:],
                                         lhsT=ohe[:, t * P:(t + 1) * P],
                                         rhs=msg[:], start=(t == 0),
                                         stop=(t == t_ch - 1))

                    # chunk finish
                    den = msb.tile([P, HEADS], FP32, tag="den")
                    nc.vector.tensor_scalar_add(den[:], agg_ps[:, FEAT:ROW],
                                                1e-16)
                    dinv = msb.tile([P, HEADS], FP32, tag="dinv")
                    nc.vector.reciprocal(dinv[:], den[:])
                    dinv2 = msb.tile([P, HEADS], FP32, tag="dinv2")
                    nc.vector.tensor_scalar_mul(dinv2[:], dinv[:], 0.25)
                    osb = msb.tile([P, FEAT], FP16, tag="osb")
                    for h in range(HEADS):
                        nc.vector.tensor_mul(
                            osb[:, h * HID:(h + 1) * HID],
                            agg_ps[:, h * HID:(h + 1) * HID],
                            dinv2[:, h:h + 1].to_broadcast([P, HID]))
                    ored = msb.tile([P, HID], FP32, tag="ored")
                    nc.vector.tensor_reduce(
                        out=ored[:],
                        in_=osb[:].rearrange("p (h c) -> p c h", h=HEADS),
                        axis=mybir.AxisListType.X, op=ALU.add)
                    obias = msb.tile([P, HID], FP32, tag="obias")
                    nc.vector.tensor_add(obias[:], ored[:], gbias_sb[:])
                    orelu = msb.tile([P, HID], FP32, tag="orelu")
                    nc.scalar.activation(orelu[:], obias[:], AF.Relu)
                    ot_ps = ops.tile([HID, P], FP32, tag="ot_ps")
                    nc.tensor.transpose(out=ot_ps[:], in_=orelu[:],
                                        identity=id32_sb[:])
                    ot_sb = msb.tile([HID, P], FP16, tag="ot_sb")
                    nc.scalar.copy(ot_sb[:], ot_ps[:])
                    fin_ps = eps.tile([P, OUT_DIM], FP32, tag="fin_ps")
                    nc.tensor.matmul(out=fin_ps[:], lhsT=ot_sb[:],
                                     rhs=wcls_sb[:], start=True, stop=False)
                    nc.tensor.matmul(out=fin_ps[:], lhsT=o1_128_sb[:],
                                     rhs=bcls_sb[:], start=False, stop=True)
                    fin_sb = msb.tile([P, OUT_DIM], FP32, tag="fin_sb")
                    nc.vector.tensor_copy(fin_sb[:], fin_ps[:])
                    nc.sync.dma_start(out_ext[c * P:(c + 1) * P, :], fin_sb[:])

    nc.compile()
    return nc


# ----------------------------------------------------------------------------
# Host-side data preparation
# ----------------------------------------------------------------------------

def prepare_host(x, edge_index, W_in, b_in, prototypes, W_l, b_l, W_r, b_r,
                 att, gat_bias, W_cls, b_cls, n_cores):
    n = x.shape[0]
    nodes_per_core = n // n_cores
    NB4 = 4 * P

    n_nodes_pad = _cdiv(n, NB4) * NB4
    npc_dense = _cdiv(nodes_per_core, NB4) * NB4
    npc_chunks = _cdiv(nodes_per_core, P) * P
    c_chunks = npc_chunks // P

    src = np.asarray(edge_index[0], dtype=np.int64)
    dst = np.asarray(edge_index[1], dtype=np.int64)
    loop = np.arange(n, dtype=np.int64)
    src = np.concatenate([src, loop])
    dst = np.concatenate([dst, loop])

    core = dst // nodes_per_core
    dstl = dst - core * nodes_per_core
    chunk = dstl // P
    seg = (src >= LO_ROWS).astype(np.int64)     # 0 = lo table, 1 = hi table

    counts = np.zeros((n_cores, c_chunks, 2), dtype=np.int64)
    np.add.at(counts, (core, chunk, seg), 1)
    t_lo = int(_cdiv(counts[:, :, 0].max(), P))
    t_hi = int(_cdiv(counts[:, :, 1].max(), P))
    t_ch = t_lo + t_hi

    order = np.lexsort((seg, chunk, core))
    src_o, core_o, chunk_o, dstl_o, seg_o = (src[order], core[order],
                                             chunk[order], dstl[order],
                                             seg[order])

    slots = t_ch * P
    idxval_slot = np.zeros((n_cores, c_chunks, slots), dtype=np.int32)
    nloc_slot = np.full((n_cores, c_chunks, slots), -1, dtype=np.int32)
    bounds = np.zeros(n_cores * c_chunks * 2 + 1, dtype=np.int64)
    np.cumsum(counts.reshape(-1), out=bounds[1:])
    flat_bucket = (core_o * c_chunks + chunk_o) * 2 + seg_o
    pos = np.arange(len(src_o)) - bounds[flat_bucket]
    slot = pos + seg_o * (t_lo * P)
    idxval_slot[core_o, chunk_o, slot] = (src_o - seg_o * LO_ROWS
                                          ).astype(np.int32)
    nloc_slot[core_o, chunk_o, slot] = (dstl_o - chunk_o * P).astype(np.int32)

    # int16 wrapped index arrays: flat position i -> [p % 16 == i % 16, i//16]
    def wrap16(vals, tseg):
        # vals [k, c, tseg*128] -> [k, 128, c*tseg*8] int16
        v = vals.reshape(n_cores, c_chunks, tseg * 8, 16)
        v = np.transpose(v, (0, 3, 1, 2))          # [k, 16, c, s]
        v = np.tile(v, (1, 8, 1, 1))               # replicate to 128 parts
        return np.ascontiguousarray(
            v.reshape(n_cores, P, c_chunks * tseg * 8)).astype(np.int16)

    idx_lo = wrap16(idxval_slot[:, :, :t_lo * P], t_lo)
    idx_hi = (wrap16(idxval_slot[:, :, t_lo * P:], t_hi) if t_hi else None)

    nl = nloc_slot.reshape(n_cores, c_chunks, t_ch, P)
    iota = np.arange(P, dtype=np.int32)
    # onehot_t [k, c, n, t*P+e] ; onehot_e [k, c, e, t*P+n]
    oh = (nl[..., None] == iota).astype(f16)          # [k, c, t, e, n]
    onehot_e = np.ascontiguousarray(
        np.transpose(oh, (0, 1, 3, 2, 4))).reshape(n_cores, c_chunks, P, -1)
    onehot_t = np.ascontiguousarray(
        np.transpose(oh, (0, 1, 4, 2, 3))).reshape(n_cores, c_chunks, P, -1)

    # dense-phase weight prep
    att_blk = np.zeros((FEAT, HEADS), dtype=np.float32)
    for h in range(HEADS):
        att_blk[h * HID:(h + 1) * HID, h] = att[h]
    p_norm = prototypes / (np.linalg.norm(prototypes, axis=1, keepdims=True)
                           + 1e-12)
    Q_l = p_norm.T @ W_l[HID:HID + 2]
    Q_r = p_norm.T @ W_r[HID:HID + 2]

    def ext260(w, b):
        w260 = np.concatenate([w, 0.6 * (w @ att_blk)], axis=1)
        b260 = np.concatenate([b, 0.6 * (b @ att_blk)])[None, :]
        return w260.astype(f16), b260.astype(f16)

    wl260, _ = ext260(W_l[:HID], b_l)
    ql260, _ = ext260(Q_l, b_l * 0)
    _, bl260 = ext260(W_l[:HID], b_l)
    wr260, _ = ext260(W_r[:HID], b_r)
    qr260, _ = ext260(Q_r, b_r * 0)
    _, br260 = ext260(W_r[:HID], b_r)

    # x swizzles
    def swizzle(xa, npad):
        G = npad // NB4
        xp = np.zeros((npad, IN_DIM), dtype=np.float32)
        xp[:len(xa)] = xa
        # [g, p, j, t, n] = x[(4g+t)*128+n, j*128+p] -> store [g, p, 2, 4*128]
        v = xp.reshape(G, 4, P, 2, P)          # [g, t, n, j, p]
        v = np.transpose(v, (0, 4, 3, 1, 2))   # [g, p, j, t, n]
        return np.ascontiguousarray(v.reshape(G, P, 2, 4 * P)).astype(f16)

    xg_all = swizzle(np.asarray(x, np.float32), n_nodes_pad)
    xg_own = [swizzle(np.asarray(x[k * nodes_per_core:(k + 1) * nodes_per_core],
                                 np.float32), npc_dense)
              for k in range(n_cores)]

    att04 = np.broadcast_to((0.4 * att.reshape(-1)).astype(f16),
                            (P, FEAT)).copy()
    shared = {
        "xg_all": xg_all,
        "w_in_a": W_in[:P].astype(f16), "w_in_b": W_in[P:].astype(f16),
        "wl260": wl260, "ql260": ql260, "bl260": bl260,
        "wr260": wr260, "qr260": qr260, "br260": br260,
        "att04": att04,
        "ident": np.eye(P, dtype=f16),
        "ident32": np.eye(P, dtype=np.float32),
        "ones64": np.ones((HID, 1), f16),
        "ones1_64": np.ones((1, HID), f16),
        "ones1_128": np.ones((1, P), f16),
        "ones1_512": np.ones((1, 4 * P), f16),
        "b_in_col": b_in[None, :].astype(f16),
        "gbias_rep": np.broadcast_to(gat_bias.astype(np.float32),
                                     (P, HID)).copy(),
        "w_cls": W_cls.astype(f16),
        "bcls_row": b_cls[None, :].astype(f16),
    }
    in_maps = []
    for k in range(n_cores):
        m = dict(shared)
        m["xg_own"] = xg_own[k]
        m["idx_lo"] = idx_lo[k]
        if t_hi:
            m["idx_hi"] = idx_hi[k]
        m["onehot_t"] = onehot_t[k]
        m["onehot_e"] = onehot_e[k]
        in_maps.append(m)
    return in_maps, n_nodes_pad, npc_dense, npc_chunks, t_lo, t_hi


_CACHE = {}


def run(inputs, n_cores=8, trace=False):
    x = np.asarray(inputs["x"])
    n = x.shape[0]
    in_maps, n_nodes_pad, npc_dense, npc_chunks, t_lo, t_hi = prepare_host(
        x, np.asarray(inputs["edge_index"]), np.asarray(inputs["W_in"]),
        np.asarray(inputs["b_in"]), np.asarray(inputs["prototypes"]),
        np.asarray(inputs["W_l"]), np.asarray(inputs["b_l"]),
        np.asarray(inputs["W_r"]), np.asarray(inputs["b_r"]),
        np.asarray(inputs["att"]), np.asarray(inputs["gat_bias"]),
        np.asarray(inputs["W_cls"]), np.asarray(inputs["b_cls"]), n_cores)
    key = (n_nodes_pad, npc_dense, npc_chunks, t_lo, t_hi, n_cores)
    if key not in _CACHE:
        _CACHE[key] = build_program(*key)
    nc = _CACHE[key]
    res = run_bass_kernel_spmd(nc, in_maps, list(range(n_cores)), trace=trace)
    npc = n // n_cores
    outs = [np.asarray(res.results[k]["out"])[:npc] for k in range(n_cores)]
    return np.concatenate(outs, axis=0), res


def kernel(**inputs):
    out, _ = run(inputs, n_cores=8)
    return out.astype(np.float32)



# revision 6
# speedup vs baseline: 1.8153x; 1.8153x over previous
"""GATv2-based CGNN forward pass on 8 Trainium2 NeuronCores.

Strategy (dst-node sharded, no collectives):
  - Each core owns N/8 destination nodes. Host buckets edges (incl. self
    loops) by dst core, then by 128-node dst chunk within the core.
  - Dense phase (replicated for xl over all nodes; own nodes for xr):
    xl/xr rows are [feat256 head-interleaved (c' = j*4+h), beta4, ones4]
    fp16, where beta = 0.2*(feat @ att_blk) so that
      logit = 0.8*sum_c att_c*relu(z_c) + beta_l[s] + beta_r[d]
    (exact rewrite of att . leaky_relu via |z| = 2 relu(z) - z).
  - Edge phase per 128-dst chunk: batched indirect-DMA gather of xl[src]
    rows; z built TRANSPOSED in PSUM ([channel, edge]) from a one-hot
    matmul of resident xr plus identity-matmul transpose of the gathered
    xl; one relu Activation moves it to SBUF; per-tile logits come from
    tall-skinny matmuls with the relu'd block as lhsT. Messages are one
    fp16 tensor_tensor multiply (broadcast alpha) and are scatter-added
    via fp8 one-hot matmuls; denominators ride along as ones*alpha cols.
  - Finish: per-chunk softmax normalize + head mean (folded 0.25 into
    W_cls), relu; classifier runs as a final batched phase.
"""

import os
import sys

import numpy as np
import ml_dtypes

for _p in ("/opt/trn_rl_repo",):
    if _p not in sys.path and os.path.isdir(_p):
        sys.path.insert(0, _p)

import concourse.bass as bass
import concourse.tile as tile
from concourse import bacc, mybir
from concourse.bass_utils import run_bass_kernel_spmd

FP16 = mybir.dt.float16
FP32 = mybir.dt.float32
FP8 = mybir.dt.float8e4
INT16 = mybir.dt.int16
AF = mybir.ActivationFunctionType
ALU = mybir.AluOpType

P = 128
HID = 64
HEADS = 4
OUT_DIM = 16
IN_DIM = 256
FEAT = HEADS * HID          # 256
ROW = FEAT + 2 * HEADS      # 264 = feat + beta + ones
TROW = 384                  # padded table row (768B, 256B-aligned)
LO_ROWS = 32768             # int16 index range per gather table
NB = 4                      # node tiles per dense group

f16 = ml_dtypes.float16 if hasattr(ml_dtypes, "float16") else np.float16
f8 = ml_dtypes.float8_e4m3


def _cdiv(a, b):
    return (a + b - 1) // b


# ----------------------------------------------------------------------------
# Device program
# ----------------------------------------------------------------------------

def build_program(n_nodes_pad, npc_dense, npc_chunks, t_lo, t_hi, n_cores):
    GA = n_nodes_pad // (NB * P)
    GB = npc_dense // (NB * P)
    C_CHUNKS = npc_chunks // P
    t_ch = t_lo + t_hi
    hi_rows = max(n_nodes_pad - LO_ROWS, P)
    xr_slots = GB * NB                    # >= C_CHUNKS

    nc = bacc.Bacc("TRN2", target_bir_lowering=False, debug=False,
                   num_devices=n_cores)

    def din(name, shape, dtype=FP16):
        return nc.dram_tensor(name, shape, dtype, kind="ExternalInput").ap()

    xg_all = din("xg_all", [GA, P, 2, NB * P])
    xg_own = din("xg_own", [GB, P, 2, NB * P])
    w_in_a = din("w_in_a", [P, HID])
    w_in_b = din("w_in_b", [P, HID])
    b_in_c = din("b_in_c", [HID, 1])
    wl264 = din("wl264", [HID, ROW])
    ql264 = din("ql264", [HID, ROW])
    bl264 = din("bl264", [1, ROW])
    wr264 = din("wr264", [HID, ROW])
    qr264 = din("qr264", [HID, ROW])
    br264 = din("br264", [1, ROW])
    att8 = din("att8", [P, 2 * HEADS])
    ident = din("ident", [P, P])
    ones6464 = din("ones6464", [HID, HID])
    o1_128 = din("o1_128", [1, P])
    gbias4 = din("gbias4", [P, HID], FP32)
    wcls4 = din("wcls4", [HID, OUT_DIM])
    bcls4 = din("bcls4", [1, NB * OUT_DIM])
    idx_lo = din("idx_lo", [P, C_CHUNKS * t_lo * 8], INT16)
    idx_hi = (din("idx_hi", [P, C_CHUNKS * t_hi * 8], INT16) if t_hi else None)
    ohcomb = din("ohcomb", [C_CHUNKS, P, 2 * t_ch * P], FP8)

    out_ext = nc.dram_tensor("out", [P, C_CHUNKS * OUT_DIM], FP32,
                             kind="ExternalOutput").ap()

    xl_lo_tab = nc.dram_tensor("xl_lo_tab", [min(n_nodes_pad, LO_ROWS), TROW],
                               FP16).ap()
    xl_hi_tab = nc.dram_tensor("xl_hi_tab", [hi_rows, TROW], FP16).ap()

    with tile.TileContext(nc) as tc:
        with tc.tile_pool(name="consts", bufs=1) as cp:
            w_in_a_sb = cp.tile([P, HID], FP16)
            nc.sync.dma_start(w_in_a_sb[:], w_in_a[:])
            w_in_b_sb = cp.tile([P, HID], FP16)
            nc.sync.dma_start(w_in_b_sb[:], w_in_b[:])
            b_in_sb = cp.tile([HID, 1], FP16)
            nc.sync.dma_start(b_in_sb[:], b_in_c[:])
            wl_sb = cp.tile([HID, ROW], FP16)
            nc.sync.dma_start(wl_sb[:], wl264[:])
            ql_sb = cp.tile([HID, ROW], FP16)
            nc.sync.dma_start(ql_sb[:], ql264[:])
            bl_sb = cp.tile([1, ROW], FP16)
            nc.sync.dma_start(bl_sb[:], bl264[:])
            wr_sb = cp.tile([HID, ROW], FP16)
            nc.sync.dma_start(wr_sb[:], wr264[:])
            qr_sb = cp.tile([HID, ROW], FP16)
            nc.sync.dma_start(qr_sb[:], qr264[:])
            br_sb = cp.tile([1, ROW], FP16)
            nc.sync.dma_start(br_sb[:], br264[:])
            att_sb = cp.tile([P, 2 * HEADS], FP16)
            nc.sync.dma_start(att_sb[:], att8[:])
            id_sb = cp.tile([P, P], FP16)
            nc.sync.dma_start(id_sb[:], ident[:])
            ones64_sb = cp.tile([HID, HID], FP16)
            nc.sync.dma_start(ones64_sb[:], ones6464[:])
            o1_sb = cp.tile([1, P], FP16)
            nc.sync.dma_start(o1_sb[:], o1_128[:])
            gbias_sb = cp.tile([P, HID], FP32)
            nc.sync.dma_start(gbias_sb[:], gbias4[:])
            wcls_sb = cp.tile([HID, OUT_DIM], FP16)
            nc.sync.dma_start(wcls_sb[:], wcls4[:])
            bcls_sb = cp.tile([1, NB * OUT_DIM], FP16)
            nc.sync.dma_start(bcls_sb[:], bcls4[:])
            idxlo_sb = cp.tile([P, C_CHUNKS * t_lo * 8], INT16)
            nc.sync.dma_start(idxlo_sb[:], idx_lo[:])
            if t_hi:
                idxhi_sb = cp.tile([P, C_CHUNKS * t_hi * 8], INT16)
                nc.sync.dma_start(idxhi_sb[:], idx_hi[:])

            eps_sb = cp.tile([HID, 1], FP32)
            nc.gpsimd.memset(eps_sb[:], 1e-12)

            xr_res = cp.tile([P, xr_slots, ROW], FP16)
            orelu_res = cp.tile([P, C_CHUNKS, HID], FP16)
            fin_res = cp.tile([P, C_CHUNKS, OUT_DIM], FP32)

            # ---------------- dense phase ----------------
            W = NB * P

            def dense_group(g, xg, w_sb, q_sb, b_sb, to_table, sb, ps):
                xsb = sb.tile([P, 2, W], FP16, tag="xsb")
                nc.sync.dma_start(xsb[:], xg[g])
                ht_ps = ps.tile([HID, W], FP32, tag="ht_ps")
                nc.tensor.matmul(out=ht_ps[:], lhsT=w_in_a_sb[:],
                                 rhs=xsb[:, 0, :], start=True, stop=False)
                nc.tensor.matmul(out=ht_ps[:], lhsT=w_in_b_sb[:],
                                 rhs=xsb[:, 1, :], start=False, stop=True)
                ht = sb.tile([HID, W], FP16, tag="ht")
                nc.scalar.activation(ht[:], ht_ps[:], AF.Relu,
                                     bias=b_in_sb[:])
                rsq = sb.tile([HID, W], FP16, tag="rsq")
                nc.vector.tensor_mul(rsq[:], ht[:], ht[:])
                nrm2_ps = ps.tile([HID, W], FP32, tag="nrm2_ps")
                nc.tensor.matmul(out=nrm2_ps[:], lhsT=ones64_sb[:],
                                 rhs=rsq[:], start=True, stop=True)
                nrmr = sb.tile([HID, W], FP32, tag="nrmr")
                nc.scalar.activation(nrmr[:], nrm2_ps[:], AF.Sqrt,
                                     bias=eps_sb[:])
                invr = sb.tile([HID, W], FP32, tag="invr")
                nc.vector.reciprocal(invr[:], nrmr[:])
                htn = sb.tile([HID, W], FP16, tag="htn")
                nc.gpsimd.tensor_mul(htn[:], ht[:], invr[:])
                xlo_g = (sb.tile([P, NB, ROW], FP16, tag="xlo_g",
                                 name="xlo_g") if to_table else None)
                for t in range(NB):
                    xl_ps = ps.tile([P, ROW], FP32, tag=f"xl_ps{t % 2}")
                    tsl = slice(t * P, (t + 1) * P)
                    nc.tensor.matmul(out=xl_ps[:], lhsT=ht[:, tsl],
                                     rhs=w_sb[:], start=True, stop=False)
                    nc.tensor.matmul(out=xl_ps[:], lhsT=htn[:, tsl],
                                     rhs=q_sb[:], start=False, stop=False)
                    nc.tensor.matmul(out=xl_ps[:], lhsT=o1_sb[:],
                                     rhs=b_sb[:], start=False, stop=True)
                    dst_ap = (xlo_g[:, t, :] if to_table
                              else xr_res[:, g * NB + t, :])
                    if t == 0:
                        nc.scalar.copy(dst_ap, xl_ps[:])
                    else:
                        nc.vector.tensor_copy(dst_ap, xl_ps[:])
                if to_table:
                    r = g * NB * P
                    if r < LO_ROWS:
                        sink = xl_lo_tab[r:r + NB * P]
                    else:
                        sink = xl_hi_tab[r - LO_ROWS:r - LO_ROWS + NB * P]
                    nc.sync.dma_start(
                        sink.rearrange("(t p) c -> p t c", p=P)[:, :, 0:ROW],
                        xlo_g[:])

            with tc.tile_pool(name="dsb", bufs=3) as dsb, \
                    tc.tile_pool(name="dps", bufs=1, space="PSUM") as dps:
                for g in range(GB):
                    dense_group(g, xg_own, wr_sb, qr_sb, br_sb, False,
                                dsb, dps)
                for g in range(GA):
                    dense_group(g, xg_all, wl_sb, ql_sb, bl_sb, True,
                                dsb, dps)

            # ---------------- edge phase ----------------
            GB_T = 8            # tiles per dma_gather call (<=1024 indices)
            ZG = 4              # tiles per z-group (PSUM bank pair)

            with tc.tile_pool(name="esb", bufs=2) as esb, \
                    tc.tile_pool(name="msb", bufs=4) as msb, \
                    tc.tile_pool(name="zps", bufs=2, space="PSUM") as zps, \
                    tc.tile_pool(name="lps", bufs=2, space="PSUM") as lps, \
                    tc.tile_pool(name="aps", bufs=2, space="PSUM") as aps:

                logits_of = {}
                xlg_of = {}
                oh_of = {}
                expv_of = {}

                def front(c):
                    oh_sb = esb.tile([P, 2 * t_ch * P], FP8, tag="oh")
                    nc.sync.dma_start(oh_sb[:], ohcomb[c])
                    xlg = esb.tile([P, t_ch, TROW], FP16, tag="xlg")
                    segs = [(t_lo, 0, xl_lo_tab, idxlo_sb)]
                    if t_hi:
                        segs.append((t_hi, t_lo, xl_hi_tab, idxhi_sb))
                    for t_seg, off, tab, idx_sb_ in segs:
                        for b in range(0, t_seg, GB_T):
                            nt = min(GB_T, t_seg - b)
                            nc.gpsimd.dma_gather(
                                out_ap=xlg[:, off + b:off + b + nt, :],
                                in_ap=tab[:],
                                idxs_ap=idx_sb_[:, (c * t_seg + b) * 8:
                                                (c * t_seg + b + nt) * 8],
                                num_idxs=nt * P, num_idxs_reg=nt * P,
                                elem_size=TROW)
                    logits_ps = lps.tile([P, t_ch * HEADS], FP32,
                                         tag="logits")
                    for g0 in range(0, t_ch, ZG):
                        gl = min(ZG, t_ch - g0)
                        zt = zps.tile([P, 2, ZG * P], FP32, tag="zt")
                        for gi in range(gl):
                            t = g0 + gi
                            esl = slice(gi * P, (gi + 1) * P)
                            ohsl = slice(t * P, (t + 1) * P)
                            for b in range(2):
                                bsl = slice(b * P, (b + 1) * P)
                                nc.tensor.matmul(
                                    out=zt[:, b, esl],
                                    lhsT=xr_res[:, c, bsl],
                                    rhs=oh_sb[:, ohsl],
                                    start=True, stop=False)
                                nc.tensor.matmul(
                                    out=zt[:, b, esl],
                                    lhsT=xlg[:, t, bsl],
                                    rhs=id_sb[:],
                                    start=False, stop=True)
                        wt = msb.tile([P, 2, ZG * P], FP16, tag="wt",
                                      bufs=3)
                        nc.scalar.activation(wt[:, :, 0:gl * P],
                                             zt[:, :, 0:gl * P], AF.Relu)
                        for gi in range(gl):
                            t = g0 + gi
                            esl = slice(gi * P, (gi + 1) * P)
                            lsl = slice(t * HEADS, (t + 1) * HEADS)
                            ohsl = slice(t * P, (t + 1) * P)
                            nc.tensor.matmul(
                                out=logits_ps[:, lsl], lhsT=id_sb[:],
                                rhs=xlg[:, t, FEAT:FEAT + HEADS],
                                start=True, stop=False)
                            nc.tensor.matmul(
                                out=logits_ps[:, lsl],
                                lhsT=oh_sb[:, ohsl],
                                rhs=xr_res[:, c, FEAT:FEAT + HEADS],
                                start=False, stop=False)
                            nc.tensor.matmul(
                                out=logits_ps[:, lsl], lhsT=wt[:, 0, esl],
                                rhs=att_sb[:, 0:HEADS],
                                start=False, stop=False)
                            nc.tensor.matmul(
                                out=logits_ps[:, lsl], lhsT=wt[:, 1, esl],
                                rhs=att_sb[:, HEADS:2 * HEADS],
                                start=False, stop=True)
                    logits_of[c] = logits_ps
                    xlg_of[c] = xlg
                    oh_of[c] = oh_sb

                def back_a(c):
                    expv = esb.tile([P, t_ch * HEADS], FP16, tag="expv")
                    nc.scalar.activation(expv[:], logits_of.pop(c)[:], AF.Exp)
                    expv_of[c] = expv

                def back_b(c):
                    xlg = xlg_of.pop(c)
                    oh_sb = oh_of.pop(c)
                    expv = expv_of.pop(c)
                    agg = aps.tile([P, ROW], FP32, tag="agg")
                    for t in range(t_ch):
                        msg = msb.tile([P, ROW], FP16, tag="msg")
                        a4 = expv[:, t * HEADS:(t + 1) * HEADS] \
                            .rearrange("p (o h) -> p o h", o=1) \
                            .to_broadcast([P, ROW // HEADS, HEADS])
                        nc.vector.tensor_tensor(
                            msg[:].rearrange("p (j h) -> p j h", h=HEADS),
                            xlg[:, t, 0:ROW].rearrange("p (j h) -> p j h",
                                                       h=HEADS),
                            a4, ALU.mult)
                        nc.tensor.matmul(
                            out=agg[:],
                            lhsT=oh_sb[:, (t_ch + t) * P:(t_ch + t + 1) * P],
                            rhs=msg[:], start=(t == 0), stop=(t == t_ch - 1))
                    den = msb.tile([P, HEADS], FP32, tag="den")
                    nc.vector.tensor_scalar_add(den[:], agg[:, ROW - HEADS:],
                                                1e-16)
                    dinv = msb.tile([P, HEADS], FP32, tag="dinv")
                    nc.vector.reciprocal(dinv[:], den[:])
                    scl = msb.tile([P, FEAT], FP32, tag="scl")
                    di4 = dinv[:].rearrange("p (o h) -> p o h", o=1) \
                        .to_broadcast([P, HID, HEADS])
                    nc.vector.tensor_tensor(
                        scl[:].rearrange("p (j h) -> p j h", h=HEADS),
                        agg[:, 0:FEAT].rearrange("p (j h) -> p j h", h=HEADS),
                        di4, ALU.mult)
                    ored = msb.tile([P, HID], FP32, tag="ored")
                    nc.vector.tensor_reduce(
                        out=ored[:],
                        in_=scl[:].rearrange("p (j h) -> p j h", h=HEADS),
                        axis=mybir.AxisListType.X, op=ALU.add)
                    obias = msb.tile([P, HID], FP32, tag="obias")
                    nc.vector.tensor_add(obias[:], ored[:], gbias_sb[:])
                    nc.scalar.activation(orelu_res[:, c, :], obias[:],
                                         AF.Relu)

                front(0)
                for c in range(C_CHUNKS):
                    back_a(c)
                    if c + 1 < C_CHUNKS:
                        front(c + 1)
                    back_b(c)

            # ---------------- classifier phase ----------------
            with tc.tile_pool(name="fsb", bufs=2) as fsb, \
                    tc.tile_pool(name="fps", bufs=2, space="PSUM") as fps:
                for c0 in range(0, C_CHUNKS, NB):
                    cl = min(NB, C_CHUNKS - c0)
                    ot_ps = fps.tile([HID, NB * P], FP16, tag="ot_ps")
                    for ci in range(cl):
                        nc.tensor.transpose(
                            out=ot_ps[:, ci * P:(ci + 1) * P],
                            in_=orelu_res[:, c0 + ci, :], identity=id_sb[:])
                    ot16 = fsb.tile([HID, NB * P], FP16, tag="ot16")
                    nc.scalar.copy(ot16[:, 0:cl * P], ot_ps[:, 0:cl * P])
                    fin_ps = fps.tile([P, NB * OUT_DIM], FP32, tag="fin_ps")
                    for ci in range(cl):
                        nc.tensor.matmul(
                            out=fin_ps[:, ci * OUT_DIM:(ci + 1) * OUT_DIM],
                            lhsT=ot16[:, ci * P:(ci + 1) * P],
                            rhs=wcls_sb[:], start=True, stop=False)
                        nc.tensor.matmul(
                            out=fin_ps[:, ci * OUT_DIM:(ci + 1) * OUT_DIM],
                            lhsT=o1_sb[:],
                            rhs=bcls_sb[:, ci * OUT_DIM:(ci + 1) * OUT_DIM],
                            start=False, stop=True)
                    nc.vector.tensor_copy(
                        fin_res[:, c0:c0 + cl, :].rearrange("p c o -> p (c o)"),
                        fin_ps[:, 0:cl * OUT_DIM])
                nc.sync.dma_start(
                    out_ext[:].rearrange("p (c o) -> p c o", o=OUT_DIM),
                    fin_res[:])

    nc.compile()
    return nc


# ----------------------------------------------------------------------------
# Host-side data preparation
# ----------------------------------------------------------------------------

def prepare_host(x, edge_index, W_in, b_in, prototypes, W_l, b_l, W_r, b_r,
                 att, gat_bias, W_cls, b_cls, n_cores):
    x = np.asarray(x, np.float32)
    W_in = np.asarray(W_in, np.float32)
    b_in = np.asarray(b_in, np.float32)
    prototypes = np.asarray(prototypes, np.float32)
    W_l = np.asarray(W_l, np.float32)
    b_l = np.asarray(b_l, np.float32)
    W_r = np.asarray(W_r, np.float32)
    b_r = np.asarray(b_r, np.float32)
    att = np.asarray(att, np.float32)
    gat_bias = np.asarray(gat_bias, np.float32)
    W_cls = np.asarray(W_cls, np.float32)
    b_cls = np.asarray(b_cls, np.float32)

    n = x.shape[0]
    nodes_per_core = n // n_cores
    NB4 = NB * P

    n_nodes_pad = _cdiv(n, NB4) * NB4
    npc_dense = _cdiv(nodes_per_core, NB4) * NB4
    npc_chunks = _cdiv(nodes_per_core, P) * P
    c_chunks = npc_chunks // P

    # --- edge bucketing (same as before) ---
    src = np.asarray(edge_index[0], dtype=np.int64)
    dst = np.asarray(edge_index[1], dtype=np.int64)
    loop = np.arange(n, dtype=np.int64)
    src = np.concatenate([src, loop])
    dst = np.concatenate([dst, loop])

    core = dst // nodes_per_core
    dstl = dst - core * nodes_per_core
    chunk = dstl // P
    seg = (src >= LO_ROWS).astype(np.int64)

    counts = np.zeros((n_cores, c_chunks, 2), dtype=np.int64)
    np.add.at(counts, (core, chunk, seg), 1)
    t_lo = int(_cdiv(counts[:, :, 0].max(), P))
    t_hi = int(_cdiv(counts[:, :, 1].max(), P))
    t_ch = t_lo + t_hi

    order = np.lexsort((seg, chunk, core))
    src_o, core_o, chunk_o, dstl_o, seg_o = (src[order], core[order],
                                             chunk[order], dstl[order],
                                             seg[order])

    slots = t_ch * P
    idxval_slot = np.zeros((n_cores, c_chunks, slots), dtype=np.int32)
    nloc_slot = np.full((n_cores, c_chunks, slots), -1, dtype=np.int32)
    bounds = np.zeros(n_cores * c_chunks * 2 + 1, dtype=np.int64)
    np.cumsum(counts.reshape(-1), out=bounds[1:])
    flat_bucket = (core_o * c_chunks + chunk_o) * 2 + seg_o
    pos = np.arange(len(src_o)) - bounds[flat_bucket]
    slot = pos + seg_o * (t_lo * P)
    idxval_slot[core_o, chunk_o, slot] = (src_o - seg_o * LO_ROWS
                                          ).astype(np.int32)
    nloc_slot[core_o, chunk_o, slot] = (dstl_o - chunk_o * P).astype(np.int32)

    def wrap16(vals, tseg):
        v = vals.reshape(n_cores, c_chunks, tseg * 8, 16)
        v = np.transpose(v, (0, 3, 1, 2))
        v = np.tile(v, (1, 8, 1, 1))
        return np.ascontiguousarray(
            v.reshape(n_cores, P, c_chunks * tseg * 8)).astype(np.int16)

    idx_lo = wrap16(idxval_slot[:, :, :t_lo * P], t_lo)
    idx_hi = (wrap16(idxval_slot[:, :, t_lo * P:], t_hi) if t_hi else None)

    nl = nloc_slot.reshape(n_cores, c_chunks, t_ch, P)
    iota = np.arange(P, dtype=np.int32)
    oh = (nl[..., None] == iota)                       # [k, c, t, e, n]
    ohcomb = np.zeros((n_cores, c_chunks, P, 2 * t_ch * P), dtype=f8)
    # oht: [n, t*P + e]
    ohcomb[:, :, :, :t_ch * P] = np.transpose(oh, (0, 1, 4, 2, 3)) \
        .reshape(n_cores, c_chunks, P, t_ch * P)
    # ohe: [e, t*P + n]
    ohcomb[:, :, :, t_ch * P:] = np.transpose(oh, (0, 1, 3, 2, 4)) \
        .reshape(n_cores, c_chunks, P, t_ch * P)

    # --- weights ---
    att_blk = np.zeros((FEAT, HEADS), dtype=np.float32)
    for h in range(HEADS):
        att_blk[h * HID:(h + 1) * HID, h] = att[h]
    p_norm = prototypes / (np.linalg.norm(prototypes, axis=1, keepdims=True)
                           + 1e-12)
    Q_l = p_norm.T @ W_l[HID:HID + 2]
    Q_r = p_norm.T @ W_r[HID:HID + 2]

    perm = np.zeros(FEAT, np.int64)           # perm[c'] = old col
    for h in range(HEADS):
        for j in range(HID):
            perm[j * HEADS + h] = h * HID + j

    def ext264(w, b, with_ones):
        w264 = np.concatenate(
            [w[:, perm], 0.2 * (w @ att_blk), np.zeros((w.shape[0], HEADS),
                                                       np.float32)], axis=1)
        ones = np.ones(HEADS, np.float32) if with_ones else \
            np.zeros(HEADS, np.float32)
        b264 = np.concatenate([b[perm], 0.2 * (b @ att_blk), ones])[None, :]
        return w264.astype(f16), b264.astype(f16)

    wl264, bl264 = ext264(W_l[:HID], b_l, True)
    ql264, _ = ext264(Q_l, b_l * 0, False)
    wr264, br264 = ext264(W_r[:HID], b_r, True)
    qr264, _ = ext264(Q_r, b_r * 0, False)

    # att8: [p, h] for block b at cols b*4: 0.8*att[h, (b*128+p)//4] if match
    att8 = np.zeros((P, 2 * HEADS), np.float32)
    for b in range(2):
        for p in range(P):
            cprime = b * P + p
            j, h = cprime // HEADS, cprime % HEADS
            att8[p, b * HEADS + h] = 0.8 * att[h, j]

    def swizzle(xa, npad):
        G = npad // NB4
        xp = np.zeros((npad, IN_DIM), dtype=np.float32)
        xp[:len(xa)] = xa
        v = xp.reshape(G, NB, P, 2, P)
        v = np.transpose(v, (0, 4, 3, 1, 2))
        return np.ascontiguousarray(v.reshape(G, P, 2, NB * P)).astype(f16)

    xg_all = swizzle(x, n_nodes_pad)
    xg_own = [swizzle(x[k * nodes_per_core:(k + 1) * nodes_per_core],
                      npc_dense) for k in range(n_cores)]

    shared = {
        "xg_all": xg_all,
        "w_in_a": W_in[:P].astype(f16), "w_in_b": W_in[P:].astype(f16),
        "b_in_c": b_in[:, None].astype(f16),
        "wl264": wl264, "ql264": ql264, "bl264": bl264,
        "wr264": wr264, "qr264": qr264, "br264": br264,
        "att8": att8.astype(f16),
        "ident": np.eye(P, dtype=f16),
        "ones6464": np.ones((HID, HID), f16),
        "o1_128": np.ones((1, P), f16),
        "gbias4": np.broadcast_to(4.0 * gat_bias.astype(np.float32),
                                  (P, HID)).copy(),
        "wcls4": (0.25 * W_cls).astype(f16),
        "bcls4": np.tile(b_cls, NB)[None, :].astype(f16),
    }
    in_maps = []
    for k in range(n_cores):
        m = dict(shared)
        m["xg_own"] = xg_own[k]
        m["idx_lo"] = idx_lo[k]
        if t_hi:
            m["idx_hi"] = idx_hi[k]
        m["ohcomb"] = ohcomb[k]
        in_maps.append(m)
    return in_maps, n_nodes_pad, npc_dense, npc_chunks, t_lo, t_hi


_CACHE = {}


def run(inputs, n_cores=8, trace=False):
    x = np.asarray(inputs["x"])
    n = x.shape[0]
    in_maps, n_nodes_pad, npc_dense, npc_chunks, t_lo, t_hi = prepare_host(
        x, np.asarray(inputs["edge_index"]), np.asarray(inputs["W_in"]),
        np.asarray(inputs["b_in"]), np.asarray(inputs["prototypes"]),
        np.asarray(inputs["W_l"]), np.asarray(inputs["b_l"]),
        np.asarray(inputs["W_r"]), np.asarray(inputs["b_r"]),
        np.asarray(inputs["att"]), np.asarray(inputs["gat_bias"]),
        np.asarray(inputs["W_cls"]), np.asarray(inputs["b_cls"]), n_cores)
    key = (n_nodes_pad, npc_dense, npc_chunks, t_lo, t_hi, n_cores)
    if key not in _CACHE:
        _CACHE[key] = build_program(*key)
    nc = _CACHE[key]
    res = run_bass_kernel_spmd(nc, in_maps, list(range(n_cores)), trace=trace)
    npc = n // n_cores
    c_chunks = npc_chunks // P
    outs = []
    for k in range(n_cores):
        o = np.asarray(res.results[k]["out"]).reshape(P, c_chunks, OUT_DIM)
        outs.append(np.transpose(o, (1, 0, 2)).reshape(npc_chunks,
                                                       OUT_DIM)[:npc])
    return np.concatenate(outs, axis=0), res


def kernel(**inputs):
    out, _ = run(inputs, n_cores=8)
    return out.astype(np.float32)


# revision 25
# speedup vs baseline: 2.5152x; 1.3856x over previous
"""GATv2-based CGNN forward pass on 8 Trainium2 NeuronCores.

Strategy (dst-node sharded, no collectives):
  - Each core owns N/8 destination nodes. Host buckets edges (incl. self
    loops) by dst core, then by 128-node dst chunk within the core.
    Per-chunk tile counts are ragged (max over cores per chunk index) so
    pad work tracks the actual edge distribution.
  - Dense phase (replicated for xl over all nodes; own nodes for xr):
    xl/xr rows are [feat256 head-interleaved (c' = j*4+h), beta4, ones4]
    fp16, where beta = 0.2*(feat @ att_blk) so that
      logit = 0.8*sum_c att_c*relu(z_c) + beta_l[s] + beta_r[d]
    (exact rewrite of att . leaky_relu via |z| = 2 relu(z) - z).
  - Edge phase per 128-dst chunk: batched indirect-DMA gather of xl[src]
    rows (4 SWDGE queues); z built TRANSPOSED in PSUM ([channel, edge])
    from a one-hot matmul of resident xr plus identity-matmul transpose
    of the gathered xl; one relu Activation moves it to SBUF; per-tile
    logits come from tall-skinny matmuls with the relu'd block as lhsT.
    Messages are one fp16 tensor_tensor multiply (broadcast alpha),
    scatter-added via fp8 one-hot matmuls; denominators ride along as
    ones*alpha columns.
  - Finish: per-chunk softmax normalize + head mean (0.25 folded into
    W_cls), relu; classifier runs as a final batched phase.
"""

import os
import sys

import numpy as np
import ml_dtypes

for _p in ("/opt/trn_rl_repo",):
    if _p not in sys.path and os.path.isdir(_p):
        sys.path.insert(0, _p)

import concourse.bass as bass
import concourse.tile as tile
from concourse import bacc, mybir
from concourse.bass_utils import run_bass_kernel_spmd

FP16 = mybir.dt.float16
FP32 = mybir.dt.float32
FP8 = mybir.dt.float8e4
INT16 = mybir.dt.int16
AF = mybir.ActivationFunctionType
ALU = mybir.AluOpType

P = 128
HID = 64
HEADS = 4
OUT_DIM = 16
IN_DIM = 256
FEAT = HEADS * HID          # 256
ROW = FEAT + 2 * HEADS      # 264 = feat + beta + ones
TROW = 384                  # padded table row (768B, 256B-aligned)
LO_ROWS = 32768             # int16 index range per gather table
NB = 4                      # node tiles per dense group

f16 = ml_dtypes.float16 if hasattr(ml_dtypes, "float16") else np.float16
f8 = ml_dtypes.float8_e4m3


def _cdiv(a, b):
    return (a + b - 1) // b


# ----------------------------------------------------------------------------
# Device program
# ----------------------------------------------------------------------------

def build_program(n_nodes_pad, npc_dense, npc_chunks, tl, th, n_cores):
    """tl/th: per-chunk lo/hi gather tile counts (tuples, shared by cores)."""
    GA = n_nodes_pad // (NB * P)
    GB = npc_dense // (NB * P)
    C_CHUNKS = npc_chunks // P
    tl = list(tl)
    th = list(th)
    tch = [a + b for a, b in zip(tl, th)]
    TMAX = max(tch)
    loS = np.concatenate([[0], np.cumsum(tl)]).astype(int)   # tile offsets
    hiS = np.concatenate([[0], np.cumsum(th)]).astype(int)
    ohS = np.concatenate([[0], np.cumsum([2 * t for t in tch])]).astype(int)
    hi_rows = max(n_nodes_pad - LO_ROWS, P)
    xr_slots = GB * NB

    nc = bacc.Bacc("TRN2", target_bir_lowering=False, debug=False,
                   num_devices=n_cores, num_swdge_queues=4)

    def din(name, shape, dtype=FP16):
        return nc.dram_tensor(name, shape, dtype, kind="ExternalInput").ap()

    xg_all = din("xg_all", [GA, P, 2, NB * P])
    xg_own = din("xg_own", [GB, P, 2, NB * P])
    w_in_a = din("w_in_a", [P, HID])
    w_in_b = din("w_in_b", [P, HID])
    b_in_c = din("b_in_c", [HID, 1])
    wql = din("wql", [HID + 1, ROW])       # [w264; b264]
    ql = din("ql", [HID, ROW])
    wqr = din("wqr", [HID + 1, ROW])
    qr = din("qr", [HID, ROW])
    att8 = din("att8", [P, 2 * HEADS])
    ident = din("ident", [P, P])
    ones6464 = din("ones6464", [HID, HID])
    o1_128 = din("o1_128", [1, P])
    gbias4 = din("gbias4", [P, HID], FP32)
    wcls4 = din("wcls4", [HID, OUT_DIM])
    bcls4 = din("bcls4", [1, NB * OUT_DIM])
    idx_lo = din("idx_lo", [P, int(loS[-1]) * 8], INT16)
    idx_hi = (din("idx_hi", [P, int(hiS[-1]) * 8], INT16)
              if hiS[-1] else None)
    ohcomb = din("ohcomb", [P, int(ohS[-1]) * P], FP8)

    out_ext = nc.dram_tensor("out", [P, C_CHUNKS * OUT_DIM], FP32,
                             kind="ExternalOutput").ap()

    xl_lo_tab = nc.dram_tensor("xl_lo_tab", [min(n_nodes_pad, LO_ROWS), TROW],
                               FP16).ap()
    xl_hi_tab = nc.dram_tensor("xl_hi_tab", [hi_rows, TROW], FP16).ap()

    qctr = [0]

    def next_q():
        qctr[0] = (qctr[0] + 1) % 4
        return qctr[0]

    with tile.TileContext(nc) as tc:
        with tc.tile_pool(name="consts", bufs=1) as cp:
            w_in_a_sb = cp.tile([P, HID], FP16)
            nc.sync.dma_start(w_in_a_sb[:], w_in_a[:])
            w_in_b_sb = cp.tile([P, HID], FP16)
            nc.sync.dma_start(w_in_b_sb[:], w_in_b[:])
            b_in_sb = cp.tile([HID, 1], FP16)
            nc.sync.dma_start(b_in_sb[:], b_in_c[:])
            wql_sb = cp.tile([HID + 1, ROW], FP16)
            nc.sync.dma_start(wql_sb[:], wql[:])
            ql_sb = cp.tile([HID, ROW], FP16)
            nc.sync.dma_start(ql_sb[:], ql[:])
            wqr_sb = cp.tile([HID + 1, ROW], FP16)
            nc.sync.dma_start(wqr_sb[:], wqr[:])
            qr_sb = cp.tile([HID, ROW], FP16)
            nc.sync.dma_start(qr_sb[:], qr[:])
            att_sb = cp.tile([P, 2 * HEADS], FP16)
            nc.sync.dma_start(att_sb[:], att8[:])
            id_sb = cp.tile([P, P], FP16)
            nc.sync.dma_start(id_sb[:], ident[:])
            ones64_sb = cp.tile([HID, HID], FP16)
            nc.sync.dma_start(ones64_sb[:], ones6464[:])
            o1_sb = cp.tile([1, P], FP16)
            nc.sync.dma_start(o1_sb[:], o1_128[:])
            gbias_sb = cp.tile([P, HID], FP32)
            nc.sync.dma_start(gbias_sb[:], gbias4[:])
            wcls_sb = cp.tile([HID, OUT_DIM], FP16)
            nc.sync.dma_start(wcls_sb[:], wcls4[:])
            bcls_sb = cp.tile([1, NB * OUT_DIM], FP16)
            nc.sync.dma_start(bcls_sb[:], bcls4[:])
            idxlo_sb = cp.tile([P, int(loS[-1]) * 8], INT16)
            nc.sync.dma_start(idxlo_sb[:], idx_lo[:])
            if idx_hi is not None:
                idxhi_sb = cp.tile([P, int(hiS[-1]) * 8], INT16)
                nc.sync.dma_start(idxhi_sb[:], idx_hi[:])
            eps_sb = cp.tile([HID, 1], FP32)
            nc.gpsimd.memset(eps_sb[:], 1e-12)

            xr_res = cp.tile([P, xr_slots, ROW], FP16)
            orelu_res = cp.tile([P, C_CHUNKS, HID], FP16)
            fin_res = cp.tile([P, C_CHUNKS, OUT_DIM], FP32)

            # ---------------- dense phase ----------------
            W = NB * P

            def dense_stage1(g, xg, wq_sb, q_sb, to_table, sb, ps):
                xsb = sb.tile([P, 2, W], FP16, tag="xsb")
                nc.sync.dma_start(xsb[:], xg[g])
                ht_ps = ps.tile([HID, W], FP32, tag="ht_ps")
                nc.tensor.matmul(out=ht_ps[:], lhsT=w_in_a_sb[:],
                                 rhs=xsb[:, 0, :], start=True, stop=False)
                nc.tensor.matmul(out=ht_ps[:], lhsT=w_in_b_sb[:],
                                 rhs=xsb[:, 1, :], start=False, stop=True)
                hta = sb.tile([HID + 1, W], FP16, tag="hta", bufs=4)
                nc.scalar.activation(hta[0:HID, :], ht_ps[:], AF.Relu,
                                     bias=b_in_sb[:])
                nc.gpsimd.memset(hta[HID:HID + 1, :], 1.0)
                rsq = sb.tile([HID, W], FP16, tag="rsq")
                nc.vector.tensor_mul(rsq[:], hta[0:HID, :], hta[0:HID, :])
                nrm2_ps = ps.tile([HID, W], FP32, tag="nrm2_ps")
                nc.tensor.matmul(out=nrm2_ps[:], lhsT=ones64_sb[:],
                                 rhs=rsq[:], start=True, stop=True)
                return hta, nrm2_ps

            def dense_stage2(hta, nrm2_ps, sb, ps):
                nrmr = sb.tile([HID, W], FP32, tag="nrmr")
                nc.scalar.activation(nrmr[:], nrm2_ps[:], AF.Sqrt,
                                     bias=eps_sb[:])
                invr = sb.tile([HID, W], FP32, tag="invr")
                nc.vector.reciprocal(invr[:], nrmr[:])
                htn = sb.tile([HID, W], FP16, tag="htn", bufs=3)
                nc.gpsimd.tensor_mul(htn[:], hta[0:HID, :], invr[:])
                return htn

            def dense_stage3(g, wq_sb, q_sb, to_table, hta, htn, sb, ps):
                if to_table:
                    dst4 = sb.tile([P, NB, ROW], FP16, tag="dst4",
                                   name="dst4")
                for half in range(2):
                    xl2 = ps.tile([P, 2, 2 * ROW - 16], FP32, tag="xl2")
                    for ti in range(2):
                        t = half * 2 + ti
                        tsl = slice(t * P, (t + 1) * P)
                        nc.tensor.matmul(out=xl2[:, ti, 0:ROW],
                                         lhsT=hta[:, tsl],
                                         rhs=wq_sb[:], start=True, stop=False)
                        nc.tensor.matmul(out=xl2[:, ti, 0:ROW],
                                         lhsT=htn[:, tsl],
                                         rhs=q_sb[:], start=False, stop=True)
                    if to_table:
                        mv_out = dst4[:, half * 2:half * 2 + 2, :]
                    else:
                        mv_out = xr_res[:, g * NB + half * 2:
                                        g * NB + half * 2 + 2, :]
                    # alternate engines for the PSUM->SBUF move
                    if half == 0:
                        nc.scalar.copy(mv_out, xl2[:, :, 0:ROW])
                    else:
                        nc.vector.tensor_copy(mv_out, xl2[:, :, 0:ROW])
                if to_table:
                    r = g * NB * P
                    if r < LO_ROWS:
                        sink = xl_lo_tab[r:r + NB * P]
                    else:
                        sink = xl_hi_tab[r - LO_ROWS:r - LO_ROWS + NB * P]
                    nc.sync.dma_start(
                        sink.rearrange("(t p) c -> p t c", p=P)[:, :, 0:ROW],
                        dst4[:])

            with tc.tile_pool(name="dsb", bufs=3) as dsb, \
                    tc.tile_pool(name="dps", bufs=2, space="PSUM") as dps:
                specs = ([(g, xg_own, wqr_sb, qr_sb, False) for g in range(GB)]
                         + [(g, xg_all, wql_sb, ql_sb, True)
                            for g in range(GA)])
                NG = len(specs)
                s1out = {}
                s2out = {}
                for i in range(NG + 2):
                    if i < NG:
                        g, xg, wq_sb, q_sb, tt = specs[i]
                        s1out[i] = dense_stage1(g, xg, wq_sb, q_sb, tt,
                                                dsb, dps)
                    if 1 <= i and i - 1 < NG:
                        hta, nrm2_ps = s1out[i - 1]
                        s2out[i - 1] = dense_stage2(hta, nrm2_ps, dsb, dps)
                    if 2 <= i and i - 2 < NG:
                        g, xg, wq_sb, q_sb, tt = specs[i - 2]
                        hta, _ = s1out.pop(i - 2)
                        dense_stage3(g, wq_sb, q_sb, tt, hta,
                                     s2out.pop(i - 2), dsb, dps)

            # ---------------- edge phase ----------------
            GB_T = 8            # tiles per dma_gather call (<=1024 indices)
            ZG = 4              # tiles per z-group (PSUM bank pair)

            with tc.tile_pool(name="esb", bufs=3) as esb, \
                    tc.tile_pool(name="msb", bufs=4) as msb, \
                    tc.tile_pool(name="zps", bufs=2, space="PSUM") as zps, \
                    tc.tile_pool(name="lps", bufs=2, space="PSUM") as lps, \
                    tc.tile_pool(name="aps", bufs=2, space="PSUM") as aps:

                logits_of = {}
                xlg_of = {}
                oh_of = {}
                expv_of = {}

                def front(c):
                    T = tch[c]
                    oh_sb = esb.tile([P, 2 * TMAX * P], FP8, tag="oh")
                    nc.sync.dma_start(oh_sb[:, 0:2 * T * P],
                                      ohcomb[:, ohS[c] * P:ohS[c + 1] * P])
                    xlg = esb.tile([P, TMAX, TROW], FP16, tag="xlg")
                    segs = [(tl[c], 0, int(loS[c]), xl_lo_tab, idxlo_sb)]
                    if th[c]:
                        segs.append((th[c], tl[c], int(hiS[c]), xl_hi_tab,
                                     idxhi_sb))
                    for t_seg, off, base, tab, idx_sb_ in segs:
                        for b in range(0, t_seg, GB_T):
                            nt = min(GB_T, t_seg - b)
                            nc.gpsimd.dma_gather(
                                out_ap=xlg[:, off + b:off + b + nt, :],
                                in_ap=tab[:],
                                idxs_ap=idx_sb_[:, (base + b) * 8:
                                                (base + b + nt) * 8],
                                num_idxs=nt * P, num_idxs_reg=nt * P,
                                elem_size=TROW, queue_num=next_q())
                    logits_ps = lps.tile([P, TMAX * HEADS], FP32,
                                         tag="logits")
                    expv = esb.tile([P, TMAX * HEADS], FP16, tag="expv")
                    exp_done = 0
                    groups = list(range(0, T, ZG))
                    for g0 in groups:
                        gl = min(ZG, T - g0)
                        zt = zps.tile([P, 2, ZG * P], FP32, tag="zt")
                        for gi in range(gl):
                            t = g0 + gi
                            esl = slice(gi * P, (gi + 1) * P)
                            ohsl = slice(t * P, (t + 1) * P)
                            for b in range(2):
                                bsl = slice(b * P, (b + 1) * P)
                                nc.tensor.matmul(
                                    out=zt[:, b, esl],
                                    lhsT=xr_res[:, c, bsl],
                                    rhs=oh_sb[:, ohsl],
                                    start=True, stop=False)
                                nc.tensor.matmul(
                                    out=zt[:, b, esl],
                                    lhsT=xlg[:, t, bsl],
                                    rhs=id_sb[:],
                                    start=False, stop=True)
                        wt = msb.tile([P, 2, ZG * P], FP16, tag="wt",
                                      bufs=3)
                        nc.scalar.activation(wt[:, :, 0:gl * P],
                                             zt[:, :, 0:gl * P], AF.Relu)
                        for gi in range(gl):
                            t = g0 + gi
                            esl = slice(gi * P, (gi + 1) * P)
                            lsl = slice(t * HEADS, (t + 1) * HEADS)
                            ohsl = slice(t * P, (t + 1) * P)
                            nc.tensor.matmul(
                                out=logits_ps[:, lsl], lhsT=id_sb[:],
                                rhs=xlg[:, t, FEAT:FEAT + HEADS],
                                start=True, stop=False)
                            nc.tensor.matmul(
                                out=logits_ps[:, lsl],
                                lhsT=oh_sb[:, ohsl],
                                rhs=xr_res[:, c, FEAT:FEAT + HEADS],
                                start=False, stop=False)
                            nc.tensor.matmul(
                                out=logits_ps[:, lsl], lhsT=wt[:, 0, esl],
                                rhs=att_sb[:, 0:HEADS],
                                start=False, stop=False)
                            nc.tensor.matmul(
                                out=logits_ps[:, lsl], lhsT=wt[:, 1, esl],
                                rhs=att_sb[:, HEADS:2 * HEADS],
                                start=False, stop=True)
                        gidx = g0 // ZG
                        done = min(g0 + gl, T)
                        if gidx % 2 == 1 or g0 == groups[-1]:
                            nc.scalar.activation(
                                expv[:, exp_done * HEADS:done * HEADS],
                                logits_ps[:, exp_done * HEADS:done * HEADS],
                                AF.Exp)
                            exp_done = done
                    xlg_of[c] = xlg
                    oh_of[c] = oh_sb
                    expv_of[c] = expv

                def back_b(c):
                    T = tch[c]
                    xlg = xlg_of.pop(c)
                    oh_sb = oh_of.pop(c)
                    expv = expv_of.pop(c)
                    agg = aps.tile([P, ROW], FP32, tag="agg")
                    JH = ROW // HEADS
                    for t0 in range(0, T, 2):
                        tn = min(2, T - t0)
                        msg = msb.tile([P, 2, ROW], FP16, tag="msg")
                        a4 = expv[:, t0 * HEADS:(t0 + tn) * HEADS] \
                            .rearrange("p (t o h) -> p t o h", t=tn, o=1) \
                            .to_broadcast([P, tn, JH, HEADS])
                        nc.vector.tensor_tensor(
                            msg[:, 0:tn, :].rearrange(
                                "p t (j h) -> p t j h", h=HEADS),
                            xlg[:, t0:t0 + tn, 0:ROW].rearrange(
                                "p t (j h) -> p t j h", h=HEADS),
                            a4, ALU.mult)
                        for ti in range(tn):
                            t = t0 + ti
                            nc.tensor.matmul(
                                out=agg[:],
                                lhsT=oh_sb[:, (T + t) * P:(T + t + 1) * P],
                                rhs=msg[:, ti, :], start=(t == 0),
                                stop=(t == T - 1))
                    den = msb.tile([P, HEADS], FP32, tag="den")
                    nc.vector.tensor_scalar_add(den[:], agg[:, ROW - HEADS:],
                                                1e-16)
                    dinv = msb.tile([P, HEADS], FP32, tag="dinv")
                    nc.vector.reciprocal(dinv[:], den[:])
                    scl = msb.tile([P, FEAT], FP32, tag="scl")
                    di4 = dinv[:].rearrange("p (o h) -> p o h", o=1) \
                        .to_broadcast([P, HID, HEADS])
                    nc.vector.tensor_tensor(
                        scl[:].rearrange("p (j h) -> p j h", h=HEADS),
                        agg[:, 0:FEAT].rearrange("p (j h) -> p j h", h=HEADS),
                        di4, ALU.mult)
                    ored = msb.tile([P, HID], FP32, tag="ored")
                    nc.vector.tensor_reduce(
                        out=ored[:],
                        in_=scl[:].rearrange("p (j h) -> p j h", h=HEADS),
                        axis=mybir.AxisListType.X, op=ALU.add)
                    obias = msb.tile([P, HID], FP32, tag="obias")
                    nc.vector.tensor_add(obias[:], ored[:], gbias_sb[:])
                    nc.scalar.activation(orelu_res[:, c, :], obias[:],
                                         AF.Relu)

                front(0)
                if C_CHUNKS > 1:
                    front(1)
                for c in range(C_CHUNKS):
                    if c + 2 < C_CHUNKS:
                        front(c + 2)
                    back_b(c)

            # ---------------- classifier phase ----------------
            with tc.tile_pool(name="fsb", bufs=2) as fsb, \
                    tc.tile_pool(name="fps", bufs=2, space="PSUM") as fps:
                for c0 in range(0, C_CHUNKS, NB):
                    cl = min(NB, C_CHUNKS - c0)
                    ot_ps = fps.tile([HID, NB * P], FP16, tag="ot_ps")
                    for ci in range(cl):
                        nc.tensor.transpose(
                            out=ot_ps[:, ci * P:(ci + 1) * P],
                            in_=orelu_res[:, c0 + ci, :], identity=id_sb[:])
                    ot16 = fsb.tile([HID, NB * P], FP16, tag="ot16")
                    nc.scalar.copy(ot16[:, 0:cl * P], ot_ps[:, 0:cl * P])
                    fin_ps = fps.tile([P, NB * OUT_DIM], FP32, tag="fin_ps")
                    for ci in range(cl):
                        nc.tensor.matmul(
                            out=fin_ps[:, ci * OUT_DIM:(ci + 1) * OUT_DIM],
                            lhsT=ot16[:, ci * P:(ci + 1) * P],
                            rhs=wcls_sb[:], start=True, stop=False)
                        nc.tensor.matmul(
                            out=fin_ps[:, ci * OUT_DIM:(ci + 1) * OUT_DIM],
                            lhsT=o1_sb[:],
                            rhs=bcls_sb[:, ci * OUT_DIM:(ci + 1) * OUT_DIM],
                            start=False, stop=True)
                    nc.vector.tensor_copy(
                        fin_res[:, c0:c0 + cl, :].rearrange(
                            "p c o -> p (c o)"),
                        fin_ps[:, 0:cl * OUT_DIM])
                nc.sync.dma_start(
                    out_ext[:].rearrange("p (c o) -> p c o", o=OUT_DIM),
                    fin_res[:])

    nc.compile()
    return nc


# ----------------------------------------------------------------------------
# Host-side data preparation
# ----------------------------------------------------------------------------

def prepare_host(x, edge_index, W_in, b_in, prototypes, W_l, b_l, W_r, b_r,
                 att, gat_bias, W_cls, b_cls, n_cores):
    x = np.asarray(x, np.float32)
    W_in = np.asarray(W_in, np.float32)
    b_in = np.asarray(b_in, np.float32)
    prototypes = np.asarray(prototypes, np.float32)
    W_l = np.asarray(W_l, np.float32)
    b_l = np.asarray(b_l, np.float32)
    W_r = np.asarray(W_r, np.float32)
    b_r = np.asarray(b_r, np.float32)
    att = np.asarray(att, np.float32)
    gat_bias = np.asarray(gat_bias, np.float32)
    W_cls = np.asarray(W_cls, np.float32)
    b_cls = np.asarray(b_cls, np.float32)

    n = x.shape[0]
    nodes_per_core = n // n_cores
    NB4 = NB * P

    n_nodes_pad = _cdiv(n, NB4) * NB4
    npc_dense = _cdiv(nodes_per_core, NB4) * NB4
    npc_chunks = _cdiv(nodes_per_core, P) * P
    c_chunks = npc_chunks // P

    # --- edge bucketing ---
    src = np.asarray(edge_index[0], dtype=np.int64)
    dst = np.asarray(edge_index[1], dtype=np.int64)
    loop = np.arange(n, dtype=np.int64)
    src = np.concatenate([src, loop])
    dst = np.concatenate([dst, loop])

    core = dst // nodes_per_core
    dstl = dst - core * nodes_per_core
    chunk = dstl // P
    seg = (src >= LO_ROWS).astype(np.int64)

    counts = np.zeros((n_cores, c_chunks, 2), dtype=np.int64)
    np.add.at(counts, (core, chunk, seg), 1)
    # ragged per-chunk tile counts: max over cores
    tl = [int(_cdiv(int(counts[:, c, 0].max()), P)) for c in range(c_chunks)]
    th = [int(_cdiv(int(counts[:, c, 1].max()), P)) for c in range(c_chunks)]
    tl = [max(t, 1) for t in tl]
    tch = [a + b for a, b in zip(tl, th)]
    loS = np.concatenate([[0], np.cumsum(tl)]).astype(np.int64)
    hiS = np.concatenate([[0], np.cumsum(th)]).astype(np.int64)
    ohS = np.concatenate([[0], np.cumsum([2 * t for t in tch])]) \
        .astype(np.int64)

    order = np.lexsort((seg, chunk, core))
    src_o, core_o, chunk_o, dstl_o, seg_o = (src[order], core[order],
                                             chunk[order], dstl[order],
                                             seg[order])
    bounds = np.zeros(n_cores * c_chunks * 2 + 1, dtype=np.int64)
    np.cumsum(counts.reshape(-1), out=bounds[1:])
    flat_bucket = (core_o * c_chunks + chunk_o) * 2 + seg_o
    pos = np.arange(len(src_o)) - bounds[flat_bucket]

    lo_slots = int(loS[-1]) * P
    hi_slots = int(hiS[-1]) * P
    idx_lo_slot = np.zeros((n_cores, lo_slots), dtype=np.int32)
    idx_hi_slot = np.zeros((n_cores, max(hi_slots, 1)), dtype=np.int32)
    # nloc in per-chunk tile space for one-hot build
    nloc_lo = np.full((n_cores, lo_slots), -1, dtype=np.int32)
    nloc_hi = np.full((n_cores, max(hi_slots, 1)), -1, dtype=np.int32)

    lo_base = loS[chunk_o] * P + pos
    hi_base = hiS[chunk_o] * P + pos
    is_lo = seg_o == 0
    idx_lo_slot[core_o[is_lo], lo_base[is_lo]] = src_o[is_lo].astype(np.int32)
    nloc_lo[core_o[is_lo], lo_base[is_lo]] = \
        (dstl_o[is_lo] - chunk_o[is_lo] * P).astype(np.int32)
    is_hi = ~is_lo
    idx_hi_slot[core_o[is_hi], hi_base[is_hi]] = \
        (src_o[is_hi] - LO_ROWS).astype(np.int32)
    nloc_hi[core_o[is_hi], hi_base[is_hi]] = \
        (dstl_o[is_hi] - chunk_o[is_hi] * P).astype(np.int32)

    def wrap16(vals):
        # [k, S*128] -> [k, 128, S*8] int16
        S = vals.shape[1] // P
        v = vals.reshape(n_cores, S * 8, 16)
        v = np.transpose(v, (0, 2, 1))
        v = np.tile(v, (1, 8, 1))
        return np.ascontiguousarray(v).astype(np.int16)

    idx_lo = wrap16(idx_lo_slot)
    idx_hi = wrap16(idx_hi_slot) if hi_slots else None

    # --- one-hot (fp8), ragged layout ---
    iota = np.arange(P, dtype=np.int32)
    ohcomb = np.zeros((n_cores, P, int(ohS[-1]) * P), dtype=f8)
    for c in range(c_chunks):
        T = tch[c]
        nl = np.concatenate(
            [nloc_lo[:, loS[c] * P:loS[c + 1] * P],
             nloc_hi[:, hiS[c] * P:hiS[c + 1] * P]], axis=1) \
            .reshape(n_cores, T, P)
        oh = (nl[..., None] == iota)                 # [k, t, e, n]
        base = int(ohS[c]) * P
        ohcomb[:, :, base:base + T * P] = np.transpose(oh, (0, 3, 1, 2)) \
            .reshape(n_cores, P, T * P)
        ohcomb[:, :, base + T * P:base + 2 * T * P] = \
            np.transpose(oh, (0, 2, 1, 3)).reshape(n_cores, P, T * P)

    # --- weights ---
    att_blk = np.zeros((FEAT, HEADS), dtype=np.float32)
    for h in range(HEADS):
        att_blk[h * HID:(h + 1) * HID, h] = att[h]
    p_norm = prototypes / (np.linalg.norm(prototypes, axis=1, keepdims=True)
                           + 1e-12)
    Q_l = p_norm.T @ W_l[HID:HID + 2]
    Q_r = p_norm.T @ W_r[HID:HID + 2]

    perm = np.zeros(FEAT, np.int64)
    for h in range(HEADS):
        for j in range(HID):
            perm[j * HEADS + h] = h * HID + j

    def ext264(w, b, with_ones):
        w264 = np.concatenate(
            [w[:, perm], 0.2 * (w @ att_blk),
             np.zeros((w.shape[0], HEADS), np.float32)], axis=1)
        ones = np.ones(HEADS, np.float32) if with_ones else \
            np.zeros(HEADS, np.float32)
        b264 = np.concatenate([b[perm], 0.2 * (b @ att_blk), ones])[None, :]
        return w264, b264

    wl264, bl264 = ext264(W_l[:HID], b_l, True)
    ql264, _ = ext264(Q_l, b_l * 0, False)
    wr264, br264 = ext264(W_r[:HID], b_r, True)
    qr264, _ = ext264(Q_r, b_r * 0, False)
    wql_ = np.concatenate([wl264, bl264], axis=0).astype(f16)
    wqr_ = np.concatenate([wr264, br264], axis=0).astype(f16)

    att8 = np.zeros((P, 2 * HEADS), np.float32)
    for b in range(2):
        for p in range(P):
            cprime = b * P + p
            j, h = cprime // HEADS, cprime % HEADS
            att8[p, b * HEADS + h] = 0.8 * att[h, j]

    def swizzle(xa, npad):
        G = npad // NB4
        xp = np.zeros((npad, IN_DIM), dtype=np.float32)
        xp[:len(xa)] = xa
        v = xp.reshape(G, NB, P, 2, P)
        v = np.transpose(v, (0, 4, 3, 1, 2))
        return np.ascontiguousarray(v.reshape(G, P, 2, NB * P)).astype(f16)

    xg_all = swizzle(x, n_nodes_pad)
    xg_own = [swizzle(x[k * nodes_per_core:(k + 1) * nodes_per_core],
                      npc_dense) for k in range(n_cores)]

    shared = {
        "xg_all": xg_all,
        "w_in_a": W_in[:P].astype(f16), "w_in_b": W_in[P:].astype(f16),
        "b_in_c": b_in[:, None].astype(f16),
        "wql": wql_, "ql": ql264.astype(f16),
        "wqr": wqr_, "qr": qr264.astype(f16),
        "att8": att8.astype(f16),
        "ident": np.eye(P, dtype=f16),
        "ones6464": np.ones((HID, HID), f16),
        "o1_128": np.ones((1, P), f16),
        "gbias4": np.broadcast_to(4.0 * gat_bias.astype(np.float32),
                                  (P, HID)).copy(),
        "wcls4": (0.25 * W_cls).astype(f16),
        "bcls4": np.tile(b_cls, NB)[None, :].astype(f16),
    }
    in_maps = []
    for k in range(n_cores):
        m = dict(shared)
        m["xg_own"] = xg_own[k]
        m["idx_lo"] = idx_lo[k]
        if idx_hi is not None:
            m["idx_hi"] = idx_hi[k]
        m["ohcomb"] = ohcomb[k]
        in_maps.append(m)
    return (in_maps, n_nodes_pad, npc_dense, npc_chunks, tuple(tl),
            tuple(th))


_CACHE = {}


def run(inputs, n_cores=8, trace=False):
    x = np.asarray(inputs["x"])
    n = x.shape[0]
    in_maps, n_nodes_pad, npc_dense, npc_chunks, tl, th = prepare_host(
        x, np.asarray(inputs["edge_index"]), np.asarray(inputs["W_in"]),
        np.asarray(inputs["b_in"]), np.asarray(inputs["prototypes"]),
        np.asarray(inputs["W_l"]), np.asarray(inputs["b_l"]),
        np.asarray(inputs["W_r"]), np.asarray(inputs["b_r"]),
        np.asarray(inputs["att"]), np.asarray(inputs["gat_bias"]),
        np.asarray(inputs["W_cls"]), np.asarray(inputs["b_cls"]), n_cores)
    key = (n_nodes_pad, npc_dense, npc_chunks, tl, th, n_cores)
    if key not in _CACHE:
        _CACHE[key] = build_program(*key)
    nc = _CACHE[key]
    res = run_bass_kernel_spmd(nc, in_maps, list(range(n_cores)), trace=trace)
    npc = n // n_cores
    c_chunks = npc_chunks // P
    outs = []
    for k in range(n_cores):
        o = np.asarray(res.results[k]["out"]).reshape(P, c_chunks, OUT_DIM)
        outs.append(np.transpose(o, (1, 0, 2)).reshape(npc_chunks,
                                                       OUT_DIM)[:npc])
    return np.concatenate(outs, axis=0), res


def kernel(**inputs):
    out, _ = run(inputs, n_cores=8)
    return out.astype(np.float32)


# revision 30
# speedup vs baseline: 2.8732x; 1.1423x over previous
"""GATv2-based CGNN forward pass on 8 Trainium2 NeuronCores.

Strategy (dst-node sharded, no collectives):
  - Each core owns N/8 destination nodes. Host buckets edges (incl. self
    loops) by dst core, then by 128-node dst chunk within the core.
    Per-chunk tile counts are ragged (max over cores per chunk index) so
    pad work tracks the actual edge distribution.
  - Dense phase (replicated for xl over all nodes; own nodes for xr):
    xl/xr rows are [feat256 head-interleaved (c' = j*4+h), beta4, ones4]
    fp16, where beta = 0.2*(feat @ att_blk) so that
      logit = 0.8*sum_c att_c*relu(z_c) + beta_l[s] + beta_r[d]
    (exact rewrite of att . leaky_relu via |z| = 2 relu(z) - z).
  - Edge phase per 128-dst chunk: batched indirect-DMA gather of xl[src]
    rows (4 SWDGE queues); z built TRANSPOSED in PSUM ([channel, edge])
    from a one-hot matmul of resident xr plus identity-matmul transpose
    of the gathered xl; one relu Activation moves it to SBUF; per-tile
    logits come from tall-skinny matmuls with the relu'd block as lhsT.
    Messages are one fp16 tensor_tensor multiply (broadcast alpha),
    scatter-added via fp8 one-hot matmuls; denominators ride along as
    ones*alpha columns.
  - Finish: per-chunk softmax normalize + head mean (0.25 folded into
    W_cls), relu; classifier runs as a final batched phase.
"""

import os
import sys

import numpy as np
import ml_dtypes

for _p in ("/opt/trn_rl_repo",):
    if _p not in sys.path and os.path.isdir(_p):
        sys.path.insert(0, _p)

import concourse.bass as bass
import concourse.tile as tile
from concourse import bacc, mybir
from concourse.bass_utils import run_bass_kernel_spmd

FP16 = mybir.dt.float16
FP32 = mybir.dt.float32
FP8 = mybir.dt.float8e4
INT16 = mybir.dt.int16
AF = mybir.ActivationFunctionType
ALU = mybir.AluOpType

P = 128
HID = 64
HEADS = 4
OUT_DIM = 16
IN_DIM = 256
FEAT = HEADS * HID          # 256
ROW = FEAT + 2 * HEADS      # 264 = feat + beta + ones
TROW = 384                  # padded table row (768B, 256B-aligned)
LO_ROWS = 32768             # int16 index range per gather table
NB = 4                      # node tiles per dense group

f16 = ml_dtypes.float16 if hasattr(ml_dtypes, "float16") else np.float16
f8 = ml_dtypes.float8_e4m3


def _cdiv(a, b):
    return (a + b - 1) // b


# ----------------------------------------------------------------------------
# Device program
# ----------------------------------------------------------------------------

def build_program(n_nodes_pad, npc_dense, npc_chunks, tl, th, n_cores):
    """tl/th: per-chunk lo/hi gather tile counts (tuples, shared by cores)."""
    GA = n_nodes_pad // (NB * P)
    GB = npc_dense // (NB * P)
    C_CHUNKS = npc_chunks // P
    tl = list(tl)
    th = list(th)
    tch = [a + b for a, b in zip(tl, th)]
    TMAX = max(tch)
    loS = np.concatenate([[0], np.cumsum(tl)]).astype(int)   # tile offsets
    hiS = np.concatenate([[0], np.cumsum(th)]).astype(int)
    ohS = np.concatenate([[0], np.cumsum([2 * t for t in tch])]).astype(int)
    hi_rows = max(n_nodes_pad - LO_ROWS, P)
    xr_slots = GB * NB

    nc = bacc.Bacc("TRN2", target_bir_lowering=False, debug=False,
                   num_devices=n_cores, num_swdge_queues=4)

    def din(name, shape, dtype=FP16):
        return nc.dram_tensor(name, shape, dtype, kind="ExternalInput").ap()

    xg_all = din("xg_all", [GA, P, 2, NB * P])
    xg_own = din("xg_own", [GB, P, 2, NB * P])
    w_in_a = din("w_in_a", [P, HID])
    w_in_b = din("w_in_b", [P, HID])
    b_in_c = din("b_in_c", [HID, 1])
    wql = din("wql", [HID + 1, ROW])       # [w264; b264]
    ql = din("ql", [HID, ROW])
    wqr = din("wqr", [HID + 1, ROW])
    qr = din("qr", [HID, ROW])
    att8 = din("att8", [P, 2 * HEADS])
    ident = din("ident", [P, P])
    ones6464 = din("ones6464", [HID, HID])
    o1_128 = din("o1_128", [1, P])
    gbias4 = din("gbias4", [P, HID], FP32)
    wcls4 = din("wcls4", [HID, OUT_DIM])
    bcls4 = din("bcls4", [1, NB * OUT_DIM])
    idx_lo = din("idx_lo", [P, int(loS[-1]) * 8], INT16)
    idx_hi = (din("idx_hi", [P, int(hiS[-1]) * 8], INT16)
              if hiS[-1] else None)
    ohcomb = din("ohcomb", [P, int(ohS[-1]) * P], FP8)

    out_ext = nc.dram_tensor("out", [P, C_CHUNKS * OUT_DIM], FP32,
                             kind="ExternalOutput").ap()

    xl_lo_tab = nc.dram_tensor("xl_lo_tab", [min(n_nodes_pad, LO_ROWS), TROW],
                               FP16).ap()
    xl_hi_tab = nc.dram_tensor("xl_hi_tab", [hi_rows, TROW], FP16).ap()

    qctr = [0]

    def next_q():
        qctr[0] = (qctr[0] + 1) % 4
        return qctr[0]

    with tile.TileContext(nc) as tc:
        with tc.tile_pool(name="consts", bufs=1) as cp:
            w_in_a_sb = cp.tile([P, HID], FP16)
            nc.sync.dma_start(w_in_a_sb[:], w_in_a[:])
            w_in_b_sb = cp.tile([P, HID], FP16)
            nc.sync.dma_start(w_in_b_sb[:], w_in_b[:])
            b_in_sb = cp.tile([HID, 1], FP16)
            nc.sync.dma_start(b_in_sb[:], b_in_c[:])
            wql_sb = cp.tile([HID + 1, ROW], FP16)
            nc.sync.dma_start(wql_sb[:], wql[:])
            ql_sb = cp.tile([HID, ROW], FP16)
            nc.sync.dma_start(ql_sb[:], ql[:])
            wqr_sb = cp.tile([HID + 1, ROW], FP16)
            nc.sync.dma_start(wqr_sb[:], wqr[:])
            qr_sb = cp.tile([HID, ROW], FP16)
            nc.sync.dma_start(qr_sb[:], qr[:])
            att_sb = cp.tile([P, 2 * HEADS], FP16)
            nc.sync.dma_start(att_sb[:], att8[:])
            id_sb = cp.tile([P, P], FP16)
            nc.sync.dma_start(id_sb[:], ident[:])
            ones64_sb = cp.tile([HID, HID], FP16)
            nc.sync.dma_start(ones64_sb[:], ones6464[:])
            o1_sb = cp.tile([1, P], FP16)
            nc.sync.dma_start(o1_sb[:], o1_128[:])
            gbias_sb = cp.tile([P, HID], FP32)
            nc.sync.dma_start(gbias_sb[:], gbias4[:])
            wcls_sb = cp.tile([HID, OUT_DIM], FP16)
            nc.sync.dma_start(wcls_sb[:], wcls4[:])
            bcls_sb = cp.tile([1, NB * OUT_DIM], FP16)
            nc.sync.dma_start(bcls_sb[:], bcls4[:])
            idxlo_sb = cp.tile([P, int(loS[-1]) * 8], INT16)
            nc.sync.dma_start(idxlo_sb[:], idx_lo[:])
            if idx_hi is not None:
                idxhi_sb = cp.tile([P, int(hiS[-1]) * 8], INT16)
                nc.sync.dma_start(idxhi_sb[:], idx_hi[:])
            eps_sb = cp.tile([HID, 1], FP32)
            nc.gpsimd.memset(eps_sb[:], 1e-12)

            xr_res = cp.tile([P, xr_slots, ROW], FP16)
            orelu_res = cp.tile([P, C_CHUNKS, HID], FP16)
            fin_res = cp.tile([P, C_CHUNKS, OUT_DIM], FP32)

            # ---------------- dense phase ----------------
            W = NB * P

            def dense_stage1(g, xg, wq_sb, q_sb, to_table, sb, ps):
                xsb = sb.tile([P, 2, W], FP16, tag="xsb")
                nc.sync.dma_start(xsb[:], xg[g])
                ht_ps = ps.tile([HID, W], FP32, tag="ht_ps")
                nc.tensor.matmul(out=ht_ps[:], lhsT=w_in_a_sb[:],
                                 rhs=xsb[:, 0, :], start=True, stop=False)
                nc.tensor.matmul(out=ht_ps[:], lhsT=w_in_b_sb[:],
                                 rhs=xsb[:, 1, :], start=False, stop=True)
                hta = sb.tile([HID + 1, W], FP16, tag="hta", bufs=4)
                nc.scalar.activation(hta[0:HID, :], ht_ps[:], AF.Relu,
                                     bias=b_in_sb[:])
                nc.gpsimd.memset(hta[HID:HID + 1, :], 1.0)
                rsq = sb.tile([HID, W], FP16, tag="rsq")
                nc.vector.tensor_mul(rsq[:], hta[0:HID, :], hta[0:HID, :])
                nrm2_ps = ps.tile([HID, W], FP32, tag="nrm2_ps")
                nc.tensor.matmul(out=nrm2_ps[:], lhsT=ones64_sb[:],
                                 rhs=rsq[:], start=True, stop=True)
                return hta, nrm2_ps

            def dense_stage2(hta, nrm2_ps, sb, ps):
                nrmr = sb.tile([HID, W], FP32, tag="nrmr")
                nc.scalar.activation(nrmr[:], nrm2_ps[:], AF.Sqrt,
                                     bias=eps_sb[:])
                invr = sb.tile([HID, W], FP32, tag="invr")
                nc.vector.reciprocal(invr[:], nrmr[:])
                htn = sb.tile([HID, W], FP16, tag="htn", bufs=3)
                nc.gpsimd.tensor_mul(htn[:], hta[0:HID, :], invr[:])
                return htn

            def dense_stage3(g, wq_sb, q_sb, to_table, hta, htn, sb, ps):
                if to_table:
                    dst4 = sb.tile([P, NB, ROW], FP16, tag="dst4",
                                   name="dst4")
                for half in range(2):
                    xl2 = ps.tile([P, 2, 2 * ROW - 16], FP32, tag="xl2")
                    for ti in range(2):
                        t = half * 2 + ti
                        tsl = slice(t * P, (t + 1) * P)
                        nc.tensor.matmul(out=xl2[:, ti, 0:ROW],
                                         lhsT=hta[:, tsl],
                                         rhs=wq_sb[:], start=True, stop=False)
                        nc.tensor.matmul(out=xl2[:, ti, 0:ROW],
                                         lhsT=htn[:, tsl],
                                         rhs=q_sb[:], start=False, stop=True)
                    if to_table:
                        mv_out = dst4[:, half * 2:half * 2 + 2, :]
                    else:
                        mv_out = xr_res[:, g * NB + half * 2:
                                        g * NB + half * 2 + 2, :]
                    # alternate engines for the PSUM->SBUF move
                    if half == 0:
                        nc.scalar.copy(mv_out, xl2[:, :, 0:ROW])
                    else:
                        nc.vector.tensor_copy(mv_out, xl2[:, :, 0:ROW])
                if to_table:
                    r = g * NB * P
                    if r < LO_ROWS:
                        sink = xl_lo_tab[r:r + NB * P]
                    else:
                        sink = xl_hi_tab[r - LO_ROWS:r - LO_ROWS + NB * P]
                    nc.sync.dma_start(
                        sink.rearrange("(t p) c -> p t c", p=P)[:, :, 0:ROW],
                        dst4[:])

            with tc.tile_pool(name="dsb", bufs=3) as dsb, \
                    tc.tile_pool(name="dps", bufs=2, space="PSUM") as dps:
                specs = ([(g, xg_own, wqr_sb, qr_sb, False) for g in range(GB)]
                         + [(g, xg_all, wql_sb, ql_sb, True)
                            for g in range(GA)])
                NG = len(specs)
                s1out = {}
                s2out = {}
                for i in range(NG + 2):
                    if i < NG:
                        g, xg, wq_sb, q_sb, tt = specs[i]
                        s1out[i] = dense_stage1(g, xg, wq_sb, q_sb, tt,
                                                dsb, dps)
                    if 1 <= i and i - 1 < NG:
                        hta, nrm2_ps = s1out[i - 1]
                        s2out[i - 1] = dense_stage2(hta, nrm2_ps, dsb, dps)
                    if 2 <= i and i - 2 < NG:
                        g, xg, wq_sb, q_sb, tt = specs[i - 2]
                        hta, _ = s1out.pop(i - 2)
                        dense_stage3(g, wq_sb, q_sb, tt, hta,
                                     s2out.pop(i - 2), dsb, dps)

            # ---------------- edge phase ----------------
            GB_T = 8            # tiles per dma_gather call (<=1024 indices)
            ZG = 4              # tiles per z-group (PSUM bank pair)

            with tc.tile_pool(name="esb", bufs=3) as esb, \
                    tc.tile_pool(name="msb", bufs=4) as msb, \
                    tc.tile_pool(name="zps", bufs=2, space="PSUM") as zps, \
                    tc.tile_pool(name="lps", bufs=1, space="PSUM") as lps, \
                    tc.tile_pool(name="aps", bufs=2, space="PSUM") as aps:

                xlg_of = {}
                oh_of = {}
                expv_of = {}

                def front(c):
                    T = tch[c]
                    oh_sb = esb.tile([P, 2 * TMAX * P], FP8, tag="oh", bufs=4)
                    nc.sync.dma_start(oh_sb[:, 0:2 * T * P],
                                      ohcomb[:, ohS[c] * P:ohS[c + 1] * P])
                    xlg = esb.tile([P, TMAX, TROW], FP16, tag="xlg", bufs=4)
                    segs = [(tl[c], 0, int(loS[c]), xl_lo_tab, idxlo_sb)]
                    if th[c]:
                        segs.append((th[c], tl[c], int(hiS[c]), xl_hi_tab,
                                     idxhi_sb))
                    for t_seg, off, base, tab, idx_sb_ in segs:
                        for b in range(0, t_seg, GB_T):
                            nt = min(GB_T, t_seg - b)
                            nc.gpsimd.dma_gather(
                                out_ap=xlg[:, off + b:off + b + nt, :],
                                in_ap=tab[:],
                                idxs_ap=idx_sb_[:, (base + b) * 8:
                                                (base + b + nt) * 8],
                                num_idxs=nt * P, num_idxs_reg=nt * P,
                                elem_size=TROW, queue_num=next_q())
                    logits_ps = lps.tile([P, TMAX * HEADS], FP32,
                                         tag="logits")
                    expv = esb.tile([P, TMAX * HEADS], FP16, tag="expv", bufs=4)
                    exp_done = 0
                    groups = list(range(0, T, ZG))
                    for g0 in groups:
                        gl = min(ZG, T - g0)
                        zt = zps.tile([P, 2, ZG * P], FP32, tag="zt")
                        for gi in range(gl):
                            t = g0 + gi
                            esl = slice(gi * P, (gi + 1) * P)
                            ohsl = slice(t * P, (t + 1) * P)
                            for b in range(2):
                                bsl = slice(b * P, (b + 1) * P)
                                nc.tensor.matmul(
                                    out=zt[:, b, esl],
                                    lhsT=xr_res[:, c, bsl],
                                    rhs=oh_sb[:, ohsl],
                                    start=True, stop=False)
                                nc.tensor.matmul(
                                    out=zt[:, b, esl],
                                    lhsT=xlg[:, t, bsl],
                                    rhs=id_sb[:],
                                    start=False, stop=True)
                        wt = msb.tile([P, 2, ZG * P], FP16, tag="wt",
                                      bufs=3)
                        nc.scalar.activation(wt[:, :, 0:gl * P],
                                             zt[:, :, 0:gl * P], AF.Relu)
                        for gi in range(gl):
                            t = g0 + gi
                            esl = slice(gi * P, (gi + 1) * P)
                            lsl = slice(t * HEADS, (t + 1) * HEADS)
                            ohsl = slice(t * P, (t + 1) * P)
                            nc.tensor.matmul(
                                out=logits_ps[:, lsl], lhsT=id_sb[:],
                                rhs=xlg[:, t, FEAT:FEAT + HEADS],
                                start=True, stop=False)
                            nc.tensor.matmul(
                                out=logits_ps[:, lsl],
                                lhsT=oh_sb[:, ohsl],
                                rhs=xr_res[:, c, FEAT:FEAT + HEADS],
                                start=False, stop=False)
                            nc.tensor.matmul(
                                out=logits_ps[:, lsl], lhsT=wt[:, 0, esl],
                                rhs=att_sb[:, 0:HEADS],
                                start=False, stop=False)
                            nc.tensor.matmul(
                                out=logits_ps[:, lsl], lhsT=wt[:, 1, esl],
                                rhs=att_sb[:, HEADS:2 * HEADS],
                                start=False, stop=True)
                        gidx = g0 // ZG
                        done = min(g0 + gl, T)
                        if gidx % 2 == 1 or g0 == groups[-1]:
                            nc.scalar.activation(
                                expv[:, exp_done * HEADS:done * HEADS],
                                logits_ps[:, exp_done * HEADS:done * HEADS],
                                AF.Exp)
                            exp_done = done
                    xlg_of[c] = xlg
                    oh_of[c] = oh_sb
                    expv_of[c] = expv

                def back_b(c):
                    T = tch[c]
                    xlg = xlg_of.pop(c)
                    oh_sb = oh_of.pop(c)
                    expv = expv_of.pop(c)
                    agg = aps.tile([P, ROW], FP32, tag="agg")
                    JH = ROW // HEADS
                    for t0 in range(0, T, 2):
                        tn = min(2, T - t0)
                        msg = msb.tile([P, 2, ROW], FP16, tag="msg")
                        a4 = expv[:, t0 * HEADS:(t0 + tn) * HEADS] \
                            .rearrange("p (t o h) -> p t o h", t=tn, o=1) \
                            .to_broadcast([P, tn, JH, HEADS])
                        nc.vector.tensor_tensor(
                            msg[:, 0:tn, :].rearrange(
                                "p t (j h) -> p t j h", h=HEADS),
                            xlg[:, t0:t0 + tn, 0:ROW].rearrange(
                                "p t (j h) -> p t j h", h=HEADS),
                            a4, ALU.mult)
                        for ti in range(tn):
                            t = t0 + ti
                            nc.tensor.matmul(
                                out=agg[:],
                                lhsT=oh_sb[:, (T + t) * P:(T + t + 1) * P],
                                rhs=msg[:, ti, :], start=(t == 0),
                                stop=(t == T - 1))
                    den = msb.tile([P, HEADS], FP32, tag="den")
                    nc.vector.tensor_scalar_add(den[:], agg[:, ROW - HEADS:],
                                                1e-16)
                    dinv = msb.tile([P, HEADS], FP32, tag="dinv")
                    nc.vector.reciprocal(dinv[:], den[:])
                    scl = msb.tile([P, FEAT], FP32, tag="scl")
                    di4 = dinv[:].rearrange("p (o h) -> p o h", o=1) \
                        .to_broadcast([P, HID, HEADS])
                    nc.vector.tensor_tensor(
                        scl[:].rearrange("p (j h) -> p j h", h=HEADS),
                        agg[:, 0:FEAT].rearrange("p (j h) -> p j h", h=HEADS),
                        di4, ALU.mult)
                    ored = msb.tile([P, HID], FP32, tag="ored")
                    nc.vector.tensor_reduce(
                        out=ored[:],
                        in_=scl[:].rearrange("p (j h) -> p j h", h=HEADS),
                        axis=mybir.AxisListType.X, op=ALU.add)
                    obias = msb.tile([P, HID], FP32, tag="obias")
                    nc.vector.tensor_add(obias[:], ored[:], gbias_sb[:])
                    nc.scalar.activation(orelu_res[:, c, :], obias[:],
                                         AF.Relu)

                def classify(c0, cl, fsb, fps):
                    # one PSUM bank: ot at fp16 cols 0:256, fin as fp32
                    # view of fp16 cols 512:576
                    cls_ps = fps.tile([P, 1024], FP16, tag="cls_ps")
                    for ci in range(cl):
                        nc.tensor.transpose(
                            out=cls_ps[0:HID, ci * P:(ci + 1) * P],
                            in_=orelu_res[:, c0 + ci, :], identity=id_sb[:])
                    ot16 = fsb.tile([HID, 2 * P], FP16, tag="ot16")
                    nc.scalar.copy(ot16[:, 0:cl * P],
                                   cls_ps[0:HID, 0:cl * P])
                    fin_ps = cls_ps[:, 512:576].bitcast(FP32)
                    for ci in range(cl):
                        nc.tensor.matmul(
                            out=fin_ps[:, ci * OUT_DIM:(ci + 1) * OUT_DIM],
                            lhsT=ot16[:, ci * P:(ci + 1) * P],
                            rhs=wcls_sb[:], start=True, stop=False)
                        nc.tensor.matmul(
                            out=fin_ps[:, ci * OUT_DIM:(ci + 1) * OUT_DIM],
                            lhsT=o1_sb[:],
                            rhs=bcls_sb[:, ci * OUT_DIM:(ci + 1) * OUT_DIM],
                            start=False, stop=True)
                    nc.vector.tensor_copy(
                        fin_res[:, c0:c0 + cl, :].rearrange(
                            "p c o -> p (c o)"),
                        fin_ps[:, 0:cl * OUT_DIM])

                with tc.tile_pool(name="fsb", bufs=2) as fsb, \
                        tc.tile_pool(name="fps", bufs=1,
                                     space="PSUM") as fps2:
                    front(0)
                    if C_CHUNKS > 1:
                        front(1)
                    for c in range(C_CHUNKS):
                        if c + 2 < C_CHUNKS:
                            front(c + 2)
                        back_b(c)
                        if c % 2 == 1:
                            classify(c - 1, 2, fsb, fps2)
                        if c % 8 == 7:
                            nc.sync.dma_start(
                                out_ext[:, (c - 7) * OUT_DIM:
                                        (c + 1) * OUT_DIM].rearrange(
                                    "p (c o) -> p c o", o=OUT_DIM),
                                fin_res[:, c - 7:c + 1, :])
                    if C_CHUNKS % 2:
                        classify(C_CHUNKS - 1, 1, fsb, fps2)
                    rem0 = (C_CHUNKS // 8) * 8
                    if rem0 < C_CHUNKS:
                        nc.sync.dma_start(
                            out_ext[:, rem0 * OUT_DIM:].rearrange(
                                "p (c o) -> p c o", o=OUT_DIM),
                            fin_res[:, rem0:, :])

    nc.compile()
    return nc


# ----------------------------------------------------------------------------
# Host-side data preparation
# ----------------------------------------------------------------------------

def prepare_host(x, edge_index, W_in, b_in, prototypes, W_l, b_l, W_r, b_r,
                 att, gat_bias, W_cls, b_cls, n_cores):
    x = np.asarray(x, np.float32)
    W_in = np.asarray(W_in, np.float32)
    b_in = np.asarray(b_in, np.float32)
    prototypes = np.asarray(prototypes, np.float32)
    W_l = np.asarray(W_l, np.float32)
    b_l = np.asarray(b_l, np.float32)
    W_r = np.asarray(W_r, np.float32)
    b_r = np.asarray(b_r, np.float32)
    att = np.asarray(att, np.float32)
    gat_bias = np.asarray(gat_bias, np.float32)
    W_cls = np.asarray(W_cls, np.float32)
    b_cls = np.asarray(b_cls, np.float32)

    n = x.shape[0]
    nodes_per_core = n // n_cores
    NB4 = NB * P

    n_nodes_pad = _cdiv(n, NB4) * NB4
    npc_dense = _cdiv(nodes_per_core, NB4) * NB4
    npc_chunks = _cdiv(nodes_per_core, P) * P
    c_chunks = npc_chunks // P

    # --- edge bucketing ---
    src = np.asarray(edge_index[0], dtype=np.int64)
    dst = np.asarray(edge_index[1], dtype=np.int64)
    loop = np.arange(n, dtype=np.int64)
    src = np.concatenate([src, loop])
    dst = np.concatenate([dst, loop])

    core = dst // nodes_per_core
    dstl = dst - core * nodes_per_core
    chunk = dstl // P
    seg = (src >= LO_ROWS).astype(np.int64)

    counts = np.zeros((n_cores, c_chunks, 2), dtype=np.int64)
    np.add.at(counts, (core, chunk, seg), 1)
    # ragged per-chunk tile counts: max over cores
    tl = [int(_cdiv(int(counts[:, c, 0].max()), P)) for c in range(c_chunks)]
    th = [int(_cdiv(int(counts[:, c, 1].max()), P)) for c in range(c_chunks)]
    tl = [max(t, 1) for t in tl]
    tch = [a + b for a, b in zip(tl, th)]
    loS = np.concatenate([[0], np.cumsum(tl)]).astype(np.int64)
    hiS = np.concatenate([[0], np.cumsum(th)]).astype(np.int64)
    ohS = np.concatenate([[0], np.cumsum([2 * t for t in tch])]) \
        .astype(np.int64)

    order = np.lexsort((seg, chunk, core))
    src_o, core_o, chunk_o, dstl_o, seg_o = (src[order], core[order],
                                             chunk[order], dstl[order],
                                             seg[order])
    bounds = np.zeros(n_cores * c_chunks * 2 + 1, dtype=np.int64)
    np.cumsum(counts.reshape(-1), out=bounds[1:])
    flat_bucket = (core_o * c_chunks + chunk_o) * 2 + seg_o
    pos = np.arange(len(src_o)) - bounds[flat_bucket]

    lo_slots = int(loS[-1]) * P
    hi_slots = int(hiS[-1]) * P
    idx_lo_slot = np.zeros((n_cores, lo_slots), dtype=np.int32)
    idx_hi_slot = np.zeros((n_cores, max(hi_slots, 1)), dtype=np.int32)
    # nloc in per-chunk tile space for one-hot build
    nloc_lo = np.full((n_cores, lo_slots), -1, dtype=np.int32)
    nloc_hi = np.full((n_cores, max(hi_slots, 1)), -1, dtype=np.int32)

    lo_base = loS[chunk_o] * P + pos
    hi_base = hiS[chunk_o] * P + pos
    is_lo = seg_o == 0
    idx_lo_slot[core_o[is_lo], lo_base[is_lo]] = src_o[is_lo].astype(np.int32)
    nloc_lo[core_o[is_lo], lo_base[is_lo]] = \
        (dstl_o[is_lo] - chunk_o[is_lo] * P).astype(np.int32)
    is_hi = ~is_lo
    idx_hi_slot[core_o[is_hi], hi_base[is_hi]] = \
        (src_o[is_hi] - LO_ROWS).astype(np.int32)
    nloc_hi[core_o[is_hi], hi_base[is_hi]] = \
        (dstl_o[is_hi] - chunk_o[is_hi] * P).astype(np.int32)

    def wrap16(vals):
        # [k, S*128] -> [k, 128, S*8] int16
        S = vals.shape[1] // P
        v = vals.reshape(n_cores, S * 8, 16)
        v = np.transpose(v, (0, 2, 1))
        v = np.tile(v, (1, 8, 1))
        return np.ascontiguousarray(v).astype(np.int16)

    idx_lo = wrap16(idx_lo_slot)
    idx_hi = wrap16(idx_hi_slot) if hi_slots else None

    # --- one-hot (fp8), ragged layout ---
    iota = np.arange(P, dtype=np.int32)
    ohcomb = np.zeros((n_cores, P, int(ohS[-1]) * P), dtype=f8)
    for c in range(c_chunks):
        T = tch[c]
        nl = np.concatenate(
            [nloc_lo[:, loS[c] * P:loS[c + 1] * P],
             nloc_hi[:, hiS[c] * P:hiS[c + 1] * P]], axis=1) \
            .reshape(n_cores, T, P)
        oh = (nl[..., None] == iota)                 # [k, t, e, n]
        base = int(ohS[c]) * P
        ohcomb[:, :, base:base + T * P] = np.transpose(oh, (0, 3, 1, 2)) \
            .reshape(n_cores, P, T * P)
        ohcomb[:, :, base + T * P:base + 2 * T * P] = \
            np.transpose(oh, (0, 2, 1, 3)).reshape(n_cores, P, T * P)

    # --- weights ---
    att_blk = np.zeros((FEAT, HEADS), dtype=np.float32)
    for h in range(HEADS):
        att_blk[h * HID:(h + 1) * HID, h] = att[h]
    p_norm = prototypes / (np.linalg.norm(prototypes, axis=1, keepdims=True)
                           + 1e-12)
    Q_l = p_norm.T @ W_l[HID:HID + 2]
    Q_r = p_norm.T @ W_r[HID:HID + 2]

    perm = np.zeros(FEAT, np.int64)
    for h in range(HEADS):
        for j in range(HID):
            perm[j * HEADS + h] = h * HID + j

    def ext264(w, b, with_ones):
        w264 = np.concatenate(
            [w[:, perm], 0.2 * (w @ att_blk),
             np.zeros((w.shape[0], HEADS), np.float32)], axis=1)
        ones = np.ones(HEADS, np.float32) if with_ones else \
            np.zeros(HEADS, np.float32)
        b264 = np.concatenate([b[perm], 0.2 * (b @ att_blk), ones])[None, :]
        return w264, b264

    wl264, bl264 = ext264(W_l[:HID], b_l, True)
    ql264, _ = ext264(Q_l, b_l * 0, False)
    wr264, br264 = ext264(W_r[:HID], b_r, True)
    qr264, _ = ext264(Q_r, b_r * 0, False)
    wql_ = np.concatenate([wl264, bl264], axis=0).astype(f16)
    wqr_ = np.concatenate([wr264, br264], axis=0).astype(f16)

    att8 = np.zeros((P, 2 * HEADS), np.float32)
    for b in range(2):
        for p in range(P):
            cprime = b * P + p
            j, h = cprime // HEADS, cprime % HEADS
            att8[p, b * HEADS + h] = 0.8 * att[h, j]

    def swizzle(xa, npad):
        G = npad // NB4
        xp = np.zeros((npad, IN_DIM), dtype=np.float32)
        xp[:len(xa)] = xa
        v = xp.reshape(G, NB, P, 2, P)
        v = np.transpose(v, (0, 4, 3, 1, 2))
        return np.ascontiguousarray(v.reshape(G, P, 2, NB * P)).astype(f16)

    xg_all = swizzle(x, n_nodes_pad)
    xg_own = [swizzle(x[k * nodes_per_core:(k + 1) * nodes_per_core],
                      npc_dense) for k in range(n_cores)]

    shared = {
        "xg_all": xg_all,
        "w_in_a": W_in[:P].astype(f16), "w_in_b": W_in[P:].astype(f16),
        "b_in_c": b_in[:, None].astype(f16),
        "wql": wql_, "ql": ql264.astype(f16),
        "wqr": wqr_, "qr": qr264.astype(f16),
        "att8": att8.astype(f16),
        "ident": np.eye(P, dtype=f16),
        "ones6464": np.ones((HID, HID), f16),
        "o1_128": np.ones((1, P), f16),
        "gbias4": np.broadcast_to(4.0 * gat_bias.astype(np.float32),
                                  (P, HID)).copy(),
        "wcls4": (0.25 * W_cls).astype(f16),
        "bcls4": np.tile(b_cls, NB)[None, :].astype(f16),
    }
    in_maps = []
    for k in range(n_cores):
        m = dict(shared)
        m["xg_own"] = xg_own[k]
        m["idx_lo"] = idx_lo[k]
        if idx_hi is not None:
            m["idx_hi"] = idx_hi[k]
        m["ohcomb"] = ohcomb[k]
        in_maps.append(m)
    return (in_maps, n_nodes_pad, npc_dense, npc_chunks, tuple(tl),
            tuple(th))


_CACHE = {}


def run(inputs, n_cores=8, trace=False):
    x = np.asarray(inputs["x"])
    n = x.shape[0]
    in_maps, n_nodes_pad, npc_dense, npc_chunks, tl, th = prepare_host(
        x, np.asarray(inputs["edge_index"]), np.asarray(inputs["W_in"]),
        np.asarray(inputs["b_in"]), np.asarray(inputs["prototypes"]),
        np.asarray(inputs["W_l"]), np.asarray(inputs["b_l"]),
        np.asarray(inputs["W_r"]), np.asarray(inputs["b_r"]),
        np.asarray(inputs["att"]), np.asarray(inputs["gat_bias"]),
        np.asarray(inputs["W_cls"]), np.asarray(inputs["b_cls"]), n_cores)
    key = (n_nodes_pad, npc_dense, npc_chunks, tl, th, n_cores)
    if key not in _CACHE:
        _CACHE[key] = build_program(*key)
    nc = _CACHE[key]
    res = run_bass_kernel_spmd(nc, in_maps, list(range(n_cores)), trace=trace)
    npc = n // n_cores
    c_chunks = npc_chunks // P
    outs = []
    for k in range(n_cores):
        o = np.asarray(res.results[k]["out"]).reshape(P, c_chunks, OUT_DIM)
        outs.append(np.transpose(o, (1, 0, 2)).reshape(npc_chunks,
                                                       OUT_DIM)[:npc])
    return np.concatenate(outs, axis=0), res


def kernel(**inputs):
    out, _ = run(inputs, n_cores=8)
    return out.astype(np.float32)


# revision 38
# speedup vs baseline: 2.9236x; 1.0175x over previous
"""GATv2-based CGNN forward pass on 8 Trainium2 NeuronCores.

Strategy (dst-node sharded, no collectives):
  - Each core owns N/8 destination nodes. Host buckets edges (incl. self
    loops) by dst core, then by 128-node dst chunk within the core.
    Per-chunk tile counts are ragged (max over cores per chunk index) so
    pad work tracks the actual edge distribution.
  - Dense phase (replicated for xl over all nodes; own nodes for xr):
    xl/xr rows are [feat256 head-interleaved (c' = j*4+h), beta4, ones4]
    fp16, where beta = 0.2*(feat @ att_blk) so that
      logit = 0.8*sum_c att_c*relu(z_c) + beta_l[s] + beta_r[d]
    (exact rewrite of att . leaky_relu via |z| = 2 relu(z) - z).
  - Edge phase per 128-dst chunk: batched indirect-DMA gather of xl[src]
    rows (4 SWDGE queues); z built TRANSPOSED in PSUM ([channel, edge])
    from a one-hot matmul of resident xr plus identity-matmul transpose
    of the gathered xl; one relu Activation moves it to SBUF; per-tile
    logits come from tall-skinny matmuls with the relu'd block as lhsT.
    Messages are one fp16 tensor_tensor multiply (broadcast alpha),
    scatter-added via fp8 one-hot matmuls; denominators ride along as
    ones*alpha columns.
  - Finish: per-chunk softmax normalize + head mean (0.25 folded into
    W_cls), relu; classifier runs as a final batched phase.
"""

import os
import sys

import numpy as np
import ml_dtypes

for _p in ("/opt/trn_rl_repo",):
    if _p not in sys.path and os.path.isdir(_p):
        sys.path.insert(0, _p)

import concourse.bass as bass
import concourse.tile as tile
from concourse import bacc, mybir
from concourse.bass_utils import run_bass_kernel_spmd

FP16 = mybir.dt.float16
FP32 = mybir.dt.float32
FP8 = mybir.dt.float8e4
INT16 = mybir.dt.int16
AF = mybir.ActivationFunctionType
ALU = mybir.AluOpType

P = 128
HID = 64
HEADS = 4
OUT_DIM = 16
IN_DIM = 256
FEAT = HEADS * HID          # 256
ROW = FEAT + 2 * HEADS      # 264 = feat + beta + ones
TROW = 384                  # padded table row (768B, 256B-aligned)
LO_ROWS = 32768             # int16 index range per gather table
NB = 4                      # node tiles per dense group

f16 = ml_dtypes.float16 if hasattr(ml_dtypes, "float16") else np.float16
f8 = ml_dtypes.float8_e4m3


def _cdiv(a, b):
    return (a + b - 1) // b


# ----------------------------------------------------------------------------
# Device program
# ----------------------------------------------------------------------------

def build_program(n_nodes_pad, npc_dense, npc_chunks, tl, th, n_cores):
    """tl/th: per-chunk lo/hi gather tile counts (tuples, shared by cores)."""
    GA = n_nodes_pad // (NB * P)
    GB = npc_dense // (NB * P)
    C_CHUNKS = npc_chunks // P
    tl = list(tl)
    th = list(th)
    tch = [a + b for a, b in zip(tl, th)]
    TMAX = max(tch)
    loS = np.concatenate([[0], np.cumsum(tl)]).astype(int)   # tile offsets
    hiS = np.concatenate([[0], np.cumsum(th)]).astype(int)
    ohS = np.concatenate([[0], np.cumsum([2 * t for t in tch])]).astype(int)
    hi_rows = max(n_nodes_pad - LO_ROWS, P)
    xr_slots = GB * NB

    nc = bacc.Bacc("TRN2", target_bir_lowering=False, debug=False,
                   num_devices=n_cores, num_swdge_queues=4)

    def din(name, shape, dtype=FP16):
        return nc.dram_tensor(name, shape, dtype, kind="ExternalInput").ap()

    xg_all = din("xg_all", [GA, P, 2, NB * P])
    xg_own = din("xg_own", [GB, P, 2, NB * P])
    w_in_a = din("w_in_a", [P, HID])
    w_in_b = din("w_in_b", [P, HID])
    b_in_c = din("b_in_c", [HID, 1])
    wql = din("wql", [HID + 1, ROW])       # [w264; b264]
    ql = din("ql", [HID, ROW])
    wqr = din("wqr", [HID + 1, ROW])
    qr = din("qr", [HID, ROW])
    att8 = din("att8", [P, 2 * HEADS])
    ident = din("ident", [P, P])
    ones6464 = din("ones6464", [HID, HID])
    o1_128 = din("o1_128", [1, P])
    gbias4 = din("gbias4", [P, HID], FP32)
    wcls4 = din("wcls4", [HID, OUT_DIM])
    bcls4 = din("bcls4", [1, NB * OUT_DIM])
    idx_lo = din("idx_lo", [P, int(loS[-1]) * 8], INT16)
    idx_hi = (din("idx_hi", [P, int(hiS[-1]) * 8], INT16)
              if hiS[-1] else None)
    ohcomb = din("ohcomb", [P, int(ohS[-1]) * P], FP8)

    out_ext = nc.dram_tensor("out", [P, C_CHUNKS * OUT_DIM], FP32,
                             kind="ExternalOutput").ap()

    xl_lo_tab = nc.dram_tensor("xl_lo_tab", [min(n_nodes_pad, LO_ROWS), TROW],
                               FP16).ap()
    xl_hi_tab = nc.dram_tensor("xl_hi_tab", [hi_rows, TROW], FP16).ap()

    qctr = [0]

    def next_q():
        qctr[0] = (qctr[0] + 1) % 4
        return qctr[0]

    with tile.TileContext(nc) as tc:
        with tc.tile_pool(name="consts", bufs=1) as cp:
            w_in_a_sb = cp.tile([P, HID], FP16)
            nc.sync.dma_start(w_in_a_sb[:], w_in_a[:])
            w_in_b_sb = cp.tile([P, HID], FP16)
            nc.sync.dma_start(w_in_b_sb[:], w_in_b[:])
            b_in_sb = cp.tile([HID, 1], FP16)
            nc.sync.dma_start(b_in_sb[:], b_in_c[:])
            wql_sb = cp.tile([HID + 1, ROW], FP16)
            nc.sync.dma_start(wql_sb[:], wql[:])
            ql_sb = cp.tile([HID, ROW], FP16)
            nc.sync.dma_start(ql_sb[:], ql[:])
            wqr_sb = cp.tile([HID + 1, ROW], FP16)
            nc.sync.dma_start(wqr_sb[:], wqr[:])
            qr_sb = cp.tile([HID, ROW], FP16)
            nc.sync.dma_start(qr_sb[:], qr[:])
            ones64_sb = cp.tile([HID, HID], FP16)
            nc.sync.dma_start(ones64_sb[:], ones6464[:])
            att_sb = cp.tile([P, 2 * HEADS], FP16)
            id_sb = cp.tile([P, P], FP16)
            o1_sb = cp.tile([1, P], FP16)
            gbias_sb = cp.tile([P, HID], FP32)
            wcls_sb = cp.tile([HID, OUT_DIM], FP16)
            bcls_sb = cp.tile([1, NB * OUT_DIM], FP16)
            idxlo_sb = cp.tile([P, int(loS[-1]) * 8], INT16)
            idxhi_sb = (cp.tile([P, int(hiS[-1]) * 8], INT16, name="idxhi_sb")
                        if idx_hi is not None else None)
            eps_sb = cp.tile([HID, 1], FP32)
            nc.gpsimd.memset(eps_sb[:], 1e-12)

            xr_res = cp.tile([P, xr_slots, ROW], FP16)
            orelu_res = cp.tile([P, C_CHUNKS, HID], FP16)
            fin_res = cp.tile([P, C_CHUNKS, OUT_DIM], FP32)

            # ---------------- dense phase ----------------
            W = NB * P

            def dense_stage1(g, xg, wq_sb, q_sb, to_table, sb, ps):
                xsb = sb.tile([P, 2, W], FP16, tag="xsb")
                nc.sync.dma_start(xsb[:], xg[g])
                ht_ps = ps.tile([HID, W], FP32, tag="ht_ps")
                nc.tensor.matmul(out=ht_ps[:], lhsT=w_in_a_sb[:],
                                 rhs=xsb[:, 0, :], start=True, stop=False)
                nc.tensor.matmul(out=ht_ps[:], lhsT=w_in_b_sb[:],
                                 rhs=xsb[:, 1, :], start=False, stop=True)
                hta = sb.tile([HID + 1, W], FP16, tag="hta", bufs=4)
                nc.scalar.activation(hta[0:HID, :], ht_ps[:], AF.Relu,
                                     bias=b_in_sb[:])
                nc.gpsimd.memset(hta[HID:HID + 1, :], 1.0)
                rsq = sb.tile([HID, W], FP16, tag="rsq")
                nc.vector.tensor_mul(rsq[:], hta[0:HID, :], hta[0:HID, :])
                nrm2_ps = ps.tile([HID, W], FP32, tag="nrm2_ps")
                nc.tensor.matmul(out=nrm2_ps[:], lhsT=ones64_sb[:],
                                 rhs=rsq[:], start=True, stop=True)
                return hta, nrm2_ps

            def dense_stage2(hta, nrm2_ps, sb, ps):
                nrmr = sb.tile([HID, W], FP32, tag="nrmr")
                nc.scalar.activation(nrmr[:], nrm2_ps[:], AF.Sqrt,
                                     bias=eps_sb[:])
                invr = sb.tile([HID, W], FP32, tag="invr")
                nc.vector.reciprocal(invr[:], nrmr[:])
                htn = sb.tile([HID, W], FP16, tag="htn", bufs=3)
                nc.gpsimd.tensor_mul(htn[:], hta[0:HID, :], invr[:])
                return htn

            def dense_stage3(g, wq_sb, q_sb, to_table, hta, htn, sb, ps):
                if to_table:
                    dst4 = sb.tile([P, NB, ROW], FP16, tag="dst4",
                                   name="dst4")
                for half in range(2):
                    xl2 = ps.tile([P, 2, 2 * ROW - 16], FP32, tag="xl2")
                    for ti in range(2):
                        t = half * 2 + ti
                        tsl = slice(t * P, (t + 1) * P)
                        nc.tensor.matmul(out=xl2[:, ti, 0:ROW],
                                         lhsT=hta[:, tsl],
                                         rhs=wq_sb[:], start=True, stop=False)
                        nc.tensor.matmul(out=xl2[:, ti, 0:ROW],
                                         lhsT=htn[:, tsl],
                                         rhs=q_sb[:], start=False, stop=True)
                    if to_table:
                        mv_out = dst4[:, half * 2:half * 2 + 2, :]
                    else:
                        mv_out = xr_res[:, g * NB + half * 2:
                                        g * NB + half * 2 + 2, :]
                    # alternate engines for the PSUM->SBUF move
                    if half == 0:
                        nc.scalar.copy(mv_out, xl2[:, :, 0:ROW])
                    else:
                        nc.vector.tensor_copy(mv_out, xl2[:, :, 0:ROW])
                if to_table:
                    r = g * NB * P
                    if r < LO_ROWS:
                        sink = xl_lo_tab[r:r + NB * P]
                    else:
                        sink = xl_hi_tab[r - LO_ROWS:r - LO_ROWS + NB * P]
                    nc.sync.dma_start(
                        sink.rearrange("(t p) c -> p t c", p=P)[:, :, 0:ROW],
                        dst4[:])

            with tc.tile_pool(name="dsb", bufs=3) as dsb, \
                    tc.tile_pool(name="dps", bufs=2, space="PSUM") as dps:
                specs = ([(g, xg_own, wqr_sb, qr_sb, False) for g in range(GB)]
                         + [(g, xg_all, wql_sb, ql_sb, True)
                            for g in range(GA)])
                NG = len(specs)
                s1out = {}
                s2out = {}
                for i in range(NG + 2):
                    if i < NG:
                        g, xg, wq_sb, q_sb, tt = specs[i]
                        s1out[i] = dense_stage1(g, xg, wq_sb, q_sb, tt,
                                                dsb, dps)
                    if 1 <= i and i - 1 < NG:
                        hta, nrm2_ps = s1out[i - 1]
                        s2out[i - 1] = dense_stage2(hta, nrm2_ps, dsb, dps)
                    if 2 <= i and i - 2 < NG:
                        g, xg, wq_sb, q_sb, tt = specs[i - 2]
                        hta, _ = s1out.pop(i - 2)
                        dense_stage3(g, wq_sb, q_sb, tt, hta,
                                     s2out.pop(i - 2), dsb, dps)

            # edge/classifier consts (emitted late so dense starts sooner)
            nc.sync.dma_start(att_sb[:], att8[:])
            nc.sync.dma_start(id_sb[:], ident[:])
            nc.sync.dma_start(o1_sb[:], o1_128[:])
            nc.sync.dma_start(gbias_sb[:], gbias4[:])
            nc.sync.dma_start(wcls_sb[:], wcls4[:])
            nc.sync.dma_start(bcls_sb[:], bcls4[:])
            nc.sync.dma_start(idxlo_sb[:], idx_lo[:])
            if idx_hi is not None:
                nc.sync.dma_start(idxhi_sb[:], idx_hi[:])

            # ---------------- edge phase ----------------
            GB_T = 8            # tiles per dma_gather call (<=1024 indices)
            ZG = 4              # tiles per z-group (PSUM bank pair)

            with tc.tile_pool(name="esb", bufs=3) as esb, \
                    tc.tile_pool(name="msb", bufs=4) as msb, \
                    tc.tile_pool(name="zps", bufs=2, space="PSUM") as zps, \
                    tc.tile_pool(name="lps", bufs=1, space="PSUM") as lps, \
                    tc.tile_pool(name="aps", bufs=2, space="PSUM") as aps:

                logits_of = {}
                xlg_of = {}
                oh_of = {}
                expv_of = {}

                def front(c):
                    T = tch[c]
                    oh_sb = esb.tile([P, 2 * TMAX * P], FP8, tag="oh", bufs=4)
                    nc.sync.dma_start(oh_sb[:, 0:2 * T * P],
                                      ohcomb[:, ohS[c] * P:ohS[c + 1] * P])
                    xlg = esb.tile([P, TMAX, TROW], FP16, tag="xlg", bufs=4)
                    segs = [(tl[c], 0, int(loS[c]), xl_lo_tab, idxlo_sb)]
                    if th[c]:
                        segs.append((th[c], tl[c], int(hiS[c]), xl_hi_tab,
                                     idxhi_sb))
                    for t_seg, off, base, tab, idx_sb_ in segs:
                        for b in range(0, t_seg, GB_T):
                            nt = min(GB_T, t_seg - b)
                            nc.gpsimd.dma_gather(
                                out_ap=xlg[:, off + b:off + b + nt, :],
                                in_ap=tab[:],
                                idxs_ap=idx_sb_[:, (base + b) * 8:
                                                (base + b + nt) * 8],
                                num_idxs=nt * P, num_idxs_reg=nt * P,
                                elem_size=TROW, queue_num=next_q())
                    logits_ps = lps.tile([P, TMAX * HEADS], FP32,
                                         tag="logits")
                    expv = esb.tile([P, TMAX * HEADS], FP16, tag="expv", bufs=4)
                    exp_done = 0
                    groups = list(range(0, T, ZG))
                    for g0 in groups:
                        gl = min(ZG, T - g0)
                        zt = zps.tile([P, 2, ZG * P], FP32, tag="zt")
                        for gi in range(gl):
                            t = g0 + gi
                            esl = slice(gi * P, (gi + 1) * P)
                            ohsl = slice(t * P, (t + 1) * P)
                            for b in range(2):
                                bsl = slice(b * P, (b + 1) * P)
                                nc.tensor.matmul(
                                    out=zt[:, b, esl],
                                    lhsT=xr_res[:, c, bsl],
                                    rhs=oh_sb[:, ohsl],
                                    start=True, stop=False)
                                nc.tensor.matmul(
                                    out=zt[:, b, esl],
                                    lhsT=xlg[:, t, bsl],
                                    rhs=id_sb[:],
                                    start=False, stop=True)
                        wt = msb.tile([P, 2, ZG * P], FP16, tag="wt",
                                      bufs=3)
                        if g0 == ZG:
                            nc.vector.tensor_scalar_max(
                                wt[:, :, 0:gl * P], zt[:, :, 0:gl * P], 0.0)
                        else:
                            nc.scalar.activation(wt[:, :, 0:gl * P],
                                                 zt[:, :, 0:gl * P], AF.Relu)
                        for gi in range(gl):
                            t = g0 + gi
                            esl = slice(gi * P, (gi + 1) * P)
                            lsl = slice(t * HEADS, (t + 1) * HEADS)
                            ohsl = slice(t * P, (t + 1) * P)
                            nc.tensor.matmul(
                                out=logits_ps[:, lsl], lhsT=id_sb[:],
                                rhs=xlg[:, t, FEAT:FEAT + HEADS],
                                start=True, stop=False)
                            nc.tensor.matmul(
                                out=logits_ps[:, lsl],
                                lhsT=oh_sb[:, ohsl],
                                rhs=xr_res[:, c, FEAT:FEAT + HEADS],
                                start=False, stop=False)
                            nc.tensor.matmul(
                                out=logits_ps[:, lsl], lhsT=wt[:, 0, esl],
                                rhs=att_sb[:, 0:HEADS],
                                start=False, stop=False)
                            nc.tensor.matmul(
                                out=logits_ps[:, lsl], lhsT=wt[:, 1, esl],
                                rhs=att_sb[:, HEADS:2 * HEADS],
                                start=False, stop=True)
                        gidx = g0 // ZG
                        done = min(g0 + gl, T)
                        if gidx % 2 == 1 or g0 == groups[-1]:
                            nc.scalar.activation(
                                expv[:, exp_done * HEADS:done * HEADS],
                                logits_ps[:, exp_done * HEADS:done * HEADS],
                                AF.Exp)
                            exp_done = done
                    xlg_of[c] = xlg
                    oh_of[c] = oh_sb
                    expv_of[c] = expv

                def back_b(c):
                    T = tch[c]
                    xlg = xlg_of.pop(c)
                    oh_sb = oh_of.pop(c)
                    expv = expv_of.pop(c)
                    agg = aps.tile([P, ROW], FP32, tag="agg")
                    JH = ROW // HEADS
                    for t0 in range(0, T, 2):
                        tn = min(2, T - t0)
                        msg = msb.tile([P, 2, ROW], FP16, tag="msg")
                        a4 = expv[:, t0 * HEADS:(t0 + tn) * HEADS] \
                            .rearrange("p (t o h) -> p t o h", t=tn, o=1) \
                            .to_broadcast([P, tn, JH, HEADS])
                        nc.vector.tensor_tensor(
                            msg[:, 0:tn, :].rearrange(
                                "p t (j h) -> p t j h", h=HEADS),
                            xlg[:, t0:t0 + tn, 0:ROW].rearrange(
                                "p t (j h) -> p t j h", h=HEADS),
                            a4, ALU.mult)
                        for ti in range(tn):
                            t = t0 + ti
                            nc.tensor.matmul(
                                out=agg[:],
                                lhsT=oh_sb[:, (T + t) * P:(T + t + 1) * P],
                                rhs=msg[:, ti, :], start=(t == 0),
                                stop=(t == T - 1))
                    den = msb.tile([P, HEADS], FP32, tag="den")
                    nc.vector.tensor_scalar_add(den[:], agg[:, ROW - HEADS:],
                                                1e-16)
                    dinv = msb.tile([P, HEADS], FP32, tag="dinv")
                    nc.vector.reciprocal(dinv[:], den[:])
                    scl = msb.tile([P, FEAT], FP32, tag="scl")
                    di4 = dinv[:].rearrange("p (o h) -> p o h", o=1) \
                        .to_broadcast([P, HID, HEADS])
                    nc.vector.tensor_tensor(
                        scl[:].rearrange("p (j h) -> p j h", h=HEADS),
                        agg[:, 0:FEAT].rearrange("p (j h) -> p j h", h=HEADS),
                        di4, ALU.mult)
                    ored = msb.tile([P, HID], FP32, tag="ored")
                    nc.vector.tensor_reduce(
                        out=ored[:],
                        in_=scl[:].rearrange("p (j h) -> p j h", h=HEADS),
                        axis=mybir.AxisListType.X, op=ALU.add)
                    obias = msb.tile([P, HID], FP32, tag="obias")
                    nc.vector.tensor_add(obias[:], ored[:], gbias_sb[:])
                    nc.vector.tensor_scalar_max(orelu_res[:, c, :],
                                                obias[:], 0.0)

                def classify(c0, cl, fsb, fps):
                    # one PSUM bank: ot at fp16 cols 0:256, fin as fp32
                    # view of fp16 cols 512:576
                    cls_ps = fps.tile([P, 1024], FP16, tag="cls_ps")
                    for ci in range(cl):
                        nc.tensor.transpose(
                            out=cls_ps[0:HID, ci * P:(ci + 1) * P],
                            in_=orelu_res[:, c0 + ci, :], identity=id_sb[:])
                    ot16 = fsb.tile([HID, 2 * P], FP16, tag="ot16")
                    nc.scalar.copy(ot16[:, 0:cl * P],
                                   cls_ps[0:HID, 0:cl * P])
                    fin_ps = cls_ps[:, 512:576].bitcast(FP32)
                    for ci in range(cl):
                        nc.tensor.matmul(
                            out=fin_ps[:, ci * OUT_DIM:(ci + 1) * OUT_DIM],
                            lhsT=ot16[:, ci * P:(ci + 1) * P],
                            rhs=wcls_sb[:], start=True, stop=False)
                        nc.tensor.matmul(
                            out=fin_ps[:, ci * OUT_DIM:(ci + 1) * OUT_DIM],
                            lhsT=o1_sb[:],
                            rhs=bcls_sb[:, ci * OUT_DIM:(ci + 1) * OUT_DIM],
                            start=False, stop=True)
                    nc.vector.tensor_copy(
                        fin_res[:, c0:c0 + cl, :].rearrange(
                            "p c o -> p (c o)"),
                        fin_ps[:, 0:cl * OUT_DIM])

                with tc.tile_pool(name="fsb", bufs=2) as fsb, \
                        tc.tile_pool(name="fps", bufs=1,
                                     space="PSUM") as fps2:
                    front(0)
                    if C_CHUNKS > 1:
                        front(1)
                    for c in range(C_CHUNKS):
                        if c + 2 < C_CHUNKS:
                            front(c + 2)
                        back_b(c)
                        if c % 2 == 1:
                            classify(c - 1, 2, fsb, fps2)
                        if c % 8 == 7:
                            nc.sync.dma_start(
                                out_ext[:, (c - 7) * OUT_DIM:
                                        (c + 1) * OUT_DIM].rearrange(
                                    "p (c o) -> p c o", o=OUT_DIM),
                                fin_res[:, c - 7:c + 1, :])
                    if C_CHUNKS % 2:
                        classify(C_CHUNKS - 1, 1, fsb, fps2)
                    rem0 = (C_CHUNKS // 8) * 8
                    if rem0 < C_CHUNKS:
                        nc.sync.dma_start(
                            out_ext[:, rem0 * OUT_DIM:].rearrange(
                                "p (c o) -> p c o", o=OUT_DIM),
                            fin_res[:, rem0:, :])

    nc.compile()
    return nc


# ----------------------------------------------------------------------------
# Host-side data preparation
# ----------------------------------------------------------------------------

def prepare_host(x, edge_index, W_in, b_in, prototypes, W_l, b_l, W_r, b_r,
                 att, gat_bias, W_cls, b_cls, n_cores):
    x = np.asarray(x, np.float32)
    W_in = np.asarray(W_in, np.float32)
    b_in = np.asarray(b_in, np.float32)
    prototypes = np.asarray(prototypes, np.float32)
    W_l = np.asarray(W_l, np.float32)
    b_l = np.asarray(b_l, np.float32)
    W_r = np.asarray(W_r, np.float32)
    b_r = np.asarray(b_r, np.float32)
    att = np.asarray(att, np.float32)
    gat_bias = np.asarray(gat_bias, np.float32)
    W_cls = np.asarray(W_cls, np.float32)
    b_cls = np.asarray(b_cls, np.float32)

    n = x.shape[0]
    nodes_per_core = n // n_cores
    NB4 = NB * P

    n_nodes_pad = _cdiv(n, NB4) * NB4
    npc_dense = _cdiv(nodes_per_core, NB4) * NB4
    npc_chunks = _cdiv(nodes_per_core, P) * P
    c_chunks = npc_chunks // P

    # --- edge bucketing ---
    src = np.asarray(edge_index[0], dtype=np.int64)
    dst = np.asarray(edge_index[1], dtype=np.int64)
    loop = np.arange(n, dtype=np.int64)
    src = np.concatenate([src, loop])
    dst = np.concatenate([dst, loop])

    core = dst // nodes_per_core
    dstl = dst - core * nodes_per_core
    chunk = dstl // P
    seg = (src >= LO_ROWS).astype(np.int64)

    counts = np.zeros((n_cores, c_chunks, 2), dtype=np.int64)
    np.add.at(counts, (core, chunk, seg), 1)
    # ragged per-chunk tile counts: max over cores
    tl = [int(_cdiv(int(counts[:, c, 0].max()), P)) for c in range(c_chunks)]
    th = [int(_cdiv(int(counts[:, c, 1].max()), P)) for c in range(c_chunks)]
    tl = [max(t, 1) for t in tl]
    tch = [a + b for a, b in zip(tl, th)]
    loS = np.concatenate([[0], np.cumsum(tl)]).astype(np.int64)
    hiS = np.concatenate([[0], np.cumsum(th)]).astype(np.int64)
    ohS = np.concatenate([[0], np.cumsum([2 * t for t in tch])]) \
        .astype(np.int64)

    order = np.lexsort((seg, chunk, core))
    src_o, core_o, chunk_o, dstl_o, seg_o = (src[order], core[order],
                                             chunk[order], dstl[order],
                                             seg[order])
    bounds = np.zeros(n_cores * c_chunks * 2 + 1, dtype=np.int64)
    np.cumsum(counts.reshape(-1), out=bounds[1:])
    flat_bucket = (core_o * c_chunks + chunk_o) * 2 + seg_o
    pos = np.arange(len(src_o)) - bounds[flat_bucket]

    lo_slots = int(loS[-1]) * P
    hi_slots = int(hiS[-1]) * P
    idx_lo_slot = np.zeros((n_cores, lo_slots), dtype=np.int32)
    idx_hi_slot = np.zeros((n_cores, max(hi_slots, 1)), dtype=np.int32)
    # nloc in per-chunk tile space for one-hot build
    nloc_lo = np.full((n_cores, lo_slots), -1, dtype=np.int32)
    nloc_hi = np.full((n_cores, max(hi_slots, 1)), -1, dtype=np.int32)

    lo_base = loS[chunk_o] * P + pos
    hi_base = hiS[chunk_o] * P + pos
    is_lo = seg_o == 0
    idx_lo_slot[core_o[is_lo], lo_base[is_lo]] = src_o[is_lo].astype(np.int32)
    nloc_lo[core_o[is_lo], lo_base[is_lo]] = \
        (dstl_o[is_lo] - chunk_o[is_lo] * P).astype(np.int32)
    is_hi = ~is_lo
    idx_hi_slot[core_o[is_hi], hi_base[is_hi]] = \
        (src_o[is_hi] - LO_ROWS).astype(np.int32)
    nloc_hi[core_o[is_hi], hi_base[is_hi]] = \
        (dstl_o[is_hi] - chunk_o[is_hi] * P).astype(np.int32)

    def wrap16(vals):
        # [k, S*128] -> [k, 128, S*8] int16
        S = vals.shape[1] // P
        v = vals.reshape(n_cores, S * 8, 16)
        v = np.transpose(v, (0, 2, 1))
        v = np.tile(v, (1, 8, 1))
        return np.ascontiguousarray(v).astype(np.int16)

    idx_lo = wrap16(idx_lo_slot)
    idx_hi = wrap16(idx_hi_slot) if hi_slots else None

    # --- one-hot (fp8), ragged layout ---
    iota = np.arange(P, dtype=np.int32)
    ohcomb = np.zeros((n_cores, P, int(ohS[-1]) * P), dtype=f8)
    for c in range(c_chunks):
        T = tch[c]
        nl = np.concatenate(
            [nloc_lo[:, loS[c] * P:loS[c + 1] * P],
             nloc_hi[:, hiS[c] * P:hiS[c + 1] * P]], axis=1) \
            .reshape(n_cores, T, P)
        oh = (nl[..., None] == iota)                 # [k, t, e, n]
        base = int(ohS[c]) * P
        ohcomb[:, :, base:base + T * P] = np.transpose(oh, (0, 3, 1, 2)) \
            .reshape(n_cores, P, T * P)
        ohcomb[:, :, base + T * P:base + 2 * T * P] = \
            np.transpose(oh, (0, 2, 1, 3)).reshape(n_cores, P, T * P)

    # --- weights ---
    att_blk = np.zeros((FEAT, HEADS), dtype=np.float32)
    for h in range(HEADS):
        att_blk[h * HID:(h + 1) * HID, h] = att[h]
    p_norm = prototypes / (np.linalg.norm(prototypes, axis=1, keepdims=True)
                           + 1e-12)
    Q_l = p_norm.T @ W_l[HID:HID + 2]
    Q_r = p_norm.T @ W_r[HID:HID + 2]

    perm = np.zeros(FEAT, np.int64)
    for h in range(HEADS):
        for j in range(HID):
            perm[j * HEADS + h] = h * HID + j

    def ext264(w, b, with_ones):
        w264 = np.concatenate(
            [w[:, perm], 0.2 * (w @ att_blk),
             np.zeros((w.shape[0], HEADS), np.float32)], axis=1)
        ones = np.ones(HEADS, np.float32) if with_ones else \
            np.zeros(HEADS, np.float32)
        b264 = np.concatenate([b[perm], 0.2 * (b @ att_blk), ones])[None, :]
        return w264, b264

    wl264, bl264 = ext264(W_l[:HID], b_l, True)
    ql264, _ = ext264(Q_l, b_l * 0, False)
    wr264, br264 = ext264(W_r[:HID], b_r, True)
    qr264, _ = ext264(Q_r, b_r * 0, False)
    wql_ = np.concatenate([wl264, bl264], axis=0).astype(f16)
    wqr_ = np.concatenate([wr264, br264], axis=0).astype(f16)

    att8 = np.zeros((P, 2 * HEADS), np.float32)
    for b in range(2):
        for p in range(P):
            cprime = b * P + p
            j, h = cprime // HEADS, cprime % HEADS
            att8[p, b * HEADS + h] = 0.8 * att[h, j]

    def swizzle(xa, npad):
        G = npad // NB4
        xp = np.zeros((npad, IN_DIM), dtype=np.float32)
        xp[:len(xa)] = xa
        v = xp.reshape(G, NB, P, 2, P)
        v = np.transpose(v, (0, 4, 3, 1, 2))
        return np.ascontiguousarray(v.reshape(G, P, 2, NB * P)).astype(f16)

    xg_all = swizzle(x, n_nodes_pad)
    xg_own = [swizzle(x[k * nodes_per_core:(k + 1) * nodes_per_core],
                      npc_dense) for k in range(n_cores)]

    shared = {
        "xg_all": xg_all,
        "w_in_a": W_in[:P].astype(f16), "w_in_b": W_in[P:].astype(f16),
        "b_in_c": b_in[:, None].astype(f16),
        "wql": wql_, "ql": ql264.astype(f16),
        "wqr": wqr_, "qr": qr264.astype(f16),
        "att8": att8.astype(f16),
        "ident": np.eye(P, dtype=f16),
        "ones6464": np.ones((HID, HID), f16),
        "o1_128": np.ones((1, P), f16),
        "gbias4": np.broadcast_to(4.0 * gat_bias.astype(np.float32),
                                  (P, HID)).copy(),
        "wcls4": (0.25 * W_cls).astype(f16),
        "bcls4": np.tile(b_cls, NB)[None, :].astype(f16),
    }
    in_maps = []
    for k in range(n_cores):
        m = dict(shared)
        m["xg_own"] = xg_own[k]
        m["idx_lo"] = idx_lo[k]
        if idx_hi is not None:
            m["idx_hi"] = idx_hi[k]
        m["ohcomb"] = ohcomb[k]
        in_maps.append(m)
    return (in_maps, n_nodes_pad, npc_dense, npc_chunks, tuple(tl),
            tuple(th))


_CACHE = {}


def run(inputs, n_cores=8, trace=False):
    x = np.asarray(inputs["x"])
    n = x.shape[0]
    in_maps, n_nodes_pad, npc_dense, npc_chunks, tl, th = prepare_host(
        x, np.asarray(inputs["edge_index"]), np.asarray(inputs["W_in"]),
        np.asarray(inputs["b_in"]), np.asarray(inputs["prototypes"]),
        np.asarray(inputs["W_l"]), np.asarray(inputs["b_l"]),
        np.asarray(inputs["W_r"]), np.asarray(inputs["b_r"]),
        np.asarray(inputs["att"]), np.asarray(inputs["gat_bias"]),
        np.asarray(inputs["W_cls"]), np.asarray(inputs["b_cls"]), n_cores)
    key = (n_nodes_pad, npc_dense, npc_chunks, tl, th, n_cores)
    if key not in _CACHE:
        _CACHE[key] = build_program(*key)
    nc = _CACHE[key]
    res = run_bass_kernel_spmd(nc, in_maps, list(range(n_cores)), trace=trace)
    npc = n // n_cores
    c_chunks = npc_chunks // P
    outs = []
    for k in range(n_cores):
        o = np.asarray(res.results[k]["out"]).reshape(P, c_chunks, OUT_DIM)
        outs.append(np.transpose(o, (1, 0, 2)).reshape(npc_chunks,
                                                       OUT_DIM)[:npc])
    return np.concatenate(outs, axis=0), res


def kernel(**inputs):
    out, _ = run(inputs, n_cores=8)
    return out.astype(np.float32)


# revision 46
# speedup vs baseline: 2.9992x; 1.0259x over previous
"""GATv2-based CGNN forward pass on 8 Trainium2 NeuronCores.

Strategy (dst-node sharded, no collectives):
  - Each core owns N/8 destination nodes. Host buckets edges (incl. self
    loops) by dst core, then by 128-node dst chunk within the core.
    Per-chunk tile counts are ragged (max over cores per chunk index) so
    pad work tracks the actual edge distribution.
  - Dense phase (replicated for xl over all nodes; own nodes for xr):
    xl/xr rows are [feat256 head-interleaved (c' = j*4+h), beta4, ones4]
    fp16, where beta = 0.2*(feat @ att_blk) so that
      logit = 0.8*sum_c att_c*relu(z_c) + beta_l[s] + beta_r[d]
    (exact rewrite of att . leaky_relu via |z| = 2 relu(z) - z).
  - Edge phase per 128-dst chunk: batched indirect-DMA gather of xl[src]
    rows (4 SWDGE queues); z built TRANSPOSED in PSUM ([channel, edge])
    from a one-hot matmul of resident xr plus identity-matmul transpose
    of the gathered xl; one relu Activation moves it to SBUF; per-tile
    logits come from tall-skinny matmuls with the relu'd block as lhsT.
    Messages are one fp16 tensor_tensor multiply (broadcast alpha),
    scatter-added via fp8 one-hot matmuls; denominators ride along as
    ones*alpha columns.
  - Finish: per-chunk softmax normalize + head mean (0.25 folded into
    W_cls), relu; classifier runs as a final batched phase.
"""

import os
import sys

import numpy as np
import ml_dtypes

for _p in ("/opt/trn_rl_repo",):
    if _p not in sys.path and os.path.isdir(_p):
        sys.path.insert(0, _p)

import concourse.bass as bass
import concourse.tile as tile
from concourse import bacc, mybir
from concourse.bass_utils import run_bass_kernel_spmd

FP16 = mybir.dt.float16
FP32 = mybir.dt.float32
FP8 = mybir.dt.float8e4
INT16 = mybir.dt.int16
AF = mybir.ActivationFunctionType
ALU = mybir.AluOpType

P = 128
HID = 64
HEADS = 4
OUT_DIM = 16
IN_DIM = 256
FEAT = HEADS * HID          # 256
ROW = FEAT + 2 * HEADS      # 264 = feat + beta + ones
TROW = 384                  # padded table row (768B, 256B-aligned)
LO_ROWS = 32768             # int16 index range per gather table
NB = 4                      # node tiles per dense group

f16 = ml_dtypes.float16 if hasattr(ml_dtypes, "float16") else np.float16
f8 = ml_dtypes.float8_e4m3


def _cdiv(a, b):
    return (a + b - 1) // b


# ----------------------------------------------------------------------------
# Device program
# ----------------------------------------------------------------------------

def build_program(n_nodes_pad, npc_dense, npc_chunks, tl, th, n_cores):
    """tl/th: per-chunk lo/hi gather tile counts (tuples, shared by cores)."""
    GA = n_nodes_pad // (NB * P)
    C_CHUNKS = npc_chunks // P
    OWN_STRIDE = (n_nodes_pad // P) // C_CHUNKS   # own tiles every 8th
    tl = list(tl)
    th = list(th)
    tch = [a + b for a, b in zip(tl, th)]
    TMAX = max(tch)
    loS = np.concatenate([[0], np.cumsum(tl)]).astype(int)   # tile offsets
    hiS = np.concatenate([[0], np.cumsum(th)]).astype(int)
    ohS = np.concatenate([[0], np.cumsum([2 * t for t in tch])]).astype(int)
    hi_rows = max(n_nodes_pad - LO_ROWS, P)
    xr_slots = C_CHUNKS

    nc = bacc.Bacc("TRN2", target_bir_lowering=False, debug=False,
                   num_devices=n_cores, num_swdge_queues=4)

    def din(name, shape, dtype=FP16):
        return nc.dram_tensor(name, shape, dtype, kind="ExternalInput").ap()

    xg_all = din("xg_all", [GA, P, 2, NB * P])
    w_in_a = din("w_in_a", [P, HID])
    w_in_b = din("w_in_b", [P, HID])
    b_in_c = din("b_in_c", [HID, 1])
    wql = din("wql", [HID + 1, ROW])       # [w264; b264]
    ql = din("ql", [HID, ROW])
    wqr = din("wqr", [HID + 1, ROW])
    qr = din("qr", [HID, ROW])
    att8 = din("att8", [P, 2 * HEADS])
    ident = din("ident", [P, P])
    ones6464 = din("ones6464", [HID, HID])
    o1_128 = din("o1_128", [1, P])
    gbias4 = din("gbias4", [P, HID], FP32)
    wcls4 = din("wcls4", [HID, OUT_DIM])
    bcls4 = din("bcls4", [1, NB * OUT_DIM])
    idx_lo = din("idx_lo", [P, int(loS[-1]) * 8], INT16)
    idx_hi = (din("idx_hi", [P, int(hiS[-1]) * 8], INT16)
              if hiS[-1] else None)
    ohcomb = din("ohcomb", [P, int(ohS[-1]) * P], FP8)

    out_ext = nc.dram_tensor("out", [P, C_CHUNKS * OUT_DIM], FP32,
                             kind="ExternalOutput").ap()

    xl_lo_tab = nc.dram_tensor("xl_lo_tab", [min(n_nodes_pad, LO_ROWS), TROW],
                               FP16).ap()
    xl_hi_tab = nc.dram_tensor("xl_hi_tab", [hi_rows, TROW], FP16).ap()

    qctr = [0]

    def next_q():
        qctr[0] = (qctr[0] + 1) % 4
        return qctr[0]

    with tile.TileContext(nc) as tc:
        with tc.tile_pool(name="consts", bufs=1) as cp:
            w_in_a_sb = cp.tile([P, HID], FP16)
            nc.sync.dma_start(w_in_a_sb[:], w_in_a[:])
            w_in_b_sb = cp.tile([P, HID], FP16)
            nc.sync.dma_start(w_in_b_sb[:], w_in_b[:])
            b_in_sb = cp.tile([HID, 1], FP16)
            nc.sync.dma_start(b_in_sb[:], b_in_c[:])
            wql_sb = cp.tile([HID + 1, ROW], FP16)
            nc.sync.dma_start(wql_sb[:], wql[:])
            ql_sb = cp.tile([HID, ROW], FP16)
            nc.sync.dma_start(ql_sb[:], ql[:])
            wqr_sb = cp.tile([HID + 1, ROW], FP16)
            nc.sync.dma_start(wqr_sb[:], wqr[:])
            qr_sb = cp.tile([HID, ROW], FP16)
            nc.sync.dma_start(qr_sb[:], qr[:])
            ones64_sb = cp.tile([HID, HID], FP16)
            nc.sync.dma_start(ones64_sb[:], ones6464[:])
            att_sb = cp.tile([P, 2 * HEADS], FP16)
            id_sb = cp.tile([P, P], FP16)
            o1_sb = cp.tile([1, P], FP16)
            gbias_sb = cp.tile([P, HID], FP32)
            wcls_sb = cp.tile([HID, OUT_DIM], FP16)
            bcls_sb = cp.tile([1, NB * OUT_DIM], FP16)
            idxlo_sb = cp.tile([P, int(loS[-1]) * 8], INT16)
            idxhi_sb = (cp.tile([P, int(hiS[-1]) * 8], INT16, name="idxhi_sb")
                        if idx_hi is not None else None)
            eps_sb = cp.tile([HID, 1], FP32)
            nc.gpsimd.memset(eps_sb[:], 1e-12)

            xr_res = cp.tile([P, xr_slots, ROW], FP16)
            orelu_res = cp.tile([P, C_CHUNKS, HID], FP16)
            fin_res = cp.tile([P, C_CHUNKS, OUT_DIM], FP32)

            # ---------------- dense phase ----------------
            W = NB * P

            def dense_stage1(g, xg, wq_sb, q_sb, to_table, sb, ps):
                xsb = sb.tile([P, 2, W], FP16, tag="xsb")
                nc.sync.dma_start(xsb[:], xg[g])
                ht_ps = ps.tile([HID, W], FP32, tag="ht_ps")
                nc.tensor.matmul(out=ht_ps[:], lhsT=w_in_a_sb[:],
                                 rhs=xsb[:, 0, :], start=True, stop=False)
                nc.tensor.matmul(out=ht_ps[:], lhsT=w_in_b_sb[:],
                                 rhs=xsb[:, 1, :], start=False, stop=True)
                hta = sb.tile([HID + 1, W], FP16, tag="hta", bufs=4)
                nc.scalar.activation(hta[0:HID, :], ht_ps[:], AF.Relu,
                                     bias=b_in_sb[:])
                nc.gpsimd.memset(hta[HID:HID + 1, :], 1.0)
                rsq = sb.tile([HID, W], FP16, tag="rsq")
                nc.vector.tensor_mul(rsq[:], hta[0:HID, :], hta[0:HID, :])
                nrm2_ps = ps.tile([HID, W], FP32, tag="nrm2_ps")
                nc.tensor.matmul(out=nrm2_ps[:], lhsT=ones64_sb[:],
                                 rhs=rsq[:], start=True, stop=True)
                return hta, nrm2_ps

            def dense_stage2(hta, nrm2_ps, sb, ps):
                nrmr = sb.tile([HID, W], FP32, tag="nrmr")
                nc.scalar.activation(nrmr[:], nrm2_ps[:], AF.Sqrt,
                                     bias=eps_sb[:])
                invr = sb.tile([HID, W], FP32, tag="invr")
                nc.vector.reciprocal(invr[:], nrmr[:])
                htn = sb.tile([HID, W], FP16, tag="htn", bufs=3)
                nc.gpsimd.tensor_mul(htn[:], hta[0:HID, :], invr[:])
                return htn

            def dense_stage3(g, wq_sb, q_sb, to_table, hta, htn, sb, ps):
                dst4 = sb.tile([P, NB, ROW], FP16, tag="dst4",
                               name="dst4")
                for half in range(2):
                    xl2 = ps.tile([P, 2, 2 * ROW - 16], FP32, tag="xl2")
                    for ti in range(2):
                        t = half * 2 + ti
                        tsl = slice(t * P, (t + 1) * P)
                        nc.tensor.matmul(out=xl2[:, ti, 0:ROW],
                                         lhsT=hta[:, tsl],
                                         rhs=wq_sb[:], start=True, stop=False)
                        nc.tensor.matmul(out=xl2[:, ti, 0:ROW],
                                         lhsT=htn[:, tsl],
                                         rhs=q_sb[:], start=False, stop=True)
                    mv_out = dst4[:, half * 2:half * 2 + 2, :]
                    # alternate engines for the PSUM->SBUF move
                    if half == 0:
                        nc.scalar.copy(mv_out, xl2[:, :, 0:ROW])
                    else:
                        nc.vector.tensor_copy(mv_out, xl2[:, :, 0:ROW])
                    # at most one own tile per half (own = every 8th tile)
                    for ti in range(2):
                        gt = g * NB + half * 2 + ti
                        if gt % OWN_STRIDE == 0 and gt // OWN_STRIDE < \
                                C_CHUNKS:
                            t = half * 2 + ti
                            tsl = slice(t * P, (t + 1) * P)
                            xr2 = ps.tile([P, 2, 2 * ROW - 16], FP32,
                                          tag="xl2", name="xr2")
                            nc.tensor.matmul(out=xr2[:, 0, 0:ROW],
                                             lhsT=hta[:, tsl],
                                             rhs=wqr_sb[:], start=True,
                                             stop=False)
                            nc.tensor.matmul(out=xr2[:, 0, 0:ROW],
                                             lhsT=htn[:, tsl],
                                             rhs=qr_sb[:], start=False,
                                             stop=True)
                            if half == 0:
                                nc.vector.tensor_copy(
                                    xr_res[:, gt // OWN_STRIDE, :],
                                    xr2[:, 0, 0:ROW])
                            else:
                                nc.scalar.copy(
                                    xr_res[:, gt // OWN_STRIDE, :],
                                    xr2[:, 0, 0:ROW])
                r = g * NB * P
                if r < LO_ROWS:
                    sink = xl_lo_tab[r:r + NB * P]
                else:
                    sink = xl_hi_tab[r - LO_ROWS:r - LO_ROWS + NB * P]
                nc.sync.dma_start(
                    sink.rearrange("(t p) c -> p t c", p=P)[:, :, 0:ROW],
                    dst4[:])

            with tc.tile_pool(name="dsb", bufs=3) as dsb, \
                    tc.tile_pool(name="dps", bufs=2, space="PSUM") as dps:
                specs = [(g, xg_all, wql_sb, ql_sb, True)
                         for g in range(GA)]
                NG = len(specs)
                s1out = {}
                s2out = {}
                for i in range(NG + 2):
                    if i < NG:
                        g, xg, wq_sb, q_sb, tt = specs[i]
                        s1out[i] = dense_stage1(g, xg, wq_sb, q_sb, tt,
                                                dsb, dps)
                    if 1 <= i and i - 1 < NG:
                        hta, nrm2_ps = s1out[i - 1]
                        s2out[i - 1] = dense_stage2(hta, nrm2_ps, dsb, dps)
                    if 2 <= i and i - 2 < NG:
                        g, xg, wq_sb, q_sb, tt = specs[i - 2]
                        hta, _ = s1out.pop(i - 2)
                        dense_stage3(g, wq_sb, q_sb, tt, hta,
                                     s2out.pop(i - 2), dsb, dps)

            # edge/classifier consts (emitted late so dense starts sooner)
            nc.sync.dma_start(att_sb[:], att8[:])
            nc.sync.dma_start(id_sb[:], ident[:])
            nc.sync.dma_start(o1_sb[:], o1_128[:])
            nc.sync.dma_start(gbias_sb[:], gbias4[:])
            nc.sync.dma_start(wcls_sb[:], wcls4[:])
            nc.sync.dma_start(bcls_sb[:], bcls4[:])
            nc.sync.dma_start(idxlo_sb[:], idx_lo[:])
            if idx_hi is not None:
                nc.sync.dma_start(idxhi_sb[:], idx_hi[:])

            # ---------------- edge phase ----------------
            GB_T = 8            # tiles per dma_gather call (<=1024 indices)
            ZG = 4              # tiles per z-group (PSUM bank pair)

            with tc.tile_pool(name="esb", bufs=3) as esb, \
                    tc.tile_pool(name="msb", bufs=4) as msb, \
                    tc.tile_pool(name="zps", bufs=2, space="PSUM") as zps, \
                    tc.tile_pool(name="lps", bufs=1, space="PSUM") as lps, \
                    tc.tile_pool(name="aps", bufs=2, space="PSUM") as aps:

                logits_of = {}
                xlg_of = {}
                oh_of = {}
                expv_of = {}

                def front(c):
                    T = tch[c]
                    oh_sb = esb.tile([P, 2 * TMAX * P], FP8, tag="oh", bufs=4)
                    nc.sync.dma_start(oh_sb[:, 0:2 * T * P],
                                      ohcomb[:, ohS[c] * P:ohS[c + 1] * P])
                    xlg = esb.tile([P, TMAX, TROW], FP16, tag="xlg", bufs=4)
                    segs = [(tl[c], 0, int(loS[c]), xl_lo_tab, idxlo_sb)]
                    if th[c]:
                        segs.append((th[c], tl[c], int(hiS[c]), xl_hi_tab,
                                     idxhi_sb))
                    for t_seg, off, base, tab, idx_sb_ in segs:
                        for b in range(0, t_seg, GB_T):
                            nt = min(GB_T, t_seg - b)
                            nc.gpsimd.dma_gather(
                                out_ap=xlg[:, off + b:off + b + nt, :],
                                in_ap=tab[:],
                                idxs_ap=idx_sb_[:, (base + b) * 8:
                                                (base + b + nt) * 8],
                                num_idxs=nt * P, num_idxs_reg=nt * P,
                                elem_size=TROW, queue_num=next_q())
                    logits_ps = lps.tile([P, TMAX * HEADS], FP32,
                                         tag="logits")
                    expv = esb.tile([P, TMAX * HEADS], FP16, tag="expv", bufs=4)
                    exp_done = 0
                    groups = list(range(0, T, ZG))
                    for g0 in groups:
                        gl = min(ZG, T - g0)
                        zt = zps.tile([P, 2, ZG * P], FP32, tag="zt")
                        for gi in range(gl):
                            t = g0 + gi
                            esl = slice(gi * P, (gi + 1) * P)
                            ohsl = slice(t * P, (t + 1) * P)
                            for b in range(2):
                                bsl = slice(b * P, (b + 1) * P)
                                nc.tensor.matmul(
                                    out=zt[:, b, esl],
                                    lhsT=xr_res[:, c, bsl],
                                    rhs=oh_sb[:, ohsl],
                                    start=True, stop=False)
                                nc.tensor.matmul(
                                    out=zt[:, b, esl],
                                    lhsT=xlg[:, t, bsl],
                                    rhs=id_sb[:],
                                    start=False, stop=True)
                        wt = msb.tile([P, 2, ZG * P], FP16, tag="wt",
                                      bufs=3)
                        if g0 == ZG:
                            nc.vector.tensor_scalar_max(
                                wt[:, :, 0:gl * P], zt[:, :, 0:gl * P], 0.0)
                        else:
                            nc.scalar.activation(wt[:, :, 0:gl * P],
                                                 zt[:, :, 0:gl * P], AF.Relu)
                        for gi in range(gl):
                            t = g0 + gi
                            esl = slice(gi * P, (gi + 1) * P)
                            lsl = slice(t * HEADS, (t + 1) * HEADS)
                            ohsl = slice(t * P, (t + 1) * P)
                            nc.tensor.matmul(
                                out=logits_ps[:, lsl], lhsT=id_sb[:],
                                rhs=xlg[:, t, FEAT:FEAT + HEADS],
                                start=True, stop=False)
                            nc.tensor.matmul(
                                out=logits_ps[:, lsl],
                                lhsT=oh_sb[:, ohsl],
                                rhs=xr_res[:, c, FEAT:FEAT + HEADS],
                                start=False, stop=False)
                            nc.tensor.matmul(
                                out=logits_ps[:, lsl], lhsT=wt[:, 0, esl],
                                rhs=att_sb[:, 0:HEADS],
                                start=False, stop=False)
                            nc.tensor.matmul(
                                out=logits_ps[:, lsl], lhsT=wt[:, 1, esl],
                                rhs=att_sb[:, HEADS:2 * HEADS],
                                start=False, stop=True)
                        gidx = g0 // ZG
                        done = min(g0 + gl, T)
                        if gidx % 2 == 1 or g0 == groups[-1]:
                            nc.scalar.activation(
                                expv[:, exp_done * HEADS:done * HEADS],
                                logits_ps[:, exp_done * HEADS:done * HEADS],
                                AF.Exp)
                            exp_done = done
                    xlg_of[c] = xlg
                    oh_of[c] = oh_sb
                    expv_of[c] = expv

                def back_b(c):
                    T = tch[c]
                    xlg = xlg_of.pop(c)
                    oh_sb = oh_of.pop(c)
                    expv = expv_of.pop(c)
                    agg = aps.tile([P, ROW], FP32, tag="agg")
                    JH = ROW // HEADS
                    for t0 in range(0, T, 2):
                        tn = min(2, T - t0)
                        msg = msb.tile([P, 2, ROW], FP16, tag="msg")
                        a4 = expv[:, t0 * HEADS:(t0 + tn) * HEADS] \
                            .rearrange("p (t o h) -> p t o h", t=tn, o=1) \
                            .to_broadcast([P, tn, JH, HEADS])
                        nc.vector.tensor_tensor(
                            msg[:, 0:tn, :].rearrange(
                                "p t (j h) -> p t j h", h=HEADS),
                            xlg[:, t0:t0 + tn, 0:ROW].rearrange(
                                "p t (j h) -> p t j h", h=HEADS),
                            a4, ALU.mult)
                        for ti in range(tn):
                            t = t0 + ti
                            nc.tensor.matmul(
                                out=agg[:],
                                lhsT=oh_sb[:, (T + t) * P:(T + t + 1) * P],
                                rhs=msg[:, ti, :], start=(t == 0),
                                stop=(t == T - 1))
                    den = msb.tile([P, HEADS], FP32, tag="den")
                    nc.vector.tensor_scalar_add(den[:], agg[:, ROW - HEADS:],
                                                1e-16)
                    dinv = msb.tile([P, HEADS], FP32, tag="dinv")
                    nc.vector.reciprocal(dinv[:], den[:])
                    scl = msb.tile([P, FEAT], FP32, tag="scl")
                    di4 = dinv[:].rearrange("p (o h) -> p o h", o=1) \
                        .to_broadcast([P, HID, HEADS])
                    nc.vector.tensor_tensor(
                        scl[:].rearrange("p (j h) -> p j h", h=HEADS),
                        agg[:, 0:FEAT].rearrange("p (j h) -> p j h", h=HEADS),
                        di4, ALU.mult)
                    ored = msb.tile([P, HID], FP32, tag="ored")
                    nc.vector.tensor_reduce(
                        out=ored[:],
                        in_=scl[:].rearrange("p (j h) -> p j h", h=HEADS),
                        axis=mybir.AxisListType.X, op=ALU.add)
                    obias = msb.tile([P, HID], FP32, tag="obias")
                    nc.vector.tensor_add(obias[:], ored[:], gbias_sb[:])
                    nc.vector.tensor_scalar_max(orelu_res[:, c, :],
                                                obias[:], 0.0)

                def classify(c0, cl, fsb, fps):
                    # one PSUM bank: ot at fp16 cols 0:256, fin as fp32
                    # view of fp16 cols 512:576
                    cls_ps = fps.tile([P, 1024], FP16, tag="cls_ps")
                    for ci in range(cl):
                        nc.tensor.transpose(
                            out=cls_ps[0:HID, ci * P:(ci + 1) * P],
                            in_=orelu_res[:, c0 + ci, :], identity=id_sb[:])
                    ot16 = fsb.tile([HID, 2 * P], FP16, tag="ot16")
                    nc.scalar.copy(ot16[:, 0:cl * P],
                                   cls_ps[0:HID, 0:cl * P])
                    fin_ps = cls_ps[:, 512:576].bitcast(FP32)
                    for ci in range(cl):
                        nc.tensor.matmul(
                            out=fin_ps[:, ci * OUT_DIM:(ci + 1) * OUT_DIM],
                            lhsT=ot16[:, ci * P:(ci + 1) * P],
                            rhs=wcls_sb[:], start=True, stop=False)
                        nc.tensor.matmul(
                            out=fin_ps[:, ci * OUT_DIM:(ci + 1) * OUT_DIM],
                            lhsT=o1_sb[:],
                            rhs=bcls_sb[:, ci * OUT_DIM:(ci + 1) * OUT_DIM],
                            start=False, stop=True)
                    nc.vector.tensor_copy(
                        fin_res[:, c0:c0 + cl, :].rearrange(
                            "p c o -> p (c o)"),
                        fin_ps[:, 0:cl * OUT_DIM])

                with tc.tile_pool(name="fsb", bufs=2) as fsb, \
                        tc.tile_pool(name="fps", bufs=1,
                                     space="PSUM") as fps2:
                    front(0)
                    if C_CHUNKS > 1:
                        front(1)
                    for c in range(C_CHUNKS):
                        if c + 2 < C_CHUNKS:
                            front(c + 2)
                        back_b(c)
                        if c % 2 == 1:
                            classify(c - 1, 2, fsb, fps2)
                        if c % 8 == 7:
                            nc.sync.dma_start(
                                out_ext[:, (c - 7) * OUT_DIM:
                                        (c + 1) * OUT_DIM].rearrange(
                                    "p (c o) -> p c o", o=OUT_DIM),
                                fin_res[:, c - 7:c + 1, :])
                    if C_CHUNKS % 2:
                        classify(C_CHUNKS - 1, 1, fsb, fps2)
                    rem0 = (C_CHUNKS // 8) * 8
                    if rem0 < C_CHUNKS:
                        nc.sync.dma_start(
                            out_ext[:, rem0 * OUT_DIM:].rearrange(
                                "p (c o) -> p c o", o=OUT_DIM),
                            fin_res[:, rem0:, :])

    nc.compile()
    return nc


# ----------------------------------------------------------------------------
# Host-side data preparation
# ----------------------------------------------------------------------------

def prepare_host(x, edge_index, W_in, b_in, prototypes, W_l, b_l, W_r, b_r,
                 att, gat_bias, W_cls, b_cls, n_cores):
    x = np.asarray(x, np.float32)
    W_in = np.asarray(W_in, np.float32)
    b_in = np.asarray(b_in, np.float32)
    prototypes = np.asarray(prototypes, np.float32)
    W_l = np.asarray(W_l, np.float32)
    b_l = np.asarray(b_l, np.float32)
    W_r = np.asarray(W_r, np.float32)
    b_r = np.asarray(b_r, np.float32)
    att = np.asarray(att, np.float32)
    gat_bias = np.asarray(gat_bias, np.float32)
    W_cls = np.asarray(W_cls, np.float32)
    b_cls = np.asarray(b_cls, np.float32)

    n = x.shape[0]
    NB4 = NB * P

    n_nodes_pad = _cdiv(n, NB4) * NB4
    npc_chunks = _cdiv(_cdiv(n, n_cores), P) * P
    nodes_per_core = npc_chunks        # 128-aligned ownership
    npc_dense = npc_chunks
    c_chunks = npc_chunks // P
    NT = n_nodes_pad // P
    STRIDE8 = NT // c_chunks
    # per-core tile permutation: rotated pos 8j holds own tile (49k+j)
    perms = []
    invs = []
    for k in range(n_cores):
        own = c_chunks * k + np.arange(c_chunks)
        foreign = np.setdiff1d(np.arange(NT), own)
        perm = np.empty(NT, np.int64)
        perm[np.arange(c_chunks) * STRIDE8] = own
        mask = np.ones(NT, bool)
        mask[np.arange(c_chunks) * STRIDE8] = False
        perm[mask] = foreign
        inv = np.empty(NT, np.int64)
        inv[perm] = np.arange(NT)
        perms.append(perm)
        invs.append(inv)
    inv_all = np.stack(invs)          # [k, NT]

    # --- edge bucketing ---
    src = np.asarray(edge_index[0], dtype=np.int64)
    dst = np.asarray(edge_index[1], dtype=np.int64)
    loop = np.arange(n, dtype=np.int64)
    src = np.concatenate([src, loop])
    dst = np.concatenate([dst, loop])

    core = dst // nodes_per_core
    dstl = dst - core * nodes_per_core
    chunk = dstl // P
    src_rot = inv_all[core, src // P] * P + src % P
    seg = (src_rot >= LO_ROWS).astype(np.int64)

    counts = np.zeros((n_cores, c_chunks, 2), dtype=np.int64)
    np.add.at(counts, (core, chunk, seg), 1)
    # ragged per-chunk tile counts: max over cores
    tl = [int(_cdiv(int(counts[:, c, 0].max()), P)) for c in range(c_chunks)]
    th = [int(_cdiv(int(counts[:, c, 1].max()), P)) for c in range(c_chunks)]
    tl = [max(t, 1) for t in tl]
    tch = [a + b for a, b in zip(tl, th)]
    loS = np.concatenate([[0], np.cumsum(tl)]).astype(np.int64)
    hiS = np.concatenate([[0], np.cumsum(th)]).astype(np.int64)
    ohS = np.concatenate([[0], np.cumsum([2 * t for t in tch])]) \
        .astype(np.int64)

    order = np.lexsort((seg, chunk, core))
    src_o, core_o, chunk_o, dstl_o, seg_o = (src[order], core[order],
                                             chunk[order], dstl[order],
                                             seg[order])
    bounds = np.zeros(n_cores * c_chunks * 2 + 1, dtype=np.int64)
    np.cumsum(counts.reshape(-1), out=bounds[1:])
    flat_bucket = (core_o * c_chunks + chunk_o) * 2 + seg_o
    pos = np.arange(len(src_o)) - bounds[flat_bucket]

    lo_slots = int(loS[-1]) * P
    hi_slots = int(hiS[-1]) * P
    idx_lo_slot = np.zeros((n_cores, lo_slots), dtype=np.int32)
    idx_hi_slot = np.zeros((n_cores, max(hi_slots, 1)), dtype=np.int32)
    # nloc in per-chunk tile space for one-hot build
    nloc_lo = np.full((n_cores, lo_slots), -1, dtype=np.int32)
    nloc_hi = np.full((n_cores, max(hi_slots, 1)), -1, dtype=np.int32)

    lo_base = loS[chunk_o] * P + pos
    hi_base = hiS[chunk_o] * P + pos
    is_lo = seg_o == 0
    srcrot_o = src_rot[order]
    idx_lo_slot[core_o[is_lo], lo_base[is_lo]] = \
        srcrot_o[is_lo].astype(np.int32)
    nloc_lo[core_o[is_lo], lo_base[is_lo]] = \
        (dstl_o[is_lo] - chunk_o[is_lo] * P).astype(np.int32)
    is_hi = ~is_lo
    idx_hi_slot[core_o[is_hi], hi_base[is_hi]] = \
        (srcrot_o[is_hi] - LO_ROWS).astype(np.int32)
    nloc_hi[core_o[is_hi], hi_base[is_hi]] = \
        (dstl_o[is_hi] - chunk_o[is_hi] * P).astype(np.int32)

    def wrap16(vals):
        # [k, S*128] -> [k, 128, S*8] int16
        S = vals.shape[1] // P
        v = vals.reshape(n_cores, S * 8, 16)
        v = np.transpose(v, (0, 2, 1))
        v = np.tile(v, (1, 8, 1))
        return np.ascontiguousarray(v).astype(np.int16)

    idx_lo = wrap16(idx_lo_slot)
    idx_hi = wrap16(idx_hi_slot) if hi_slots else None

    # --- one-hot (fp8), ragged layout ---
    iota = np.arange(P, dtype=np.int32)
    ohcomb = np.zeros((n_cores, P, int(ohS[-1]) * P), dtype=f8)
    for c in range(c_chunks):
        T = tch[c]
        nl = np.concatenate(
            [nloc_lo[:, loS[c] * P:loS[c + 1] * P],
             nloc_hi[:, hiS[c] * P:hiS[c + 1] * P]], axis=1) \
            .reshape(n_cores, T, P)
        oh = (nl[..., None] == iota)                 # [k, t, e, n]
        base = int(ohS[c]) * P
        ohcomb[:, :, base:base + T * P] = np.transpose(oh, (0, 3, 1, 2)) \
            .reshape(n_cores, P, T * P)
        ohcomb[:, :, base + T * P:base + 2 * T * P] = \
            np.transpose(oh, (0, 2, 1, 3)).reshape(n_cores, P, T * P)

    # --- weights ---
    att_blk = np.zeros((FEAT, HEADS), dtype=np.float32)
    for h in range(HEADS):
        att_blk[h * HID:(h + 1) * HID, h] = att[h]
    p_norm = prototypes / (np.linalg.norm(prototypes, axis=1, keepdims=True)
                           + 1e-12)
    Q_l = p_norm.T @ W_l[HID:HID + 2]
    Q_r = p_norm.T @ W_r[HID:HID + 2]

    perm = np.zeros(FEAT, np.int64)
    for h in range(HEADS):
        for j in range(HID):
            perm[j * HEADS + h] = h * HID + j

    def ext264(w, b, with_ones):
        w264 = np.concatenate(
            [w[:, perm], 0.2 * (w @ att_blk),
             np.zeros((w.shape[0], HEADS), np.float32)], axis=1)
        ones = np.ones(HEADS, np.float32) if with_ones else \
            np.zeros(HEADS, np.float32)
        b264 = np.concatenate([b[perm], 0.2 * (b @ att_blk), ones])[None, :]
        return w264, b264

    wl264, bl264 = ext264(W_l[:HID], b_l, True)
    ql264, _ = ext264(Q_l, b_l * 0, False)
    wr264, br264 = ext264(W_r[:HID], b_r, True)
    qr264, _ = ext264(Q_r, b_r * 0, False)
    wql_ = np.concatenate([wl264, bl264], axis=0).astype(f16)
    wqr_ = np.concatenate([wr264, br264], axis=0).astype(f16)

    att8 = np.zeros((P, 2 * HEADS), np.float32)
    for b in range(2):
        for p in range(P):
            cprime = b * P + p
            j, h = cprime // HEADS, cprime % HEADS
            att8[p, b * HEADS + h] = 0.8 * att[h, j]

    def swizzle(xa, npad):
        G = npad // NB4
        xp = np.zeros((npad, IN_DIM), dtype=np.float32)
        xp[:len(xa)] = xa
        v = xp.reshape(G, NB, P, 2, P)
        v = np.transpose(v, (0, 4, 3, 1, 2))
        return np.ascontiguousarray(v.reshape(G, P, 2, NB * P)).astype(f16)

    xp_t = np.zeros((n_nodes_pad, IN_DIM), np.float32)
    xp_t[:n] = x
    xp_t = xp_t.reshape(NT, P, IN_DIM)
    xg_rot = [swizzle(xp_t[perms[k]].reshape(n_nodes_pad, IN_DIM),
                      n_nodes_pad) for k in range(n_cores)]

    shared = {
        "w_in_a": W_in[:P].astype(f16), "w_in_b": W_in[P:].astype(f16),
        "b_in_c": b_in[:, None].astype(f16),
        "wql": wql_, "ql": ql264.astype(f16),
        "wqr": wqr_, "qr": qr264.astype(f16),
        "att8": att8.astype(f16),
        "ident": np.eye(P, dtype=f16),
        "ones6464": np.ones((HID, HID), f16),
        "o1_128": np.ones((1, P), f16),
        "gbias4": np.broadcast_to(4.0 * gat_bias.astype(np.float32),
                                  (P, HID)).copy(),
        "wcls4": (0.25 * W_cls).astype(f16),
        "bcls4": np.tile(b_cls, NB)[None, :].astype(f16),
    }
    in_maps = []
    for k in range(n_cores):
        m = dict(shared)
        m["xg_all"] = xg_rot[k]
        m["idx_lo"] = idx_lo[k]
        if idx_hi is not None:
            m["idx_hi"] = idx_hi[k]
        m["ohcomb"] = ohcomb[k]
        in_maps.append(m)
    return (in_maps, n_nodes_pad, npc_dense, npc_chunks, tuple(tl),
            tuple(th))


_CACHE = {}


def run(inputs, n_cores=8, trace=False):
    x = np.asarray(inputs["x"])
    n = x.shape[0]
    in_maps, n_nodes_pad, npc_dense, npc_chunks, tl, th = prepare_host(
        x, np.asarray(inputs["edge_index"]), np.asarray(inputs["W_in"]),
        np.asarray(inputs["b_in"]), np.asarray(inputs["prototypes"]),
        np.asarray(inputs["W_l"]), np.asarray(inputs["b_l"]),
        np.asarray(inputs["W_r"]), np.asarray(inputs["b_r"]),
        np.asarray(inputs["att"]), np.asarray(inputs["gat_bias"]),
        np.asarray(inputs["W_cls"]), np.asarray(inputs["b_cls"]), n_cores)
    key = (n_nodes_pad, npc_dense, npc_chunks, tl, th, n_cores)
    if key not in _CACHE:
        _CACHE[key] = build_program(*key)
    nc = _CACHE[key]
    res = run_bass_kernel_spmd(nc, in_maps, list(range(n_cores)), trace=trace)
    c_chunks = npc_chunks // P
    outs = []
    for k in range(n_cores):
        o = np.asarray(res.results[k]["out"]).reshape(P, c_chunks, OUT_DIM)
        outs.append(np.transpose(o, (1, 0, 2)).reshape(npc_chunks, OUT_DIM))
    return np.concatenate(outs, axis=0)[:n], res


def kernel(**inputs):
    out, _ = run(inputs, n_cores=8)
    return out.astype(np.float32)


# revision 47
# speedup vs baseline: 3.0711x; 1.0240x over previous
"""GATv2-based CGNN forward pass on 8 Trainium2 NeuronCores.

Strategy (dst-node sharded, no collectives):
  - Each core owns N/8 destination nodes. Host buckets edges (incl. self
    loops) by dst core, then by 128-node dst chunk within the core.
    Per-chunk tile counts are ragged (max over cores per chunk index) so
    pad work tracks the actual edge distribution.
  - Dense phase (replicated for xl over all nodes; own nodes for xr):
    xl/xr rows are [feat256 head-interleaved (c' = j*4+h), beta4, ones4]
    fp16, where beta = 0.2*(feat @ att_blk) so that
      logit = 0.8*sum_c att_c*relu(z_c) + beta_l[s] + beta_r[d]
    (exact rewrite of att . leaky_relu via |z| = 2 relu(z) - z).
  - Edge phase per 128-dst chunk: batched indirect-DMA gather of xl[src]
    rows (4 SWDGE queues); z built TRANSPOSED in PSUM ([channel, edge])
    from a one-hot matmul of resident xr plus identity-matmul transpose
    of the gathered xl; one relu Activation moves it to SBUF; per-tile
    logits come from tall-skinny matmuls with the relu'd block as lhsT.
    Messages are one fp16 tensor_tensor multiply (broadcast alpha),
    scatter-added via fp8 one-hot matmuls; denominators ride along as
    ones*alpha columns.
  - Finish: per-chunk softmax normalize + head mean (0.25 folded into
    W_cls), relu; classifier runs as a final batched phase.
"""

import os
import sys

import numpy as np
import ml_dtypes

for _p in ("/opt/trn_rl_repo",):
    if _p not in sys.path and os.path.isdir(_p):
        sys.path.insert(0, _p)

import concourse.bass as bass
import concourse.tile as tile
from concourse import bacc, mybir
from concourse.bass_utils import run_bass_kernel_spmd

FP16 = mybir.dt.float16
FP32 = mybir.dt.float32
FP8 = mybir.dt.float8e4
INT16 = mybir.dt.int16
AF = mybir.ActivationFunctionType
ALU = mybir.AluOpType

P = 128
HID = 64
HEADS = 4
OUT_DIM = 16
IN_DIM = 256
FEAT = HEADS * HID          # 256
ROW = FEAT + 2 * HEADS      # 264 = feat + beta + ones
TROW = 384                  # padded table row (768B, 256B-aligned)
LO_ROWS = 32768             # int16 index range per gather table
NB = 4                      # node tiles per dense group

f16 = ml_dtypes.float16 if hasattr(ml_dtypes, "float16") else np.float16
f8 = ml_dtypes.float8_e4m3


def _cdiv(a, b):
    return (a + b - 1) // b


# ----------------------------------------------------------------------------
# Device program
# ----------------------------------------------------------------------------

def build_program(n_nodes_pad, npc_dense, npc_chunks, tl, th, n_cores):
    """tl/th: per-chunk lo/hi gather tile counts (tuples, shared by cores)."""
    GA = n_nodes_pad // (NB * P)
    C_CHUNKS = npc_chunks // P
    OWN_STRIDE = (n_nodes_pad // P) // C_CHUNKS   # own tiles every 8th
    tl = list(tl)
    th = list(th)
    tch = [a + b for a, b in zip(tl, th)]
    TMAX = max(tch)
    loS = np.concatenate([[0], np.cumsum(tl)]).astype(int)   # tile offsets
    hiS = np.concatenate([[0], np.cumsum(th)]).astype(int)
    ohS = np.concatenate([[0], np.cumsum([2 * t for t in tch])]).astype(int)
    hi_rows = max(n_nodes_pad - LO_ROWS, P)
    xr_slots = C_CHUNKS

    nc = bacc.Bacc("TRN2", target_bir_lowering=False, debug=False,
                   num_devices=n_cores, num_swdge_queues=4)

    def din(name, shape, dtype=FP16):
        return nc.dram_tensor(name, shape, dtype, kind="ExternalInput").ap()

    xg_all = din("xg_all", [GA, P, 2, NB * P])
    w_in_a = din("w_in_a", [P, HID])
    w_in_b = din("w_in_b", [P, HID])
    b_in_c = din("b_in_c", [HID, 1])
    wql = din("wql", [HID + 1, ROW])       # [w264; b264]
    ql = din("ql", [HID, ROW])
    wqr = din("wqr", [HID + 1, ROW])
    qr = din("qr", [HID, ROW])
    att8 = din("att8", [P, 2 * HEADS])
    ident = din("ident", [P, P])
    ones6464 = din("ones6464", [HID, HID])
    o1_128 = din("o1_128", [1, P])
    gbias4 = din("gbias4", [P, HID], FP32)
    wcls4 = din("wcls4", [HID, OUT_DIM])
    bcls4 = din("bcls4", [1, NB * OUT_DIM])
    idx_lo = din("idx_lo", [P, int(loS[-1]) * 8], INT16)
    idx_hi = (din("idx_hi", [P, int(hiS[-1]) * 8], INT16)
              if hiS[-1] else None)
    ohcomb = din("ohcomb", [P, int(ohS[-1]) * P], FP8)

    out_ext = nc.dram_tensor("out", [P, C_CHUNKS * OUT_DIM], FP32,
                             kind="ExternalOutput").ap()

    xl_lo_tab = nc.dram_tensor("xl_lo_tab", [min(n_nodes_pad, LO_ROWS), TROW],
                               FP16).ap()
    xl_hi_tab = nc.dram_tensor("xl_hi_tab", [hi_rows, TROW], FP16).ap()

    qctr = [0]

    def next_q():
        qctr[0] = (qctr[0] + 1) % 4
        return qctr[0]

    with tile.TileContext(nc) as tc:
        with tc.tile_pool(name="consts", bufs=1) as cp:
            w_in_a_sb = cp.tile([P, HID], FP16)
            nc.sync.dma_start(w_in_a_sb[:], w_in_a[:])
            w_in_b_sb = cp.tile([P, HID], FP16)
            nc.sync.dma_start(w_in_b_sb[:], w_in_b[:])
            b_in_sb = cp.tile([HID, 1], FP16)
            nc.sync.dma_start(b_in_sb[:], b_in_c[:])
            wql_sb = cp.tile([HID + 1, ROW], FP16)
            nc.sync.dma_start(wql_sb[:], wql[:])
            ql_sb = cp.tile([HID, ROW], FP16)
            nc.sync.dma_start(ql_sb[:], ql[:])
            wqr_sb = cp.tile([HID + 1, ROW], FP16)
            nc.sync.dma_start(wqr_sb[:], wqr[:])
            qr_sb = cp.tile([HID, ROW], FP16)
            nc.sync.dma_start(qr_sb[:], qr[:])
            ones64_sb = cp.tile([HID, HID], FP16)
            nc.sync.dma_start(ones64_sb[:], ones6464[:])
            att_sb = cp.tile([P, 2 * HEADS], FP16)
            id_sb = cp.tile([P, P], FP16)
            o1_sb = cp.tile([1, P], FP16)
            gbias_sb = cp.tile([P, HID], FP32)
            wcls_sb = cp.tile([HID, OUT_DIM], FP16)
            bcls_sb = cp.tile([1, NB * OUT_DIM], FP16)
            idxlo_sb = cp.tile([P, int(loS[-1]) * 8], INT16)
            idxhi_sb = (cp.tile([P, int(hiS[-1]) * 8], INT16, name="idxhi_sb")
                        if idx_hi is not None else None)
            eps_sb = cp.tile([HID, 1], FP32)
            nc.gpsimd.memset(eps_sb[:], 1e-12)

            xr_res = cp.tile([P, xr_slots, ROW], FP16)
            orelu_res = cp.tile([P, C_CHUNKS, HID], FP16)
            fin_res = cp.tile([P, C_CHUNKS, OUT_DIM], FP32)

            # ---------------- dense phase ----------------
            W = NB * P

            def dense_stage1(g, xg, wq_sb, q_sb, to_table, sb, ps):
                xsb = sb.tile([P, 2, W], FP16, tag="xsb")
                nc.sync.dma_start(xsb[:], xg[g])
                ht_ps = ps.tile([HID, W], FP32, tag="ht_ps")
                nc.tensor.matmul(out=ht_ps[:], lhsT=w_in_a_sb[:],
                                 rhs=xsb[:, 0, :], start=True, stop=False)
                nc.tensor.matmul(out=ht_ps[:], lhsT=w_in_b_sb[:],
                                 rhs=xsb[:, 1, :], start=False, stop=True)
                hta = sb.tile([HID + 1, W], FP16, tag="hta", bufs=5)
                nc.scalar.activation(hta[0:HID, :], ht_ps[:], AF.Relu,
                                     bias=b_in_sb[:])
                nc.gpsimd.memset(hta[HID:HID + 1, :], 1.0)
                rsq = sb.tile([HID, W], FP16, tag="rsq")
                nc.vector.tensor_mul(rsq[:], hta[0:HID, :], hta[0:HID, :])
                nrm2_ps = ps.tile([HID, W], FP32, tag="nrm2_ps")
                nc.tensor.matmul(out=nrm2_ps[:], lhsT=ones64_sb[:],
                                 rhs=rsq[:], start=True, stop=True)
                return hta, nrm2_ps

            def dense_stage2(hta, nrm2_ps, sb, ps):
                nrmr = sb.tile([HID, W], FP32, tag="nrmr")
                nc.scalar.activation(nrmr[:], nrm2_ps[:], AF.Sqrt,
                                     bias=eps_sb[:])
                invr = sb.tile([HID, W], FP32, tag="invr")
                nc.vector.reciprocal(invr[:], nrmr[:])
                htn = sb.tile([HID, W], FP16, tag="htn", bufs=3)
                nc.gpsimd.tensor_mul(htn[:], hta[0:HID, :], invr[:])
                return htn

            def dense_stage3(g, wq_sb, q_sb, to_table, hta, htn, sb, ps):
                dst4 = sb.tile([P, NB, ROW], FP16, tag="dst4",
                               name="dst4")
                for half in range(2):
                    xl2 = ps.tile([P, 2, 2 * ROW - 16], FP32, tag="xl2")
                    for ti in range(2):
                        t = half * 2 + ti
                        tsl = slice(t * P, (t + 1) * P)
                        nc.tensor.matmul(out=xl2[:, ti, 0:ROW],
                                         lhsT=hta[:, tsl],
                                         rhs=wq_sb[:], start=True, stop=False)
                        nc.tensor.matmul(out=xl2[:, ti, 0:ROW],
                                         lhsT=htn[:, tsl],
                                         rhs=q_sb[:], start=False, stop=True)
                    mv_out = dst4[:, half * 2:half * 2 + 2, :]
                    # alternate engines for the PSUM->SBUF move
                    if half == 0:
                        nc.scalar.copy(mv_out, xl2[:, :, 0:ROW])
                    else:
                        nc.vector.tensor_copy(mv_out, xl2[:, :, 0:ROW])
                    # at most one own tile per half (own = every 8th tile)
                    for ti in range(2):
                        gt = g * NB + half * 2 + ti
                        if gt % OWN_STRIDE == 0 and gt // OWN_STRIDE < \
                                C_CHUNKS:
                            t = half * 2 + ti
                            tsl = slice(t * P, (t + 1) * P)
                            xr2 = ps.tile([P, 2, 2 * ROW - 16], FP32,
                                          tag="xl2", name="xr2")
                            nc.tensor.matmul(out=xr2[:, 0, 0:ROW],
                                             lhsT=hta[:, tsl],
                                             rhs=wqr_sb[:], start=True,
                                             stop=False)
                            nc.tensor.matmul(out=xr2[:, 0, 0:ROW],
                                             lhsT=htn[:, tsl],
                                             rhs=qr_sb[:], start=False,
                                             stop=True)
                            if half == 0:
                                nc.vector.tensor_copy(
                                    xr_res[:, gt // OWN_STRIDE, :],
                                    xr2[:, 0, 0:ROW])
                            else:
                                nc.scalar.copy(
                                    xr_res[:, gt // OWN_STRIDE, :],
                                    xr2[:, 0, 0:ROW])
                r = g * NB * P
                if r < LO_ROWS:
                    sink = xl_lo_tab[r:r + NB * P]
                else:
                    sink = xl_hi_tab[r - LO_ROWS:r - LO_ROWS + NB * P]
                nc.sync.dma_start(
                    sink.rearrange("(t p) c -> p t c", p=P)[:, :, 0:ROW],
                    dst4[:])

            with tc.tile_pool(name="dsb", bufs=3) as dsb, \
                    tc.tile_pool(name="dps", bufs=2, space="PSUM") as dps:
                specs = [(g, xg_all, wql_sb, ql_sb, True)
                         for g in range(GA)]
                NG = len(specs)
                s1out = {}
                s2out = {}
                for i in range(NG + 2):
                    if i < NG:
                        g, xg, wq_sb, q_sb, tt = specs[i]
                        s1out[i] = dense_stage1(g, xg, wq_sb, q_sb, tt,
                                                dsb, dps)
                    if 1 <= i and i - 1 < NG:
                        hta, nrm2_ps = s1out[i - 1]
                        s2out[i - 1] = dense_stage2(hta, nrm2_ps, dsb, dps)
                    if 2 <= i and i - 2 < NG:
                        g, xg, wq_sb, q_sb, tt = specs[i - 2]
                        hta, _ = s1out.pop(i - 2)
                        dense_stage3(g, wq_sb, q_sb, tt, hta,
                                     s2out.pop(i - 2), dsb, dps)

            # edge/classifier consts (emitted late so dense starts sooner)
            nc.sync.dma_start(att_sb[:], att8[:])
            nc.sync.dma_start(id_sb[:], ident[:])
            nc.sync.dma_start(o1_sb[:], o1_128[:])
            nc.sync.dma_start(gbias_sb[:], gbias4[:])
            nc.sync.dma_start(wcls_sb[:], wcls4[:])
            nc.sync.dma_start(bcls_sb[:], bcls4[:])
            nc.sync.dma_start(idxlo_sb[:], idx_lo[:])
            if idx_hi is not None:
                nc.sync.dma_start(idxhi_sb[:], idx_hi[:])

            # ---------------- edge phase ----------------
            GB_T = 8            # tiles per dma_gather call (<=1024 indices)
            ZG = 4              # tiles per z-group (PSUM bank pair)

            with tc.tile_pool(name="esb", bufs=3) as esb, \
                    tc.tile_pool(name="msb", bufs=4) as msb, \
                    tc.tile_pool(name="zps", bufs=2, space="PSUM") as zps, \
                    tc.tile_pool(name="lps", bufs=1, space="PSUM") as lps, \
                    tc.tile_pool(name="aps", bufs=2, space="PSUM") as aps:

                logits_of = {}
                xlg_of = {}
                oh_of = {}
                expv_of = {}

                def front(c):
                    T = tch[c]
                    oh_sb = esb.tile([P, 2 * TMAX * P], FP8, tag="oh", bufs=5)
                    nc.sync.dma_start(oh_sb[:, 0:2 * T * P],
                                      ohcomb[:, ohS[c] * P:ohS[c + 1] * P])
                    xlg = esb.tile([P, TMAX, TROW], FP16, tag="xlg", bufs=5)
                    segs = [(tl[c], 0, int(loS[c]), xl_lo_tab, idxlo_sb)]
                    if th[c]:
                        segs.append((th[c], tl[c], int(hiS[c]), xl_hi_tab,
                                     idxhi_sb))
                    for t_seg, off, base, tab, idx_sb_ in segs:
                        for b in range(0, t_seg, GB_T):
                            nt = min(GB_T, t_seg - b)
                            nc.gpsimd.dma_gather(
                                out_ap=xlg[:, off + b:off + b + nt, :],
                                in_ap=tab[:],
                                idxs_ap=idx_sb_[:, (base + b) * 8:
                                                (base + b + nt) * 8],
                                num_idxs=nt * P, num_idxs_reg=nt * P,
                                elem_size=TROW, queue_num=next_q())
                    logits_ps = lps.tile([P, TMAX * HEADS], FP32,
                                         tag="logits")
                    expv = esb.tile([P, TMAX * HEADS], FP16, tag="expv", bufs=4)
                    exp_done = 0
                    groups = list(range(0, T, ZG))
                    for g0 in groups:
                        gl = min(ZG, T - g0)
                        zt = zps.tile([P, 2, ZG * P], FP32, tag="zt")
                        for gi in range(gl):
                            t = g0 + gi
                            esl = slice(gi * P, (gi + 1) * P)
                            ohsl = slice(t * P, (t + 1) * P)
                            for b in range(2):
                                bsl = slice(b * P, (b + 1) * P)
                                nc.tensor.matmul(
                                    out=zt[:, b, esl],
                                    lhsT=xr_res[:, c, bsl],
                                    rhs=oh_sb[:, ohsl],
                                    start=True, stop=False)
                                nc.tensor.matmul(
                                    out=zt[:, b, esl],
                                    lhsT=xlg[:, t, bsl],
                                    rhs=id_sb[:],
                                    start=False, stop=True)
                        wt = msb.tile([P, 2, ZG * P], FP16, tag="wt",
                                      bufs=3)
                        if g0 == ZG:
                            nc.vector.tensor_scalar_max(
                                wt[:, :, 0:gl * P], zt[:, :, 0:gl * P], 0.0)
                        else:
                            nc.scalar.activation(wt[:, :, 0:gl * P],
                                                 zt[:, :, 0:gl * P], AF.Relu)
                        for gi in range(gl):
                            t = g0 + gi
                            esl = slice(gi * P, (gi + 1) * P)
                            lsl = slice(t * HEADS, (t + 1) * HEADS)
                            ohsl = slice(t * P, (t + 1) * P)
                            nc.tensor.matmul(
                                out=logits_ps[:, lsl], lhsT=id_sb[:],
                                rhs=xlg[:, t, FEAT:FEAT + HEADS],
                                start=True, stop=False)
                            nc.tensor.matmul(
                                out=logits_ps[:, lsl],
                                lhsT=oh_sb[:, ohsl],
                                rhs=xr_res[:, c, FEAT:FEAT + HEADS],
                                start=False, stop=False)
                            nc.tensor.matmul(
                                out=logits_ps[:, lsl], lhsT=wt[:, 0, esl],
                                rhs=att_sb[:, 0:HEADS],
                                start=False, stop=False)
                            nc.tensor.matmul(
                                out=logits_ps[:, lsl], lhsT=wt[:, 1, esl],
                                rhs=att_sb[:, HEADS:2 * HEADS],
                                start=False, stop=True)
                        gidx = g0 // ZG
                        done = min(g0 + gl, T)
                        if gidx % 2 == 1 or g0 == groups[-1]:
                            nc.scalar.activation(
                                expv[:, exp_done * HEADS:done * HEADS],
                                logits_ps[:, exp_done * HEADS:done * HEADS],
                                AF.Exp)
                            exp_done = done
                    xlg_of[c] = xlg
                    oh_of[c] = oh_sb
                    expv_of[c] = expv

                def back_b(c):
                    T = tch[c]
                    xlg = xlg_of.pop(c)
                    oh_sb = oh_of.pop(c)
                    expv = expv_of.pop(c)
                    agg = aps.tile([P, ROW], FP32, tag="agg")
                    JH = ROW // HEADS
                    for t0 in range(0, T, 2):
                        tn = min(2, T - t0)
                        msg = msb.tile([P, 2, ROW], FP16, tag="msg")
                        a4 = expv[:, t0 * HEADS:(t0 + tn) * HEADS] \
                            .rearrange("p (t o h) -> p t o h", t=tn, o=1) \
                            .to_broadcast([P, tn, JH, HEADS])
                        nc.vector.tensor_tensor(
                            msg[:, 0:tn, :].rearrange(
                                "p t (j h) -> p t j h", h=HEADS),
                            xlg[:, t0:t0 + tn, 0:ROW].rearrange(
                                "p t (j h) -> p t j h", h=HEADS),
                            a4, ALU.mult)
                        for ti in range(tn):
                            t = t0 + ti
                            nc.tensor.matmul(
                                out=agg[:],
                                lhsT=oh_sb[:, (T + t) * P:(T + t + 1) * P],
                                rhs=msg[:, ti, :], start=(t == 0),
                                stop=(t == T - 1))
                    den = msb.tile([P, HEADS], FP32, tag="den")
                    nc.vector.tensor_scalar_add(den[:], agg[:, ROW - HEADS:],
                                                1e-16)
                    dinv = msb.tile([P, HEADS], FP32, tag="dinv")
                    nc.vector.reciprocal(dinv[:], den[:])
                    scl = msb.tile([P, FEAT], FP32, tag="scl")
                    di4 = dinv[:].rearrange("p (o h) -> p o h", o=1) \
                        .to_broadcast([P, HID, HEADS])
                    nc.vector.tensor_tensor(
                        scl[:].rearrange("p (j h) -> p j h", h=HEADS),
                        agg[:, 0:FEAT].rearrange("p (j h) -> p j h", h=HEADS),
                        di4, ALU.mult)
                    ored = msb.tile([P, HID], FP32, tag="ored")
                    nc.vector.tensor_reduce(
                        out=ored[:],
                        in_=scl[:].rearrange("p (j h) -> p j h", h=HEADS),
                        axis=mybir.AxisListType.X, op=ALU.add)
                    obias = msb.tile([P, HID], FP32, tag="obias")
                    nc.vector.tensor_add(obias[:], ored[:], gbias_sb[:])
                    nc.vector.tensor_scalar_max(orelu_res[:, c, :],
                                                obias[:], 0.0)

                def classify(c0, cl, fsb, fps):
                    # one PSUM bank: ot at fp16 cols 0:256, fin as fp32
                    # view of fp16 cols 512:576
                    cls_ps = fps.tile([P, 1024], FP16, tag="cls_ps")
                    for ci in range(cl):
                        nc.tensor.transpose(
                            out=cls_ps[0:HID, ci * P:(ci + 1) * P],
                            in_=orelu_res[:, c0 + ci, :], identity=id_sb[:])
                    ot16 = fsb.tile([HID, 2 * P], FP16, tag="ot16")
                    nc.scalar.copy(ot16[:, 0:cl * P],
                                   cls_ps[0:HID, 0:cl * P])
                    fin_ps = cls_ps[:, 512:576].bitcast(FP32)
                    for ci in range(cl):
                        nc.tensor.matmul(
                            out=fin_ps[:, ci * OUT_DIM:(ci + 1) * OUT_DIM],
                            lhsT=ot16[:, ci * P:(ci + 1) * P],
                            rhs=wcls_sb[:], start=True, stop=False)
                        nc.tensor.matmul(
                            out=fin_ps[:, ci * OUT_DIM:(ci + 1) * OUT_DIM],
                            lhsT=o1_sb[:],
                            rhs=bcls_sb[:, ci * OUT_DIM:(ci + 1) * OUT_DIM],
                            start=False, stop=True)
                    nc.vector.tensor_copy(
                        fin_res[:, c0:c0 + cl, :].rearrange(
                            "p c o -> p (c o)"),
                        fin_ps[:, 0:cl * OUT_DIM])

                with tc.tile_pool(name="fsb", bufs=2) as fsb, \
                        tc.tile_pool(name="fps", bufs=1,
                                     space="PSUM") as fps2:
                    front(0)
                    if C_CHUNKS > 1:
                        front(1)
                    for c in range(C_CHUNKS):
                        if c + 2 < C_CHUNKS:
                            front(c + 2)
                        back_b(c)
                        if c % 2 == 1:
                            classify(c - 1, 2, fsb, fps2)
                        if c % 8 == 7:
                            nc.sync.dma_start(
                                out_ext[:, (c - 7) * OUT_DIM:
                                        (c + 1) * OUT_DIM].rearrange(
                                    "p (c o) -> p c o", o=OUT_DIM),
                                fin_res[:, c - 7:c + 1, :])
                    if C_CHUNKS % 2:
                        classify(C_CHUNKS - 1, 1, fsb, fps2)
                    rem0 = (C_CHUNKS // 8) * 8
                    if rem0 < C_CHUNKS:
                        nc.sync.dma_start(
                            out_ext[:, rem0 * OUT_DIM:].rearrange(
                                "p (c o) -> p c o", o=OUT_DIM),
                            fin_res[:, rem0:, :])

    nc.compile()
    return nc


# ----------------------------------------------------------------------------
# Host-side data preparation
# ----------------------------------------------------------------------------

def prepare_host(x, edge_index, W_in, b_in, prototypes, W_l, b_l, W_r, b_r,
                 att, gat_bias, W_cls, b_cls, n_cores):
    x = np.asarray(x, np.float32)
    W_in = np.asarray(W_in, np.float32)
    b_in = np.asarray(b_in, np.float32)
    prototypes = np.asarray(prototypes, np.float32)
    W_l = np.asarray(W_l, np.float32)
    b_l = np.asarray(b_l, np.float32)
    W_r = np.asarray(W_r, np.float32)
    b_r = np.asarray(b_r, np.float32)
    att = np.asarray(att, np.float32)
    gat_bias = np.asarray(gat_bias, np.float32)
    W_cls = np.asarray(W_cls, np.float32)
    b_cls = np.asarray(b_cls, np.float32)

    n = x.shape[0]
    NB4 = NB * P

    n_nodes_pad = _cdiv(n, NB4) * NB4
    npc_chunks = _cdiv(_cdiv(n, n_cores), P) * P
    nodes_per_core = npc_chunks        # 128-aligned ownership
    npc_dense = npc_chunks
    c_chunks = npc_chunks // P
    NT = n_nodes_pad // P
    STRIDE8 = NT // c_chunks
    # per-core tile permutation: rotated pos 8j holds own tile (49k+j)
    perms = []
    invs = []
    for k in range(n_cores):
        own = c_chunks * k + np.arange(c_chunks)
        foreign = np.setdiff1d(np.arange(NT), own)
        perm = np.empty(NT, np.int64)
        perm[np.arange(c_chunks) * STRIDE8] = own
        mask = np.ones(NT, bool)
        mask[np.arange(c_chunks) * STRIDE8] = False
        perm[mask] = foreign
        inv = np.empty(NT, np.int64)
        inv[perm] = np.arange(NT)
        perms.append(perm)
        invs.append(inv)
    inv_all = np.stack(invs)          # [k, NT]

    # --- edge bucketing ---
    src = np.asarray(edge_index[0], dtype=np.int64)
    dst = np.asarray(edge_index[1], dtype=np.int64)
    loop = np.arange(n, dtype=np.int64)
    src = np.concatenate([src, loop])
    dst = np.concatenate([dst, loop])

    core = dst // nodes_per_core
    dstl = dst - core * nodes_per_core
    chunk = dstl // P
    src_rot = inv_all[core, src // P] * P + src % P
    seg = (src_rot >= LO_ROWS).astype(np.int64)

    counts = np.zeros((n_cores, c_chunks, 2), dtype=np.int64)
    np.add.at(counts, (core, chunk, seg), 1)
    # ragged per-chunk tile counts: max over cores
    tl = [int(_cdiv(int(counts[:, c, 0].max()), P)) for c in range(c_chunks)]
    th = [int(_cdiv(int(counts[:, c, 1].max()), P)) for c in range(c_chunks)]
    tl = [max(t, 1) for t in tl]
    tch = [a + b for a, b in zip(tl, th)]
    loS = np.concatenate([[0], np.cumsum(tl)]).astype(np.int64)
    hiS = np.concatenate([[0], np.cumsum(th)]).astype(np.int64)
    ohS = np.concatenate([[0], np.cumsum([2 * t for t in tch])]) \
        .astype(np.int64)

    order = np.lexsort((seg, chunk, core))
    src_o, core_o, chunk_o, dstl_o, seg_o = (src[order], core[order],
                                             chunk[order], dstl[order],
                                             seg[order])
    bounds = np.zeros(n_cores * c_chunks * 2 + 1, dtype=np.int64)
    np.cumsum(counts.reshape(-1), out=bounds[1:])
    flat_bucket = (core_o * c_chunks + chunk_o) * 2 + seg_o
    pos = np.arange(len(src_o)) - bounds[flat_bucket]

    lo_slots = int(loS[-1]) * P
    hi_slots = int(hiS[-1]) * P
    idx_lo_slot = np.zeros((n_cores, lo_slots), dtype=np.int32)
    idx_hi_slot = np.zeros((n_cores, max(hi_slots, 1)), dtype=np.int32)
    # nloc in per-chunk tile space for one-hot build
    nloc_lo = np.full((n_cores, lo_slots), -1, dtype=np.int32)
    nloc_hi = np.full((n_cores, max(hi_slots, 1)), -1, dtype=np.int32)

    lo_base = loS[chunk_o] * P + pos
    hi_base = hiS[chunk_o] * P + pos
    is_lo = seg_o == 0
    srcrot_o = src_rot[order]
    idx_lo_slot[core_o[is_lo], lo_base[is_lo]] = \
        srcrot_o[is_lo].astype(np.int32)
    nloc_lo[core_o[is_lo], lo_base[is_lo]] = \
        (dstl_o[is_lo] - chunk_o[is_lo] * P).astype(np.int32)
    is_hi = ~is_lo
    idx_hi_slot[core_o[is_hi], hi_base[is_hi]] = \
        (srcrot_o[is_hi] - LO_ROWS).astype(np.int32)
    nloc_hi[core_o[is_hi], hi_base[is_hi]] = \
        (dstl_o[is_hi] - chunk_o[is_hi] * P).astype(np.int32)

    def wrap16(vals):
        # [k, S*128] -> [k, 128, S*8] int16
        S = vals.shape[1] // P
        v = vals.reshape(n_cores, S * 8, 16)
        v = np.transpose(v, (0, 2, 1))
        v = np.tile(v, (1, 8, 1))
        return np.ascontiguousarray(v).astype(np.int16)

    idx_lo = wrap16(idx_lo_slot)
    idx_hi = wrap16(idx_hi_slot) if hi_slots else None

    # --- one-hot (fp8), ragged layout ---
    iota = np.arange(P, dtype=np.int32)
    ohcomb = np.zeros((n_cores, P, int(ohS[-1]) * P), dtype=f8)
    for c in range(c_chunks):
        T = tch[c]
        nl = np.concatenate(
            [nloc_lo[:, loS[c] * P:loS[c + 1] * P],
             nloc_hi[:, hiS[c] * P:hiS[c + 1] * P]], axis=1) \
            .reshape(n_cores, T, P)
        oh = (nl[..., None] == iota)                 # [k, t, e, n]
        base = int(ohS[c]) * P
        ohcomb[:, :, base:base + T * P] = np.transpose(oh, (0, 3, 1, 2)) \
            .reshape(n_cores, P, T * P)
        ohcomb[:, :, base + T * P:base + 2 * T * P] = \
            np.transpose(oh, (0, 2, 1, 3)).reshape(n_cores, P, T * P)

    # --- weights ---
    att_blk = np.zeros((FEAT, HEADS), dtype=np.float32)
    for h in range(HEADS):
        att_blk[h * HID:(h + 1) * HID, h] = att[h]
    p_norm = prototypes / (np.linalg.norm(prototypes, axis=1, keepdims=True)
                           + 1e-12)
    Q_l = p_norm.T @ W_l[HID:HID + 2]
    Q_r = p_norm.T @ W_r[HID:HID + 2]

    perm = np.zeros(FEAT, np.int64)
    for h in range(HEADS):
        for j in range(HID):
            perm[j * HEADS + h] = h * HID + j

    def ext264(w, b, with_ones):
        w264 = np.concatenate(
            [w[:, perm], 0.2 * (w @ att_blk),
             np.zeros((w.shape[0], HEADS), np.float32)], axis=1)
        ones = np.ones(HEADS, np.float32) if with_ones else \
            np.zeros(HEADS, np.float32)
        b264 = np.concatenate([b[perm], 0.2 * (b @ att_blk), ones])[None, :]
        return w264, b264

    wl264, bl264 = ext264(W_l[:HID], b_l, True)
    ql264, _ = ext264(Q_l, b_l * 0, False)
    wr264, br264 = ext264(W_r[:HID], b_r, True)
    qr264, _ = ext264(Q_r, b_r * 0, False)
    wql_ = np.concatenate([wl264, bl264], axis=0).astype(f16)
    wqr_ = np.concatenate([wr264, br264], axis=0).astype(f16)

    att8 = np.zeros((P, 2 * HEADS), np.float32)
    for b in range(2):
        for p in range(P):
            cprime = b * P + p
            j, h = cprime // HEADS, cprime % HEADS
            att8[p, b * HEADS + h] = 0.8 * att[h, j]

    def swizzle(xa, npad):
        G = npad // NB4
        xp = np.zeros((npad, IN_DIM), dtype=np.float32)
        xp[:len(xa)] = xa
        v = xp.reshape(G, NB, P, 2, P)
        v = np.transpose(v, (0, 4, 3, 1, 2))
        return np.ascontiguousarray(v.reshape(G, P, 2, NB * P)).astype(f16)

    xp_t = np.zeros((n_nodes_pad, IN_DIM), np.float32)
    xp_t[:n] = x
    xp_t = xp_t.reshape(NT, P, IN_DIM)
    xg_rot = [swizzle(xp_t[perms[k]].reshape(n_nodes_pad, IN_DIM),
                      n_nodes_pad) for k in range(n_cores)]

    shared = {
        "w_in_a": W_in[:P].astype(f16), "w_in_b": W_in[P:].astype(f16),
        "b_in_c": b_in[:, None].astype(f16),
        "wql": wql_, "ql": ql264.astype(f16),
        "wqr": wqr_, "qr": qr264.astype(f16),
        "att8": att8.astype(f16),
        "ident": np.eye(P, dtype=f16),
        "ones6464": np.ones((HID, HID), f16),
        "o1_128": np.ones((1, P), f16),
        "gbias4": np.broadcast_to(4.0 * gat_bias.astype(np.float32),
                                  (P, HID)).copy(),
        "wcls4": (0.25 * W_cls).astype(f16),
        "bcls4": np.tile(b_cls, NB)[None, :].astype(f16),
    }
    in_maps = []
    for k in range(n_cores):
        m = dict(shared)
        m["xg_all"] = xg_rot[k]
        m["idx_lo"] = idx_lo[k]
        if idx_hi is not None:
            m["idx_hi"] = idx_hi[k]
        m["ohcomb"] = ohcomb[k]
        in_maps.append(m)
    return (in_maps, n_nodes_pad, npc_dense, npc_chunks, tuple(tl),
            tuple(th))


_CACHE = {}


def run(inputs, n_cores=8, trace=False):
    x = np.asarray(inputs["x"])
    n = x.shape[0]
    in_maps, n_nodes_pad, npc_dense, npc_chunks, tl, th = prepare_host(
        x, np.asarray(inputs["edge_index"]), np.asarray(inputs["W_in"]),
        np.asarray(inputs["b_in"]), np.asarray(inputs["prototypes"]),
        np.asarray(inputs["W_l"]), np.asarray(inputs["b_l"]),
        np.asarray(inputs["W_r"]), np.asarray(inputs["b_r"]),
        np.asarray(inputs["att"]), np.asarray(inputs["gat_bias"]),
        np.asarray(inputs["W_cls"]), np.asarray(inputs["b_cls"]), n_cores)
    key = (n_nodes_pad, npc_dense, npc_chunks, tl, th, n_cores)
    if key not in _CACHE:
        _CACHE[key] = build_program(*key)
    nc = _CACHE[key]
    res = run_bass_kernel_spmd(nc, in_maps, list(range(n_cores)), trace=trace)
    c_chunks = npc_chunks // P
    outs = []
    for k in range(n_cores):
        o = np.asarray(res.results[k]["out"]).reshape(P, c_chunks, OUT_DIM)
        outs.append(np.transpose(o, (1, 0, 2)).reshape(npc_chunks, OUT_DIM))
    return np.concatenate(outs, axis=0)[:n], res


def kernel(**inputs):
    out, _ = run(inputs, n_cores=8)
    return out.astype(np.float32)


# revision 54
# speedup vs baseline: 3.0960x; 1.0081x over previous
"""GATv2-based CGNN forward pass on 8 Trainium2 NeuronCores.

Strategy (dst-node sharded, no collectives):
  - Each core owns N/8 destination nodes. Host buckets edges (incl. self
    loops) by dst core, then by 128-node dst chunk within the core.
    Per-chunk tile counts are ragged (max over cores per chunk index) so
    pad work tracks the actual edge distribution.
  - Dense phase (replicated for xl over all nodes; own nodes for xr):
    xl/xr rows are [feat256 head-interleaved (c' = j*4+h), beta4, ones4]
    fp16, where beta = 0.2*(feat @ att_blk) so that
      logit = 0.8*sum_c att_c*relu(z_c) + beta_l[s] + beta_r[d]
    (exact rewrite of att . leaky_relu via |z| = 2 relu(z) - z).
  - Edge phase per 128-dst chunk: batched indirect-DMA gather of xl[src]
    rows (4 SWDGE queues); z built TRANSPOSED in PSUM ([channel, edge])
    from a one-hot matmul of resident xr plus identity-matmul transpose
    of the gathered xl; one relu Activation moves it to SBUF; per-tile
    logits come from tall-skinny matmuls with the relu'd block as lhsT.
    Messages are one fp16 tensor_tensor multiply (broadcast alpha),
    scatter-added via fp8 one-hot matmuls; denominators ride along as
    ones*alpha columns.
  - Finish: per-chunk softmax normalize + head mean (0.25 folded into
    W_cls), relu; classifier runs as a final batched phase.
"""

import os
import sys

import numpy as np
import ml_dtypes

for _p in ("/opt/trn_rl_repo",):
    if _p not in sys.path and os.path.isdir(_p):
        sys.path.insert(0, _p)

import concourse.bass as bass
import concourse.tile as tile
from concourse import bacc, mybir
from concourse.bass_utils import run_bass_kernel_spmd

FP16 = mybir.dt.float16
FP32 = mybir.dt.float32
FP8 = mybir.dt.float8e4
INT16 = mybir.dt.int16
AF = mybir.ActivationFunctionType
ALU = mybir.AluOpType

P = 128
HID = 64
HEADS = 4
OUT_DIM = 16
IN_DIM = 256
FEAT = HEADS * HID          # 256
ROW = FEAT + 2 * HEADS      # 264 = feat + beta + ones
TROW = 384                  # padded table row (768B, 256B-aligned)
LO_ROWS = 32768             # int16 index range per gather table
NB = 4                      # node tiles per dense group

f16 = ml_dtypes.float16 if hasattr(ml_dtypes, "float16") else np.float16
f8 = ml_dtypes.float8_e4m3


def _cdiv(a, b):
    return (a + b - 1) // b


# ----------------------------------------------------------------------------
# Device program
# ----------------------------------------------------------------------------

def build_program(n_nodes_pad, npc_dense, npc_chunks, tl, th, n_cores):
    """tl/th: per-chunk lo/hi gather tile counts (tuples, shared by cores)."""
    GA = n_nodes_pad // (NB * P)
    C_CHUNKS = npc_chunks // P
    OWN_STRIDE = (n_nodes_pad // P) // C_CHUNKS   # own tiles every 8th
    tl = list(tl)
    th = list(th)
    tch = [a + b for a, b in zip(tl, th)]
    TMAX = max(tch)
    loS = np.concatenate([[0], np.cumsum(tl)]).astype(int)   # tile offsets
    hiS = np.concatenate([[0], np.cumsum(th)]).astype(int)
    ohS = np.concatenate([[0], np.cumsum([2 * t for t in tch])]).astype(int)
    hi_rows = max(n_nodes_pad - LO_ROWS, P)
    xr_slots = C_CHUNKS

    nc = bacc.Bacc("TRN2", target_bir_lowering=False, debug=False,
                   num_devices=n_cores, num_swdge_queues=4)

    def din(name, shape, dtype=FP16):
        return nc.dram_tensor(name, shape, dtype, kind="ExternalInput").ap()

    xg_all = din("xg_all", [GA, P, 2, NB * P])
    w_in_a = din("w_in_a", [P, HID])
    w_in_b = din("w_in_b", [P, HID])
    b_in_c = din("b_in_c", [HID, 1])
    wql = din("wql", [HID + 1, ROW])       # [w264; b264]
    ql = din("ql", [HID, ROW])
    wqr = din("wqr", [HID + 1, ROW])
    qr = din("qr", [HID, ROW])
    att8 = din("att8", [P, 2 * HEADS])
    ident = din("ident", [P, P])
    ones6464 = din("ones6464", [HID, HID])
    o1_128 = din("o1_128", [1, P])
    gbias4 = din("gbias4", [P, HID], FP32)
    wcls4 = din("wcls4", [HID, OUT_DIM])
    bcls4 = din("bcls4", [1, NB * OUT_DIM])
    idx_lo = din("idx_lo", [P, int(loS[-1]) * 8], INT16)
    idx_hi = (din("idx_hi", [P, int(hiS[-1]) * 8], INT16)
              if hiS[-1] else None)
    ohcomb = din("ohcomb", [P, int(ohS[-1]) * P], FP8)

    out_ext = nc.dram_tensor("out", [P, C_CHUNKS * OUT_DIM], FP32,
                             kind="ExternalOutput").ap()

    xl_lo_tab = nc.dram_tensor("xl_lo_tab", [min(n_nodes_pad, LO_ROWS), TROW],
                               FP16).ap()
    xl_hi_tab = nc.dram_tensor("xl_hi_tab", [hi_rows, TROW], FP16).ap()

    qctr = [0]

    def next_q():
        qctr[0] = (qctr[0] + 1) % 4
        return qctr[0]

    with tile.TileContext(nc) as tc:
        with tc.tile_pool(name="consts", bufs=1) as cp:
            w_in_a_sb = cp.tile([P, HID], FP16)
            nc.sync.dma_start(w_in_a_sb[:], w_in_a[:])
            w_in_b_sb = cp.tile([P, HID], FP16)
            nc.sync.dma_start(w_in_b_sb[:], w_in_b[:])
            b_in_sb = cp.tile([HID, 1], FP16)
            nc.sync.dma_start(b_in_sb[:], b_in_c[:])
            wql_sb = cp.tile([HID + 1, ROW], FP16)
            nc.sync.dma_start(wql_sb[:], wql[:])
            ql_sb = cp.tile([HID, ROW], FP16)
            nc.sync.dma_start(ql_sb[:], ql[:])
            wqr_sb = cp.tile([HID + 1, ROW], FP16)
            nc.sync.dma_start(wqr_sb[:], wqr[:])
            qr_sb = cp.tile([HID, ROW], FP16)
            nc.sync.dma_start(qr_sb[:], qr[:])
            ones64_sb = cp.tile([HID, HID], FP16)
            nc.sync.dma_start(ones64_sb[:], ones6464[:])
            att_sb = cp.tile([P, 2 * HEADS], FP16)
            id_sb = cp.tile([P, P], FP16)
            o1_sb = cp.tile([1, P], FP16)
            gbias_sb = cp.tile([P, HID], FP32)
            wcls_sb = cp.tile([HID, OUT_DIM], FP16)
            bcls_sb = cp.tile([1, NB * OUT_DIM], FP16)
            idxlo_sb = cp.tile([P, int(loS[-1]) * 8], INT16)
            idxhi_sb = (cp.tile([P, int(hiS[-1]) * 8], INT16, name="idxhi_sb")
                        if idx_hi is not None else None)
            eps_sb = cp.tile([HID, 1], FP32)
            nc.gpsimd.memset(eps_sb[:], 1e-12)

            xr_res = cp.tile([P, xr_slots, ROW], FP16)
            orelu_res = cp.tile([P, C_CHUNKS, HID], FP16)
            fin_res = cp.tile([P, C_CHUNKS, OUT_DIM], FP32)

            # ---------------- dense phase ----------------
            W = NB * P

            def dense_stage1(g, xg, wq_sb, q_sb, to_table, sb, ps):
                xsb = sb.tile([P, 2, W], FP16, tag="xsb", bufs=4)
                nc.sync.dma_start(xsb[:], xg[g])
                ht_ps = ps.tile([HID, W], FP32, tag="ht_ps")
                nc.tensor.matmul(out=ht_ps[:], lhsT=w_in_a_sb[:],
                                 rhs=xsb[:, 0, :], start=True, stop=False)
                nc.tensor.matmul(out=ht_ps[:], lhsT=w_in_b_sb[:],
                                 rhs=xsb[:, 1, :], start=False, stop=True)
                hta = sb.tile([HID + 1, W], FP16, tag="hta", bufs=5)
                nc.scalar.activation(hta[0:HID, :], ht_ps[:], AF.Relu,
                                     bias=b_in_sb[:])
                nc.gpsimd.memset(hta[HID:HID + 1, :], 1.0)
                rsq = sb.tile([HID, W], FP16, tag="rsq")
                nc.vector.tensor_mul(rsq[:], hta[0:HID, :], hta[0:HID, :])
                nrm2_ps = ps.tile([HID, W], FP32, tag="nrm2_ps")
                nc.tensor.matmul(out=nrm2_ps[:], lhsT=ones64_sb[:],
                                 rhs=rsq[:], start=True, stop=True)
                return hta, nrm2_ps

            def dense_stage2(hta, nrm2_ps, sb, ps):
                nrmr = sb.tile([HID, W], FP32, tag="nrmr")
                nc.scalar.activation(nrmr[:], nrm2_ps[:], AF.Sqrt,
                                     bias=eps_sb[:])
                invr = sb.tile([HID, W], FP32, tag="invr")
                nc.vector.reciprocal(invr[:], nrmr[:])
                htn = sb.tile([HID, W], FP16, tag="htn", bufs=3)
                nc.gpsimd.tensor_mul(htn[:], hta[0:HID, :], invr[:])
                return htn

            def dense_stage3(g, wq_sb, q_sb, to_table, hta, htn, sb, ps):
                dst4 = sb.tile([P, NB, ROW], FP16, tag="dst4",
                               name="dst4", bufs=4)
                for half in range(2):
                    xl2 = ps.tile([P, 2, 2 * ROW - 16], FP32, tag="xl2")
                    for ti in range(2):
                        t = half * 2 + ti
                        tsl = slice(t * P, (t + 1) * P)
                        nc.tensor.matmul(out=xl2[:, ti, 0:ROW],
                                         lhsT=hta[:, tsl],
                                         rhs=wq_sb[:], start=True, stop=False)
                        nc.tensor.matmul(out=xl2[:, ti, 0:ROW],
                                         lhsT=htn[:, tsl],
                                         rhs=q_sb[:], start=False, stop=True)
                    mv_out = dst4[:, half * 2:half * 2 + 2, :]
                    # alternate engines for the PSUM->SBUF move
                    if half == 0:
                        nc.scalar.copy(mv_out, xl2[:, :, 0:ROW])
                    else:
                        nc.vector.tensor_copy(mv_out, xl2[:, :, 0:ROW])
                    # at most one own tile per half (own = every 8th tile)
                    for ti in range(2):
                        gt = g * NB + half * 2 + ti
                        if gt % OWN_STRIDE == 0 and gt // OWN_STRIDE < \
                                C_CHUNKS:
                            t = half * 2 + ti
                            tsl = slice(t * P, (t + 1) * P)
                            xr2 = ps.tile([P, 2, 2 * ROW - 16], FP32,
                                          tag="xl2", name="xr2")
                            nc.tensor.matmul(out=xr2[:, 0, 0:ROW],
                                             lhsT=hta[:, tsl],
                                             rhs=wqr_sb[:], start=True,
                                             stop=False)
                            nc.tensor.matmul(out=xr2[:, 0, 0:ROW],
                                             lhsT=htn[:, tsl],
                                             rhs=qr_sb[:], start=False,
                                             stop=True)
                            if half == 0:
                                nc.vector.tensor_copy(
                                    xr_res[:, gt // OWN_STRIDE, :],
                                    xr2[:, 0, 0:ROW])
                            else:
                                nc.scalar.copy(
                                    xr_res[:, gt // OWN_STRIDE, :],
                                    xr2[:, 0, 0:ROW])
                r = g * NB * P
                if r < LO_ROWS:
                    sink = xl_lo_tab[r:r + NB * P]
                else:
                    sink = xl_hi_tab[r - LO_ROWS:r - LO_ROWS + NB * P]
                nc.sync.dma_start(
                    sink.rearrange("(t p) c -> p t c", p=P)[:, :, 0:ROW],
                    dst4[:])

            with tc.tile_pool(name="dsb", bufs=3) as dsb, \
                    tc.tile_pool(name="dps", bufs=2, space="PSUM") as dps:
                specs = [(g, xg_all, wql_sb, ql_sb, True)
                         for g in range(GA)]
                NG = len(specs)
                s1out = {}
                s2out = {}
                for i in range(NG + 2):
                    if i < NG:
                        g, xg, wq_sb, q_sb, tt = specs[i]
                        s1out[i] = dense_stage1(g, xg, wq_sb, q_sb, tt,
                                                dsb, dps)
                    if 1 <= i and i - 1 < NG:
                        hta, nrm2_ps = s1out[i - 1]
                        s2out[i - 1] = dense_stage2(hta, nrm2_ps, dsb, dps)
                    if 2 <= i and i - 2 < NG:
                        g, xg, wq_sb, q_sb, tt = specs[i - 2]
                        hta, _ = s1out.pop(i - 2)
                        dense_stage3(g, wq_sb, q_sb, tt, hta,
                                     s2out.pop(i - 2), dsb, dps)

            # edge/classifier consts (emitted late so dense starts sooner)
            nc.sync.dma_start(att_sb[:], att8[:])
            nc.sync.dma_start(id_sb[:], ident[:])
            nc.sync.dma_start(o1_sb[:], o1_128[:])
            nc.sync.dma_start(gbias_sb[:], gbias4[:])
            nc.sync.dma_start(wcls_sb[:], wcls4[:])
            nc.sync.dma_start(bcls_sb[:], bcls4[:])
            nc.sync.dma_start(idxlo_sb[:], idx_lo[:])
            if idx_hi is not None:
                nc.sync.dma_start(idxhi_sb[:], idx_hi[:])

            # ---------------- edge phase ----------------
            GB_T = 8            # tiles per dma_gather call (<=1024 indices)
            ZG = 4              # tiles per z-group (PSUM bank pair)

            with tc.tile_pool(name="esb", bufs=3) as esb, \
                    tc.tile_pool(name="msb", bufs=4) as msb, \
                    tc.tile_pool(name="zps", bufs=2, space="PSUM") as zps, \
                    tc.tile_pool(name="lps", bufs=1, space="PSUM") as lps, \
                    tc.tile_pool(name="aps", bufs=2, space="PSUM") as aps:

                logits_of = {}
                xlg_of = {}
                oh_of = {}
                expv_of = {}

                def front(c):
                    T = tch[c]
                    oh_sb = esb.tile([P, 2 * TMAX * P], FP8, tag="oh", bufs=5)
                    nc.sync.dma_start(oh_sb[:, 0:2 * T * P],
                                      ohcomb[:, ohS[c] * P:ohS[c + 1] * P])
                    xlg = esb.tile([P, TMAX, TROW], FP16, tag="xlg", bufs=5)
                    segs = [(tl[c], 0, int(loS[c]), xl_lo_tab, idxlo_sb)]
                    if th[c]:
                        segs.append((th[c], tl[c], int(hiS[c]), xl_hi_tab,
                                     idxhi_sb))
                    for t_seg, off, base, tab, idx_sb_ in segs:
                        for b in range(0, t_seg, GB_T):
                            nt = min(GB_T, t_seg - b)
                            nc.gpsimd.dma_gather(
                                out_ap=xlg[:, off + b:off + b + nt, :],
                                in_ap=tab[:],
                                idxs_ap=idx_sb_[:, (base + b) * 8:
                                                (base + b + nt) * 8],
                                num_idxs=nt * P, num_idxs_reg=nt * P,
                                elem_size=TROW, queue_num=next_q())
                    logits_ps = lps.tile([P, TMAX * HEADS], FP32,
                                         tag="logits")
                    expv = esb.tile([P, TMAX * HEADS], FP16, tag="expv", bufs=4)
                    exp_done = 0
                    groups = list(range(0, T, ZG))
                    for g0 in groups:
                        gl = min(ZG, T - g0)
                        zt = zps.tile([P, 2, ZG * P], FP32, tag="zt")
                        for gi in range(gl):
                            t = g0 + gi
                            esl = slice(gi * P, (gi + 1) * P)
                            ohsl = slice(t * P, (t + 1) * P)
                            for b in range(2):
                                bsl = slice(b * P, (b + 1) * P)
                                nc.tensor.matmul(
                                    out=zt[:, b, esl],
                                    lhsT=xr_res[:, c, bsl],
                                    rhs=oh_sb[:, ohsl],
                                    start=True, stop=False)
                                nc.tensor.matmul(
                                    out=zt[:, b, esl],
                                    lhsT=xlg[:, t, bsl],
                                    rhs=id_sb[:],
                                    start=False, stop=True)
                        wt = msb.tile([P, 2, ZG * P], FP16, tag="wt",
                                      bufs=3)
                        if g0 == ZG:
                            nc.vector.tensor_scalar_max(
                                wt[:, :, 0:gl * P], zt[:, :, 0:gl * P], 0.0)
                        else:
                            nc.scalar.activation(wt[:, :, 0:gl * P],
                                                 zt[:, :, 0:gl * P], AF.Relu)
                        for gi in range(gl):
                            t = g0 + gi
                            esl = slice(gi * P, (gi + 1) * P)
                            lsl = slice(t * HEADS, (t + 1) * HEADS)
                            ohsl = slice(t * P, (t + 1) * P)
                            nc.tensor.matmul(
                                out=logits_ps[:, lsl], lhsT=id_sb[:],
                                rhs=xlg[:, t, FEAT:FEAT + HEADS],
                                start=True, stop=False)
                            nc.tensor.matmul(
                                out=logits_ps[:, lsl],
                                lhsT=oh_sb[:, ohsl],
                                rhs=xr_res[:, c, FEAT:FEAT + HEADS],
                                start=False, stop=False)
                            nc.tensor.matmul(
                                out=logits_ps[:, lsl], lhsT=wt[:, 0, esl],
                                rhs=att_sb[:, 0:HEADS],
                                start=False, stop=False)
                            nc.tensor.matmul(
                                out=logits_ps[:, lsl], lhsT=wt[:, 1, esl],
                                rhs=att_sb[:, HEADS:2 * HEADS],
                                start=False, stop=True)
                        gidx = g0 // ZG
                        done = min(g0 + gl, T)
                        if gidx % 2 == 1 or g0 == groups[-1]:
                            nc.scalar.activation(
                                expv[:, exp_done * HEADS:done * HEADS],
                                logits_ps[:, exp_done * HEADS:done * HEADS],
                                AF.Exp)
                            exp_done = done
                    xlg_of[c] = xlg
                    oh_of[c] = oh_sb
                    expv_of[c] = expv

                def back_b(c):
                    T = tch[c]
                    xlg = xlg_of.pop(c)
                    oh_sb = oh_of.pop(c)
                    expv = expv_of.pop(c)
                    agg = aps.tile([P, ROW], FP32, tag="agg")
                    JH = ROW // HEADS
                    for t0 in range(0, T, 2):
                        tn = min(2, T - t0)
                        msg = msb.tile([P, 2, ROW], FP16, tag="msg", bufs=6)
                        a4 = expv[:, t0 * HEADS:(t0 + tn) * HEADS] \
                            .rearrange("p (t o h) -> p t o h", t=tn, o=1) \
                            .to_broadcast([P, tn, JH, HEADS])
                        nc.vector.tensor_tensor(
                            msg[:, 0:tn, :].rearrange(
                                "p t (j h) -> p t j h", h=HEADS),
                            xlg[:, t0:t0 + tn, 0:ROW].rearrange(
                                "p t (j h) -> p t j h", h=HEADS),
                            a4, ALU.mult)
                        for ti in range(tn):
                            t = t0 + ti
                            nc.tensor.matmul(
                                out=agg[:],
                                lhsT=oh_sb[:, (T + t) * P:(T + t + 1) * P],
                                rhs=msg[:, ti, :], start=(t == 0),
                                stop=(t == T - 1))
                    den = msb.tile([P, HEADS], FP32, tag="den")
                    nc.vector.tensor_scalar_add(den[:], agg[:, ROW - HEADS:],
                                                1e-16)
                    dinv = msb.tile([P, HEADS], FP32, tag="dinv")
                    nc.vector.reciprocal(dinv[:], den[:])
                    scl = msb.tile([P, FEAT], FP32, tag="scl")
                    di4 = dinv[:].rearrange("p (o h) -> p o h", o=1) \
                        .to_broadcast([P, HID, HEADS])
                    nc.vector.tensor_tensor(
                        scl[:].rearrange("p (j h) -> p j h", h=HEADS),
                        agg[:, 0:FEAT].rearrange("p (j h) -> p j h", h=HEADS),
                        di4, ALU.mult)
                    ored = msb.tile([P, HID], FP32, tag="ored")
                    nc.vector.tensor_reduce(
                        out=ored[:],
                        in_=scl[:].rearrange("p (j h) -> p j h", h=HEADS),
                        axis=mybir.AxisListType.X, op=ALU.add)
                    obias = msb.tile([P, HID], FP32, tag="obias")
                    nc.vector.tensor_add(obias[:], ored[:], gbias_sb[:])
                    nc.vector.tensor_scalar_max(orelu_res[:, c, :],
                                                obias[:], 0.0)

                def classify(c0, cl, fsb, fps):
                    # one PSUM bank: ot at fp16 cols 0:256, fin as fp32
                    # view of fp16 cols 512:576
                    cls_ps = fps.tile([P, 1024], FP16, tag="cls_ps")
                    for ci in range(cl):
                        nc.tensor.transpose(
                            out=cls_ps[0:HID, ci * P:(ci + 1) * P],
                            in_=orelu_res[:, c0 + ci, :], identity=id_sb[:])
                    ot16 = fsb.tile([HID, 2 * P], FP16, tag="ot16")
                    nc.scalar.copy(ot16[:, 0:cl * P],
                                   cls_ps[0:HID, 0:cl * P])
                    fin_ps = cls_ps[:, 512:576].bitcast(FP32)
                    for ci in range(cl):
                        nc.tensor.matmul(
                            out=fin_ps[:, ci * OUT_DIM:(ci + 1) * OUT_DIM],
                            lhsT=ot16[:, ci * P:(ci + 1) * P],
                            rhs=wcls_sb[:], start=True, stop=False)
                        nc.tensor.matmul(
                            out=fin_ps[:, ci * OUT_DIM:(ci + 1) * OUT_DIM],
                            lhsT=o1_sb[:],
                            rhs=bcls_sb[:, ci * OUT_DIM:(ci + 1) * OUT_DIM],
                            start=False, stop=True)
                    nc.vector.tensor_copy(
                        fin_res[:, c0:c0 + cl, :].rearrange(
                            "p c o -> p (c o)"),
                        fin_ps[:, 0:cl * OUT_DIM])

                with tc.tile_pool(name="fsb", bufs=2) as fsb, \
                        tc.tile_pool(name="fps", bufs=1,
                                     space="PSUM") as fps2:
                    front(0)
                    if C_CHUNKS > 1:
                        front(1)
                    for c in range(C_CHUNKS):
                        if c + 2 < C_CHUNKS:
                            front(c + 2)
                        back_b(c)
                        if c % 2 == 1:
                            classify(c - 1, 2, fsb, fps2)
                        if c % 8 == 7:
                            nc.sync.dma_start(
                                out_ext[:, (c - 7) * OUT_DIM:
                                        (c + 1) * OUT_DIM].rearrange(
                                    "p (c o) -> p c o", o=OUT_DIM),
                                fin_res[:, c - 7:c + 1, :])
                    if C_CHUNKS % 2:
                        classify(C_CHUNKS - 1, 1, fsb, fps2)
                    rem0 = (C_CHUNKS // 8) * 8
                    if rem0 < C_CHUNKS:
                        nc.sync.dma_start(
                            out_ext[:, rem0 * OUT_DIM:].rearrange(
                                "p (c o) -> p c o", o=OUT_DIM),
                            fin_res[:, rem0:, :])

    nc.compile()
    return nc


# ----------------------------------------------------------------------------
# Host-side data preparation
# ----------------------------------------------------------------------------

def prepare_host(x, edge_index, W_in, b_in, prototypes, W_l, b_l, W_r, b_r,
                 att, gat_bias, W_cls, b_cls, n_cores):
    x = np.asarray(x, np.float32)
    W_in = np.asarray(W_in, np.float32)
    b_in = np.asarray(b_in, np.float32)
    prototypes = np.asarray(prototypes, np.float32)
    W_l = np.asarray(W_l, np.float32)
    b_l = np.asarray(b_l, np.float32)
    W_r = np.asarray(W_r, np.float32)
    b_r = np.asarray(b_r, np.float32)
    att = np.asarray(att, np.float32)
    gat_bias = np.asarray(gat_bias, np.float32)
    W_cls = np.asarray(W_cls, np.float32)
    b_cls = np.asarray(b_cls, np.float32)

    n = x.shape[0]
    NB4 = NB * P

    n_nodes_pad = _cdiv(n, NB4) * NB4
    npc_chunks = _cdiv(_cdiv(n, n_cores), P) * P
    nodes_per_core = npc_chunks        # 128-aligned ownership
    npc_dense = npc_chunks
    c_chunks = npc_chunks // P
    NT = n_nodes_pad // P
    STRIDE8 = NT // c_chunks
    # per-core tile permutation: rotated pos 8j holds own tile (49k+j)
    perms = []
    invs = []
    for k in range(n_cores):
        own = c_chunks * k + np.arange(c_chunks)
        foreign = np.setdiff1d(np.arange(NT), own)
        perm = np.empty(NT, np.int64)
        perm[np.arange(c_chunks) * STRIDE8] = own
        mask = np.ones(NT, bool)
        mask[np.arange(c_chunks) * STRIDE8] = False
        perm[mask] = foreign
        inv = np.empty(NT, np.int64)
        inv[perm] = np.arange(NT)
        perms.append(perm)
        invs.append(inv)
    inv_all = np.stack(invs)          # [k, NT]

    # --- edge bucketing ---
    src = np.asarray(edge_index[0], dtype=np.int64)
    dst = np.asarray(edge_index[1], dtype=np.int64)
    loop = np.arange(n, dtype=np.int64)
    src = np.concatenate([src, loop])
    dst = np.concatenate([dst, loop])

    core = dst // nodes_per_core
    dstl = dst - core * nodes_per_core
    chunk = dstl // P
    src_rot = inv_all[core, src // P] * P + src % P
    seg = (src_rot >= LO_ROWS).astype(np.int64)

    counts = np.zeros((n_cores, c_chunks, 2), dtype=np.int64)
    np.add.at(counts, (core, chunk, seg), 1)
    # ragged per-chunk tile counts: max over cores
    tl = [int(_cdiv(int(counts[:, c, 0].max()), P)) for c in range(c_chunks)]
    th = [int(_cdiv(int(counts[:, c, 1].max()), P)) for c in range(c_chunks)]
    tl = [max(t, 1) for t in tl]
    tch = [a + b for a, b in zip(tl, th)]
    loS = np.concatenate([[0], np.cumsum(tl)]).astype(np.int64)
    hiS = np.concatenate([[0], np.cumsum(th)]).astype(np.int64)
    ohS = np.concatenate([[0], np.cumsum([2 * t for t in tch])]) \
        .astype(np.int64)

    order = np.lexsort((seg, chunk, core))
    src_o, core_o, chunk_o, dstl_o, seg_o = (src[order], core[order],
                                             chunk[order], dstl[order],
                                             seg[order])
    bounds = np.zeros(n_cores * c_chunks * 2 + 1, dtype=np.int64)
    np.cumsum(counts.reshape(-1), out=bounds[1:])
    flat_bucket = (core_o * c_chunks + chunk_o) * 2 + seg_o
    pos = np.arange(len(src_o)) - bounds[flat_bucket]

    lo_slots = int(loS[-1]) * P
    hi_slots = int(hiS[-1]) * P
    idx_lo_slot = np.zeros((n_cores, lo_slots), dtype=np.int32)
    idx_hi_slot = np.zeros((n_cores, max(hi_slots, 1)), dtype=np.int32)
    # nloc in per-chunk tile space for one-hot build
    nloc_lo = np.full((n_cores, lo_slots), -1, dtype=np.int32)
    nloc_hi = np.full((n_cores, max(hi_slots, 1)), -1, dtype=np.int32)

    lo_base = loS[chunk_o] * P + pos
    hi_base = hiS[chunk_o] * P + pos
    is_lo = seg_o == 0
    srcrot_o = src_rot[order]
    idx_lo_slot[core_o[is_lo], lo_base[is_lo]] = \
        srcrot_o[is_lo].astype(np.int32)
    nloc_lo[core_o[is_lo], lo_base[is_lo]] = \
        (dstl_o[is_lo] - chunk_o[is_lo] * P).astype(np.int32)
    is_hi = ~is_lo
    idx_hi_slot[core_o[is_hi], hi_base[is_hi]] = \
        (srcrot_o[is_hi] - LO_ROWS).astype(np.int32)
    nloc_hi[core_o[is_hi], hi_base[is_hi]] = \
        (dstl_o[is_hi] - chunk_o[is_hi] * P).astype(np.int32)

    def wrap16(vals):
        # [k, S*128] -> [k, 128, S*8] int16
        S = vals.shape[1] // P
        v = vals.reshape(n_cores, S * 8, 16)
        v = np.transpose(v, (0, 2, 1))
        v = np.tile(v, (1, 8, 1))
        return np.ascontiguousarray(v).astype(np.int16)

    idx_lo = wrap16(idx_lo_slot)
    idx_hi = wrap16(idx_hi_slot) if hi_slots else None

    # --- one-hot (fp8), ragged layout ---
    iota = np.arange(P, dtype=np.int32)
    ohcomb = np.zeros((n_cores, P, int(ohS[-1]) * P), dtype=f8)
    for c in range(c_chunks):
        T = tch[c]
        nl = np.concatenate(
            [nloc_lo[:, loS[c] * P:loS[c + 1] * P],
             nloc_hi[:, hiS[c] * P:hiS[c + 1] * P]], axis=1) \
            .reshape(n_cores, T, P)
        oh = (nl[..., None] == iota)                 # [k, t, e, n]
        base = int(ohS[c]) * P
        ohcomb[:, :, base:base + T * P] = np.transpose(oh, (0, 3, 1, 2)) \
            .reshape(n_cores, P, T * P)
        ohcomb[:, :, base + T * P:base + 2 * T * P] = \
            np.transpose(oh, (0, 2, 1, 3)).reshape(n_cores, P, T * P)

    # --- weights ---
    att_blk = np.zeros((FEAT, HEADS), dtype=np.float32)
    for h in range(HEADS):
        att_blk[h * HID:(h + 1) * HID, h] = att[h]
    p_norm = prototypes / (np.linalg.norm(prototypes, axis=1, keepdims=True)
                           + 1e-12)
    Q_l = p_norm.T @ W_l[HID:HID + 2]
    Q_r = p_norm.T @ W_r[HID:HID + 2]

    perm = np.zeros(FEAT, np.int64)
    for h in range(HEADS):
        for j in range(HID):
            perm[j * HEADS + h] = h * HID + j

    def ext264(w, b, with_ones):
        w264 = np.concatenate(
            [w[:, perm], 0.2 * (w @ att_blk),
             np.zeros((w.shape[0], HEADS), np.float32)], axis=1)
        ones = np.ones(HEADS, np.float32) if with_ones else \
            np.zeros(HEADS, np.float32)
        b264 = np.concatenate([b[perm], 0.2 * (b @ att_blk), ones])[None, :]
        return w264, b264

    wl264, bl264 = ext264(W_l[:HID], b_l, True)
    ql264, _ = ext264(Q_l, b_l * 0, False)
    wr264, br264 = ext264(W_r[:HID], b_r, True)
    qr264, _ = ext264(Q_r, b_r * 0, False)
    wql_ = np.concatenate([wl264, bl264], axis=0).astype(f16)
    wqr_ = np.concatenate([wr264, br264], axis=0).astype(f16)

    att8 = np.zeros((P, 2 * HEADS), np.float32)
    for b in range(2):
        for p in range(P):
            cprime = b * P + p
            j, h = cprime // HEADS, cprime % HEADS
            att8[p, b * HEADS + h] = 0.8 * att[h, j]

    def swizzle(xa, npad):
        G = npad // NB4
        xp = np.zeros((npad, IN_DIM), dtype=np.float32)
        xp[:len(xa)] = xa
        v = xp.reshape(G, NB, P, 2, P)
        v = np.transpose(v, (0, 4, 3, 1, 2))
        return np.ascontiguousarray(v.reshape(G, P, 2, NB * P)).astype(f16)

    xp_t = np.zeros((n_nodes_pad, IN_DIM), np.float32)
    xp_t[:n] = x
    xp_t = xp_t.reshape(NT, P, IN_DIM)
    xg_rot = [swizzle(xp_t[perms[k]].reshape(n_nodes_pad, IN_DIM),
                      n_nodes_pad) for k in range(n_cores)]

    shared = {
        "w_in_a": W_in[:P].astype(f16), "w_in_b": W_in[P:].astype(f16),
        "b_in_c": b_in[:, None].astype(f16),
        "wql": wql_, "ql": ql264.astype(f16),
        "wqr": wqr_, "qr": qr264.astype(f16),
        "att8": att8.astype(f16),
        "ident": np.eye(P, dtype=f16),
        "ones6464": np.ones((HID, HID), f16),
        "o1_128": np.ones((1, P), f16),
        "gbias4": np.broadcast_to(4.0 * gat_bias.astype(np.float32),
                                  (P, HID)).copy(),
        "wcls4": (0.25 * W_cls).astype(f16),
        "bcls4": np.tile(b_cls, NB)[None, :].astype(f16),
    }
    in_maps = []
    for k in range(n_cores):
        m = dict(shared)
        m["xg_all"] = xg_rot[k]
        m["idx_lo"] = idx_lo[k]
        if idx_hi is not None:
            m["idx_hi"] = idx_hi[k]
        m["ohcomb"] = ohcomb[k]
        in_maps.append(m)
    return (in_maps, n_nodes_pad, npc_dense, npc_chunks, tuple(tl),
            tuple(th))


_CACHE = {}


def run(inputs, n_cores=8, trace=False):
    x = np.asarray(inputs["x"])
    n = x.shape[0]
    in_maps, n_nodes_pad, npc_dense, npc_chunks, tl, th = prepare_host(
        x, np.asarray(inputs["edge_index"]), np.asarray(inputs["W_in"]),
        np.asarray(inputs["b_in"]), np.asarray(inputs["prototypes"]),
        np.asarray(inputs["W_l"]), np.asarray(inputs["b_l"]),
        np.asarray(inputs["W_r"]), np.asarray(inputs["b_r"]),
        np.asarray(inputs["att"]), np.asarray(inputs["gat_bias"]),
        np.asarray(inputs["W_cls"]), np.asarray(inputs["b_cls"]), n_cores)
    key = (n_nodes_pad, npc_dense, npc_chunks, tl, th, n_cores)
    if key not in _CACHE:
        _CACHE[key] = build_program(*key)
    nc = _CACHE[key]
    res = run_bass_kernel_spmd(nc, in_maps, list(range(n_cores)), trace=trace)
    c_chunks = npc_chunks // P
    outs = []
    for k in range(n_cores):
        o = np.asarray(res.results[k]["out"]).reshape(P, c_chunks, OUT_DIM)
        outs.append(np.transpose(o, (1, 0, 2)).reshape(npc_chunks, OUT_DIM))
    return np.concatenate(outs, axis=0)[:n], res


def kernel(**inputs):
    out, _ = run(inputs, n_cores=8)
    return out.astype(np.float32)
